# revision 1
# baseline (speedup 1.0000x reference)
"""Trainium2 Bass kernel for nn_DocREModel (DocRE: gather -> RGCN -> SE -> 5x5 convs).

Sharding: 4 documents x 2 cores each. Each pair replicates the cheap upstream
(mention/link/ea gathers -> RGCN -> fmap/SE) and splits the dominant 5x5 conv
stack by output channels, with two intra-pair AllGathers; output halves are
assembled on host. All index-driven gathers happen on host (pure data
movement; one SPMD program serves all 8 cores), all dense math on device.

Precision/layout choices:
- float32r (TF32-mode, full PE rate at moving free-dim >= 256) for the f32
  path; bf16 weights+activations for the RGCN and conv stack (halves the
  dominant weight DMA), f32 PSUM accumulation throughout.
- Convs are 25 shift-tap matmuls over zero-padded 26x26 images via strided
  APs (no im2col copies). conv2/conv3 start on the locally-computed input
  half before the pair AllGather completes; the other half is extracted
  SPMD-safely with host-supplied 0/1 masks and per-core (own, other)
  weight-chunk ordering.
- RGCN folds the self-loop in as a 4th identity relation so each layer is
  one u = h^T @ [A0^T|A1^T|A2^T|I] matmul plus one PSUM accumulation over
  stacked (relation, chunk) weights -- no transposes in the loop.
- Host packs ~70 small constant/weight/activation tensors into a few large
  DMA-friendly tensors, ordered by first use.
"""

import numpy as np
import ml_dtypes

import concourse.bacc as bacc
import concourse.tile as tile
from concourse import mybir
from concourse.bass_utils import run_bass_kernel_spmd
from concourse.masks import make_identity

F32 = mybir.dt.float32
F32R = mybir.dt.float32r
BF16 = mybir.dt.bfloat16
AF = mybir.ActivationFunctionType
ALU = mybir.AluOpType

NB, H, C, HID, EMB = 4, 12, 1024, 768, 512
E, M, L, SPAN = 22, 4, 16, 32
TD, INTER = 20, 256
NN = E + E * M + L
NREL, NLAYERS = 3, 4
EM, EMH, HS, LS = E * M, E * M * H, H * SPAN, L * SPAN
D0 = EMB + TD           # 532
EE = E * E              # 484
PADW = 26 * 26          # 676 padded 26x26 image
N_CORES = 8


def _build_adj():
    A = np.zeros((NREL, NN, NN), np.float32)
    for e in range(E):
        for m in range(M):
            mi = E + e * M + m
            A[0, e, mi] = A[0, mi, e] = 1.0
            for m2 in range(M):
                if m2 != m:
                    A[1, mi, E + e * M + m2] = 1.0
            li = E + E * M + ((e * M + m) % L)
            A[2, mi, li] = A[2, li, mi] = 1.0
    A = A / (A.sum(-1, keepdims=True) + 1e-5)
    return A


_TYPES = np.concatenate([np.zeros(E, np.int32), np.ones(EM, np.int32),
                         np.full(L, 2, np.int32)])

_KC0 = [(0, 128), (128, 128), (256, 128), (384, 128), (512, 20)]   # 532 rows
_KC1 = [(0, 128), (128, 128), (256, 128), (384, 128)]              # 512 rows


def _const_layout():
    """Column layout of the packed f32r constant tensor [128, CR]."""
    lay = {}
    c = 0

    def add(nm, cols):
        nonlocal c
        lay[nm] = (c, cols)
        c += cols
    for kc in range(6):
        add(f"wtr{kc}", EMB)
    add("brow", EMB)
    add("onescol", 1)
    add("onesrow", 128)
    add("g2T", E)
    for kc in range(4):
        add(f"sumT{kc}", L)
    add("aallT", NREL * NN)
    add("tfeat", TD)
    for kc in range(4):
        add(f"fsw1T{kc}", INTER)
    for kc in range(4):
        add(f"fcw1T{kc}", INTER)
    for kc in range(2):
        add(f"fsw2T{kc}", EMB)
    for kc in range(2):
        add(f"fcw2T{kc}", EMB)
    for kc in range(9):
        add(f"gT{kc}", E)
    return lay, c


def _constf_layout():
    lay = {}
    c = 0

    def add(nm, cols):
        nonlocal c
        lay[nm] = (c, cols)
        c += cols
    for nm, nch in (("ses1", 2), ("seb1", 2), ("fcs1", 2), ("fcb1", 2),
                    ("ses2", 4), ("seb2", 4), ("fcs2", 4), ("fcb2", 4)):
        for kc in range(nch):
            add(f"{nm}{kc}", 1)
    add("b1h", 1)
    add("b2h", 1)
    add("b3h0", 1)
    add("b3h1", 1)
    add("mtop", 1)
    add("mbot", 1)
    add("identf", 128)
    return lay, c


def _actr_layout():
    lay = {}
    c = 0

    def add(nm, cols):
        nonlocal c
        lay[nm] = (c, cols)
        c += cols
    for kc in range(6):
        add(f"xmT{kc}", EM)
    for kc in range(6):
        add(f"xspT{kc}", LS)
    for kc in range(3):
        add(f"attl{kc}", LS)
    return lay, c


_LAY_R, _CR = _const_layout()
_LAY_F, _CF = _constf_layout()
_LAY_A, _CA = _actr_layout()


def build_program(solo=False, stages=4):
    nc = bacc.Bacc("TRN2", target_bir_lowering=False, debug=False)

    def din(name, shape, dt=F32R):
        return nc.dram_tensor(name, list(shape), dt, kind="ExternalInput").ap()

    # packed inputs (see _const_layout/_constf_layout/_actr_layout)
    constr_d = din("constr", [128, _CR])
    constf_d = din("constf", [128, _CF], F32)
    actr_d = din("actr", [128, _CA])
    xp_d = din("xp", [128, 8 * HID], BF16)
    amp_d = din("amp", [128, 9 * C], BF16)
    gTb_d = din("gTb", [128, 9 * E], BF16)
    wstp_d = [din("wstp0", [128, 20 * EMB], BF16)] + \
             [din(f"wstp{i}", [128, 16 * EMB], BF16) for i in (1, 2, 3)]
    w1sb_d = din("w1sb", [4, 128, 25 * 128], BF16)
    w2sb_d = din("w2sb", [2, 128, 25 * 128], BF16)
    w3sb_d = din("w3sb", [2, 128, 25 * 256], BF16)
    aallTb_d = din("aallTb", [NN, (NREL + 1) * NN], BF16)
    tfb_d = din("tfb", [NN, TD])
    identb_d = din("identb", [128, 128], BF16)

    out_d = nc.dram_tensor("out", [256, EE], F32, kind="ExternalOutput").ap()

    groups = [[0, 1], [2, 3], [4, 5], [6, 7]]

    with tile.TileContext(nc) as tc:
      with tc.tile_pool(name="pconst", bufs=1) as pconst, \
           tc.tile_pool(name="pwork", bufs=1) as pwork, \
           tc.tile_pool(name="pdram", bufs=1, space="DRAM") as pdram:

        constr = pconst.tile([128, _CR], F32R)
        constf = pconst.tile([128, _CF], F32)
        identb = pconst.tile([128, 128], BF16)
        aallTb = pconst.tile([NN, (NREL + 1) * NN], BF16)

        def cr(nm, rows=128):
            c0, cols = _LAY_R[nm]
            return constr[0:rows, c0:c0 + cols]

        def cf(nm, rows=128):
            c0, cols = _LAY_F[nm]
            return constf[0:rows, c0:c0 + cols]

        wtr = [cr(f"wtr{kc}") for kc in range(6)]
        brow = cr("brow", rows=1)
        onescol = cr("onescol")
        onesrow = cr("onesrow", rows=1)
        g2T = cr("g2T", rows=EM)
        sumT = [cr(f"sumT{kc}") for kc in range(4)]
        aallT = cr("aallT", rows=NN)
        sew = {nm: [cr(f"{nm}{kc}") for kc in range(n)]
               for nm, n in (("fsw1T", 4), ("fcw1T", 4), ("fsw2T", 2),
                             ("fcw2T", 2))}
        sev = {nm: [cf(f"{nm}{kc}") for kc in range(n)]
               for nm, n in (("ses1", 2), ("seb1", 2), ("fcs1", 2), ("fcb1", 2),
                             ("ses2", 4), ("seb2", 4), ("fcs2", 4),
                             ("fcb2", 4))}
        b1h = cf("b1h")
        b2h = cf("b2h")
        b3h = [cf("b3h0"), cf("b3h1")]
        ident = cf("identf")

        # persistent intermediates
        h0 = pwork.tile([NN, D0], F32R)
        nc.scalar.dma_start(h0[:, EMB:D0], tfb_d[:])
        ectxT_sb = [pwork.tile([128, E], F32, tag=f"ectxT{i}", name=f"ectxT{i}")
                    for i in range(4)]

        # ================= stage 1: gathered-row transforms =================
        with tc.tile_pool(name="pbig", bufs=1) as pbig:
            actr = pbig.tile([128, _CA], F32R)

            def ca(nm, rows=128):
                c0, cols = _LAY_A[nm]
                return actr[0:rows, c0:c0 + cols]

            xmT = [ca(f"xmT{kc}") for kc in range(6)]
            xspT = [ca(f"xspT{kc}") for kc in range(6)]
            attl = [ca(f"attl{kc}") for kc in range(3)]

            xp = pbig.tile([128, 8 * HID], BF16)
            gTb = pbig.tile([128, 9 * E], BF16)
            nc.sync.dma_start(constf[:], constf_d[:])
            nc.scalar.dma_start(gTb[:], gTb_d[:])

            expm = pbig.tile([EM, EMB], F32R)
            sp_ps = []
            wsb = [pbig.tile([128, 1], F32, tag=f"wsb{i}", name=f"wsb{i}")
                   for i in range(4)]
            wsp = [pbig.tile([128, EMB], F32R, tag=f"wsp{i}", name=f"wsp{i}")
                   for i in range(4)]
            ea_sb = pbig.tile([E, C], F32R)
            eaT = [pbig.tile([128, E], BF16, tag=f"eaT{i}", name=f"eaT{i}")
                   for i in range(8)]
            z_sb = [pbig.tile([128, E], F32R, tag=f"z{i}", name=f"z{i}")
                    for i in range(6)]
            easumT = pbig.tile([1, E], F32R)

            with tc.tile_pool(name="ps1b", bufs=1, space="PSUM") as ps1b:
                # ea = G^T @ attm ; normalize rows (attm/gT streamed)
                ea_p0 = ps1b.tile([E, 512], F32, tag="ea0", name="ea0")
                ea_p1 = ps1b.tile([E, 512], F32, tag="ea1", name="ea1")
                ampt = []
                for g in range(3):
                    t = pbig.tile([128, 3 * C], BF16, tag=f"amp{g}",
                                  name=f"amp{g}")
                    nc.gpsimd.dma_start(t[:],
                                        amp_d[:, g * 3 * C:(g + 1) * 3 * C])
                    ampt.append(t)
                for kc in range(9):
                    rows = 128 if kc < 8 else 32
                    at = ampt[kc // 3][0:rows, (kc % 3) * C:(kc % 3) * C + C]
                    gt = gTb[0:rows, kc * E:(kc + 1) * E]
                    nc.tensor.matmul(ea_p0[:], gt, at[:, 0:512],
                                     start=(kc == 0), stop=(kc == 8))
                    nc.tensor.matmul(ea_p1[:], gt, at[:, 512:1024],
                                     start=(kc == 0), stop=(kc == 8))
                r0 = pbig.tile([E, 1], F32)
                r1 = pbig.tile([E, 1], F32)
                nc.vector.tensor_reduce(r0[:], ea_p0[:], mybir.AxisListType.X,
                                        ALU.add)
                nc.vector.tensor_reduce(r1[:], ea_p1[:], mybir.AxisListType.X,
                                        ALU.add)
                rsum = pbig.tile([E, 1], F32)
                nc.vector.tensor_tensor(out=rsum[:], in0=r0[:], in1=r1[:],
                                        op=ALU.add)
                rsum2 = pbig.tile([E, 1], F32)
                nc.vector.tensor_scalar(out=rsum2[:], in0=rsum[:], scalar1=1e-5,
                                        scalar2=None, op0=ALU.add)
                rinv = pbig.tile([E, 1], F32)
                nc.vector.reciprocal(rinv[:], rsum2[:])
                nc.scalar.activation(ea_sb[:, 0:512], ea_p0[:], AF.Copy,
                                     scale=rinv[:])
                nc.scalar.activation(ea_sb[:, 512:1024], ea_p1[:], AF.Copy,
                                     scale=rinv[:])
                easum = pbig.tile([E, 1], F32)
                nc.vector.tensor_tensor(out=easum[:], in0=rsum[:], in1=rinv[:],
                                        op=ALU.mult)
                for kc in range(8):
                    tp = ps1b.tile([128, E], F32, tag="eaTt", name="eaTt", bufs=2)
                    nc.tensor.transpose(tp[:],
                                        ea_sb[:, kc * 128:(kc + 1) * 128]
                                        .bitcast(F32), ident[0:E, 0:E])
                    if kc % 2 == 0:
                        nc.scalar.copy(eaT[kc][:], tp[:])
                    else:
                        nc.vector.tensor_copy(out=eaT[kc][:], in_=tp[:])
                tp = ps1b.tile([1, E], F32, tag="easumt", name="easumt")
                nc.tensor.transpose(tp[:], easum[:], ident[0:E, 0:E])
                nc.scalar.copy(easumT[:], tp[:])


            nc.sync.dma_start(constr[:, 0:4197], constr_d[:, 0:4197])
            wstp0t = pconst.tile([128, 20 * EMB], BF16)
            nc.sync.dma_start(wstp0t[:], wstp_d[0][:])
            nc.scalar.dma_start(identb[:], identb_d[:])
            nc.scalar.dma_start(aallTb[:], aallTb_d[:])
            nc.gpsimd.dma_start(xp[:], xp_d[:])
            with tc.tile_pool(name="ps1c", bufs=1, space="PSUM") as ps1c:
                # zT = ea_n @ x  [22, 768] (two 384-wide halves)
                zt_ps = [ps1c.tile([E, 384], F32, tag=f"zt_p{i}",
                                   name=f"zt_p{i}") for i in range(2)]
                for kc in range(8):
                    xt = xp[:, kc * HID:(kc + 1) * HID]
                    for hh in range(2):
                        nc.tensor.matmul(zt_ps[hh][:], eaT[kc][:],
                                         xt[:, hh * 384:(hh + 1) * 384],
                                         start=(kc == 0), stop=(kc == 7))
                zt_sb = pbig.tile([E, HID], F32)
                nc.scalar.copy(zt_sb[:, 0:384], zt_ps[0][:])
                nc.scalar.copy(zt_sb[:, 384:768], zt_ps[1][:])
                # z chunks [128, 22] via transposes
                for kc in range(6):
                    ztp = ps1c.tile([128, E], F32, tag="ztp", name="ztp", bufs=2)
                    nc.tensor.transpose(ztp[:],
                                        zt_sb[:, kc * 128:(kc + 1) * 128],
                                        ident[0:E, 0:E])
                    if kc % 2 == 0:
                        nc.scalar.copy(z_sb[kc][:], ztp[:])
                    else:
                        nc.vector.tensor_copy(out=z_sb[kc][:], in_=ztp[:])
                # ecT2 = z^T-chunks as lhsT @ Wtr -> [22, 512], + b (x) easum
                ec2_p = ps1c.tile([E, EMB], F32, tag="ec2", name="ec2")
                for kc in range(6):
                    nc.tensor.matmul(ec2_p[:], z_sb[kc][:], wtr[kc][:],
                                     start=(kc == 0), stop=False)
                nc.tensor.matmul(ec2_p[:], easumT[:], brow[:],
                                 start=False, stop=True)
                ec2_sb = pbig.tile([E, EMB], F32)
                nc.scalar.copy(ec2_sb[:], ec2_p[:])
                # transpose to ectxT chunks [128, 22]
                for mc in range(4):
                    ecp = ps1c.tile([128, E], F32, tag="ecp", name="ecp", bufs=2)
                    nc.tensor.transpose(ecp[:],
                                        ec2_sb[:, mc * 128:(mc + 1) * 128],
                                        ident[0:E, 0:E])
                    if mc % 2 == 0:
                        nc.scalar.copy(ectxT_sb[mc][:], ecp[:])
                    else:
                        nc.vector.tensor_copy(out=ectxT_sb[mc][:], in_=ecp[:])


            nc.scalar.dma_start(actr[:], actr_d[:])
            with tc.tile_pool(name="ps1a", bufs=1, space="PSUM") as ps1a:
                # mentions: mrep = x_m @ Wtr + b
                mrep_p = ps1a.tile([EM, EMB], F32, tag="mrep", name="mrep")
                for kc in range(6):
                    nc.tensor.matmul(mrep_p[:], xmT[kc][:, 0:EM], wtr[kc][:],
                                     start=(kc == 0), stop=False)
                nc.tensor.matmul(mrep_p[:], onesrow[0:1, 0:EM], brow[:],
                                 start=False, stop=True)
                mrep_sb = pbig.tile([EM, EMB], F32R)
                nc.scalar.copy(mrep_sb[:], mrep_p[:])
                nc.sync.dma_start(h0[E:E + EM, 0:EMB], mrep_sb[:])
                nc.scalar.activation(expm[:], mrep_p[:], AF.Exp)
                # e_rep = ln(G2 @ exp(mrep))
                ep_p = ps1a.tile([E, EMB], F32, tag="ep", name="ep")
                nc.tensor.matmul(ep_p[:], g2T[:], expm[:], start=True, stop=True)
                nc.scalar.activation(h0[0:E, 0:EMB], ep_p[:], AF.Ln)

                # spans: sp = x_span @ Wtr + b
                for mc in range(4):
                    sp_p = ps1a.tile([128, EMB], F32, tag="sp_p", name="sp_p",
                                     bufs=4)
                    for kc in range(6):
                        nc.tensor.matmul(sp_p[:],
                                         xspT[kc][:, mc * 128:(mc + 1) * 128],
                                         wtr[kc][:], start=(kc == 0), stop=False)
                    nc.tensor.matmul(sp_p[:], onesrow[:], brow[:],
                                     start=False, stop=True)
                    spc = pbig.tile([128, EMB], F32, tag="spc", name="spc",
                                    bufs=4)
                    nc.scalar.copy(spc[:], sp_p[:])
                    sp_ps.append(spc)
                # w = colsum(attl) / 384
                for mc in range(4):
                    w_p = ps1a.tile([128, 1], F32, tag="w_p", name="w_p", bufs=1)
                    for kc in range(3):
                        nc.tensor.matmul(w_p[:],
                                         attl[kc][:, mc * 128:(mc + 1) * 128]
                                         .bitcast(F32),
                                         onescol[:].bitcast(F32),
                                         start=(kc == 0), stop=(kc == 2))
                    nc.scalar.activation(wsb[mc][:], w_p[:], AF.Copy,
                                         scale=1.0 / (H * SPAN))
                # wsp = psum(sp) * w ; link = SUM^T @ wsp
                for mc in range(4):
                    nc.vector.tensor_scalar(out=wsp[mc][:], in0=sp_ps[mc][:],
                                            scalar1=wsb[mc][:], scalar2=None,
                                            op0=ALU.mult)
                link_p = ps1a.tile([L, EMB], F32, tag="link", name="link")
                for kc in range(4):
                    nc.tensor.matmul(link_p[:], sumT[kc][:], wsp[kc][:],
                                     start=(kc == 0), stop=(kc == 3))
                link_sb = pbig.tile([L, EMB], F32R)
                nc.scalar.copy(link_sb[:], link_p[:])
                nc.sync.dma_start(h0[E + EM:NN, 0:EMB], link_sb[:])


            nc.scalar.dma_start(constr[:, 4197:_CR], constr_d[:, 4197:_CR])

        h0b = pwork.tile([NN, D0], BF16)
        nc.vector.tensor_copy(out=h0b[:], in_=h0[:])

        if stages >= 2:
          # ================= stage 2: RGCN (4 layers) =================
          ecT = [pwork.tile([128, E], F32R, tag=f"ecT{i}", name=f"ecT{i}")
                 for i in range(4)]
          with tc.tile_pool(name="prgw", bufs=1) as prgw, \
               tc.tile_pool(name="prg", bufs=2) as prg, \
               tc.tile_pool(name="psr", bufs=1, space="PSUM") as psr:
              h = h0b
              wstp_t = [wstp0t]
              for layer in range(1, NLAYERS):
                  nk = len(_KC1)
                  t = prgw.tile([128, 16 * EMB], BF16, tag=f"wstp{layer}",
                                name=f"wstp{layer}")
                  nc.sync.dma_start(t[:, 0:(NREL + 1) * nk * EMB],
                                    wstp_d[layer][:, 0:(NREL + 1) * nk * EMB])
                  wstp_t.append(t)
              for layer in range(NLAYERS):
                  din_l = D0 if layer == 0 else EMB
                  kcs = _KC0 if layer == 0 else _KC1
                  nk = len(kcs)
                  wstp = wstp_t[layer]
                  wst_t = [wstp[:, (r * nk + si) * EMB:(r * nk + si + 1) * EMB]
                           for r in range(NREL + 1) for si in range(nk)]
                  # u = h^T @ A_allT per d-chunk
                  u_sb = []
                  for si, (s0, sl) in enumerate(kcs):
                      u_p = psr.tile([128, (NREL + 1) * NN], F32, tag="u_p", name="u_p",
                                     bufs=2)
                      nc.tensor.matmul(u_p[0:sl, :], h[0:NN, s0:s0 + sl],
                                       aallTb[:], start=True, stop=True)
                      u = prg.tile([128, (NREL + 1) * NN], BF16, tag=f"u{si}",
                                   name=f"u{si}")
                      if si % 2 == 0:
                          nc.scalar.copy(u[0:sl, :], u_p[0:sl, :])
                      else:
                          nc.vector.tensor_copy(out=u[0:sl, :],
                                                in_=u_p[0:sl, :])
                      u_sb.append(u)
                  # y = sum_r (u_r)^T @ Wst_r + h @ Wself
                  y_p = psr.tile([NN, EMB], F32, tag="y_p", name="y_p")
                  n_mm = (NREL + 1) * nk
                  k_mm = 0
                  for si, (s0, sl) in enumerate(kcs):
                      for r in range(NREL + 1):
                          nc.tensor.matmul(
                              y_p[:], u_sb[si][0:sl, r * NN:(r + 1) * NN],
                              wst_t[r * nk + si][0:sl, :],
                              start=(k_mm == 0), stop=(k_mm == n_mm - 1))
                          k_mm += 1
                  hn = prg.tile([NN, EMB], BF16, tag="h_next", name="h_next")
                  nc.scalar.activation(hn[:], y_p[:], AF.Relu)
                  h = hn

              # conv1 weights: first-use order is right after rgcn weights
              w1 = []
              for kc in range(4):
                  t = pconst.tile([128, 25 * 128], BF16, tag=f"w1_{kc}",
                                  name=f"w1_{kc}")
                  nc.gpsimd.dma_start(t[:], w1sb_d[kc])
                  w1.append(t)

              # entity_struT + e_ctxT -> ecT
              for mc in range(4):
                  tp = psr.tile([128, E], F32, tag="est", name="est", bufs=2)
                  nc.tensor.matmul(tp[:], h[0:E, mc * 128:(mc + 1) * 128],
                                   identb[0:E, 0:E], start=True, stop=True)
                  nc.vector.tensor_tensor(out=ecT[mc][:], in0=tp[:],
                                          in1=ectxT_sb[mc][:], op=ALU.add)

        if stages >= 3:
          # ================= stage 3: fmap + SE =================
          fmap = [pwork.tile([128, EE], F32R, tag=f"fmap{i}", name=f"fmap{i}")
                  for i in range(4)]
          pooled = [pwork.tile([128, 1], F32R, tag=f"pool{i}", name=f"pool{i}")
                    for i in range(4)]
          fusedp = [pwork.tile([128, PADW], BF16, tag=f"fusedp{i}",
                               name=f"fusedp{i}") for i in range(4)]
          for mc in range(4):
              o6v = fmap[mc][:].rearrange("p (i j) -> p i j", i=E)
              in0 = ecT[mc][:].rearrange("p (i j) -> p i j", j=1) \
                  .to_broadcast([128, E, E])
              in1 = ecT[mc][:].rearrange("p (o j) -> p o j", o=1) \
                  .to_broadcast([128, E, E])
              nc.vector.tensor_tensor(out=o6v, in0=in0, in1=in1, op=ALU.mult)
              rs = pwork.tile([128, 1], F32, tag=f"rs{mc}", name=f"rs{mc}")
              nc.vector.tensor_reduce(rs[:], ecT[mc][:], mybir.AxisListType.X,
                                      ALU.add)
              nc.scalar.activation(pooled[mc][:], rs[:], AF.Square, scale=1.0 / E)

          with tc.tile_pool(name="pse", bufs=1, space="PSUM") as pse:
              s1_sb = [pwork.tile([128, EE], F32R, tag=f"s1_{i}", name=f"s1_{i}")
                       for i in range(2)]
              for oc in range(2):
                  s1_p = pse.tile([128, EE], F32, tag="s1p", name="s1p", bufs=2)
                  for mc in range(4):
                      nc.tensor.matmul(s1_p[:],
                                       sew["fsw1T"][mc][:, oc * 128:(oc + 1) * 128],
                                       fmap[mc][:], start=(mc == 0), stop=(mc == 3))
                  nc.scalar.activation(s1_sb[oc][:], s1_p[:], AF.Relu,
                                       bias=sev["seb1"][oc][:],
                                       scale=sev["ses1"][oc][:])
              c1_sb = [pwork.tile([128, 1], F32R, tag=f"c1_{i}", name=f"c1_{i}")
                       for i in range(2)]
              for oc in range(2):
                  c1_p = pse.tile([128, 1], F32, tag="c1p", name="c1p")
                  for mc in range(4):
                      nc.tensor.matmul(c1_p[:],
                                       sew["fcw1T"][mc][:, oc * 128:(oc + 1) * 128]
                                       .bitcast(F32),
                                       pooled[mc][:].bitcast(F32),
                                       start=(mc == 0), stop=(mc == 3))
                  nc.scalar.activation(c1_sb[oc][:], c1_p[:], AF.Relu,
                                       bias=sev["fcb1"][oc][:],
                                       scale=sev["fcs1"][oc][:])
              cbb = [pwork.tile([128, 1], F32, tag=f"cbb{i}", name=f"cbb{i}")
                     for i in range(4)]
              for mc in range(4):
                  c2_p = pse.tile([128, 1], F32, tag="c2p", name="c2p")
                  for kc in range(2):
                      nc.tensor.matmul(c2_p[:],
                                       sew["fcw2T"][kc][:, mc * 128:(mc + 1) * 128]
                                       .bitcast(F32),
                                       c1_sb[kc][:].bitcast(F32),
                                       start=(kc == 0), stop=(kc == 1))
                  cb = pwork.tile([128, 1], F32, tag=f"cb{mc}", name=f"cb{mc}")
                  nc.scalar.activation(cb[:], c2_p[:], AF.Identity,
                                       bias=sev["fcb2"][mc][:],
                                       scale=sev["fcs2"][mc][:])
                  nc.vector.tensor_tensor(out=cbb[mc][:], in0=cb[:],
                                          in1=sev["seb2"][mc][:], op=ALU.add)
              for mc in range(4):
                  nc.vector.memset(fusedp[mc][:], 0.0)
              for mc in range(4):
                  s2_p = pse.tile([128, EE], F32, tag="s2p", name="s2p", bufs=2)
                  for kc in range(2):
                      nc.tensor.matmul(s2_p[:],
                                       sew["fsw2T"][kc][:, mc * 128:(mc + 1) * 128],
                                       s1_sb[kc][:], start=(kc == 0), stop=(kc == 1))
                  sig = pwork.tile([128, EE], F32, tag="sig", name="sig", bufs=2)
                  nc.scalar.activation(sig[:], s2_p[:], AF.Sigmoid,
                                       bias=cbb[mc][:], scale=sev["ses2"][mc][:])
                  outv = fusedp[mc][:].rearrange("p (i j) -> p i j", j=26)[:, 2:24,
                                                                          2:24]
                  nc.vector.tensor_tensor(
                      out=outv,
                      in0=fmap[mc][:].rearrange("p (i j) -> p i j", i=E),
                      in1=sig[:].rearrange("p (i j) -> p i j", i=E),
                      op=ALU.mult)

        if stages >= 4:
          # ================= stage 4: conv stack =================
          def tap_view(padt, tap):
              dy, dx = tap // 5, tap % 5
              return padt[:].rearrange("p (i j) -> p i j", j=26)[:, dy:dy + 22,
                                                                dx:dx + 22]

          with tc.tile_pool(name="pcw", bufs=1) as pcw, \
               tc.tile_pool(name="psc", bufs=1, space="PSUM") as psc:
              w2 = []
              for kc in range(2):
                  t = pcw.tile([128, 25 * 128], BF16, tag=f"w2_{kc}",
                               name=f"w2_{kc}")
                  nc.gpsimd.dma_start(t[:], w2sb_d[kc])
                  w2.append(t)
              w3 = []
              for kc in range(2):
                  t = pcw.tile([128, 25 * 256], BF16, tag=f"w3_{kc}",
                               name=f"w3_{kc}")
                  nc.gpsimd.dma_start(t[:], w3sb_d[kc])
                  w3.append(t)

              own1 = pcw.tile([128, PADW], BF16)
              oth1 = pcw.tile([128, PADW], BF16)
              own2 = pcw.tile([128, PADW], BF16)
              oth2 = pcw.tile([128, PADW], BF16)
              for t_ in (own1, oth1, own2, oth2):
                  nc.vector.memset(t_[:], 0.0)
              mtop = cf("mtop")
              mbot = cf("mbot")

              def interior(t_):
                  return t_[:].rearrange("p (i j) -> p i j", j=26)[:, 2:24, 2:24]

              # conv1 (my half of 256 out channels) -> own1 padded directly
              r1_p = psc.tile([128, EE], F32, tag="convp", name="convp", bufs=2)
              first = True
              for kc in range(4):
                  for tap in range(25):
                      nc.tensor.matmul(r1_p[:],
                                       w1[kc][:, tap * 128:(tap + 1) * 128],
                                       tap_view(fusedp[kc], tap),
                                       start=first, stop=(kc == 3 and tap == 24))
                      first = False
              nc.scalar.activation(interior(own1), r1_p[:], AF.Relu, bias=b1h[:])

              r1b = pdram.tile([128, EE], BF16)
              r1g = pdram.tile([256, EE], BF16)
              nc.sync.dma_start(r1b[:], interior(own1))
              if solo:
                  nc.sync.dma_start(r1g[0:128, :], r1b[:])
                  nc.sync.dma_start(r1g[128:256, :], r1b[:])
              else:
                  nc.gpsimd.collective_compute(
                      "AllGather", ALU.bypass, replica_groups=groups,
                      ins=[r1b[:].opt()], outs=[r1g[:].opt()])
              g1t = pcw.tile([128, EE], BF16)
              g1b = pcw.tile([128, EE], BF16)
              nc.sync.dma_start(g1t[:], r1g[0:128, :])
              nc.sync.dma_start(g1b[:], r1g[128:256, :])
              tmp1 = pcw.tile([128, EE], F32)
              nc.vector.tensor_scalar(out=tmp1[:], in0=g1b[:], scalar1=mbot[:],
                                      scalar2=None, op0=ALU.mult)
              nc.vector.scalar_tensor_tensor(out=interior(oth1), in0=g1t[:],
                                             scalar=mtop[:], in1=tmp1[:],
                                             op0=ALU.mult, op1=ALU.add)

              # conv2: own half first (no gather dependency), then other half
              r2_p = psc.tile([128, EE], F32, tag="convp", name="convp2", bufs=2)
              for tap in range(25):
                  nc.tensor.matmul(r2_p[:], w2[0][:, tap * 128:(tap + 1) * 128],
                                   tap_view(own1, tap),
                                   start=(tap == 0), stop=False)
              for tap in range(25):
                  nc.tensor.matmul(r2_p[:], w2[1][:, tap * 128:(tap + 1) * 128],
                                   tap_view(oth1, tap),
                                   start=False, stop=(tap == 24))
              nc.scalar.activation(interior(own2), r2_p[:], AF.Relu, bias=b2h[:])

              r2b = pdram.tile([128, EE], BF16)
              r2g = pdram.tile([256, EE], BF16)
              nc.sync.dma_start(r2b[:], interior(own2))
              if solo:
                  nc.sync.dma_start(r2g[0:128, :], r2b[:])
                  nc.sync.dma_start(r2g[128:256, :], r2b[:])
              else:
                  nc.gpsimd.collective_compute(
                      "AllGather", ALU.bypass, replica_groups=groups,
                      ins=[r2b[:].opt()], outs=[r2g[:].opt()])
              g2t = pcw.tile([128, EE], BF16)
              g2b = pcw.tile([128, EE], BF16)
              nc.sync.dma_start(g2t[:], r2g[0:128, :])
              nc.sync.dma_start(g2b[:], r2g[128:256, :])
              tmp2 = pcw.tile([128, EE], F32)
              nc.vector.tensor_scalar(out=tmp2[:], in0=g2b[:], scalar1=mbot[:],
                                      scalar2=None, op0=ALU.mult)
              nc.vector.scalar_tensor_tensor(out=interior(oth2), in0=g2t[:],
                                             scalar=mtop[:], in1=tmp2[:],
                                             op0=ALU.mult, op1=ALU.add)

              # conv3 (my 256 of 512 out channels): both chunks' own-half
              # taps first, widening the window that hides the r2 AllGather
              r3_ps = [psc.tile([128, EE], F32, tag="convp3", name="convp3",
                                bufs=2) for _ in range(2)]
              for oc in range(2):
                  for tap in range(25):
                      nc.tensor.matmul(
                          r3_ps[oc][:],
                          w3[0][:, tap * 256 + oc * 128:
                                tap * 256 + (oc + 1) * 128],
                          tap_view(own2, tap), start=(tap == 0), stop=False)
              for oc in range(2):
                  for tap in range(25):
                      nc.tensor.matmul(
                          r3_ps[oc][:],
                          w3[1][:, tap * 256 + oc * 128:
                                tap * 256 + (oc + 1) * 128],
                          tap_view(oth2, tap), start=False, stop=(tap == 24))
                  o_sb = pcw.tile([128, EE], F32, tag=f"osb{oc}",
                                  name=f"osb{oc}")
                  nc.scalar.activation(o_sb[:], r3_ps[oc][:], AF.Relu,
                                       bias=b3h[oc][:])
                  nc.sync.dma_start(out_d[oc * 128:(oc + 1) * 128, :], o_sb[:])

    nc.compile()
    return nc


_NC_CACHE = None


def _get_program():
    global _NC_CACHE
    if _NC_CACHE is None:
        _NC_CACHE = build_program()
    return _NC_CACHE


def _prep_shared(w):
    """Packed weights/constants identical on every core."""
    ADJ = _build_adj()
    out = {}
    constr = np.zeros((128, _CR), np.float32)

    def put(nm, arr):
        c0, cols = _LAY_R[nm]
        r, cc = arr.shape
        constr[0:r, c0:c0 + cc] = arr
    wt = w['W_trans']
    for kc in range(6):
        put(f"wtr{kc}", wt[kc * 128:(kc + 1) * 128])
    put("brow", w['b_trans'].reshape(1, EMB))
    put("onescol", np.ones((128, 1), np.float32))
    put("onesrow", np.ones((128, 128), np.float32))
    g2T = np.zeros((EM, E), np.float32)
    for e in range(E):
        g2T[e * M:(e + 1) * M, e] = 1.0
    put("g2T", g2T)
    sumT = np.kron(np.eye(L, dtype=np.float32), np.ones((SPAN, 1), np.float32))
    for kc in range(4):
        put(f"sumT{kc}", sumT[kc * 128:(kc + 1) * 128])
    put("aallT", np.concatenate([ADJ[r].T for r in range(NREL)], axis=1))
    put("tfeat", np.ascontiguousarray(w['type_embed'][_TYPES]))
    for nm, arr, nch in (("fsw1T", w['fs_w1'].T, 4), ("fcw1T", w['fc_w1'].T, 4),
                         ("fsw2T", w['fs_w2'].T, 2), ("fcw2T", w['fc_w2'].T, 2)):
        for kc in range(nch):
            put(f"{nm}{kc}", np.ascontiguousarray(arr[kc * 128:(kc + 1) * 128]))
    gT = np.zeros((EMH, E), np.float32)
    for e in range(E):
        gT[e * M * H:(e + 1) * M * H, e] = 1.0 / (M * H)
    for kc in range(9):
        r = min(128, EMH - kc * 128)
        put(f"gT{kc}", gT[kc * 128:kc * 128 + r])
    out['constr'] = constr
    gTb = np.zeros((128, 9 * E), np.float32)
    for kc in range(9):
        r = min(128, EMH - kc * 128)
        gTb[0:r, kc * E:(kc + 1) * E] = gT[kc * 128:kc * 128 + r]
    out['gTb'] = gTb.astype(ml_dtypes.bfloat16)
    out['aallTb'] = np.concatenate(
        [ADJ[r].T for r in range(NREL)] + [np.eye(NN, dtype=np.float32)],
        axis=1).astype(ml_dtypes.bfloat16)
    out['tfb'] = np.ascontiguousarray(w['type_embed'][_TYPES])
    out['identb'] = np.eye(128, dtype=np.float32).astype(ml_dtypes.bfloat16)

    constf = np.zeros((128, _CF), np.float32)

    def putf(nm, arr):
        c0, cols = _LAY_F[nm]
        constf[0:arr.shape[0], c0:c0 + 1] = arr.reshape(-1, 1)
    vecs = {"ses1": w['fs_g1'], "seb1": w['fs_b1'] * w['fs_g1'] + w['fs_be1'],
            "fcs1": w['fc_g1'], "fcb1": w['fc_b1'] * w['fc_g1'] + w['fc_be1'],
            "ses2": w['fs_g2'], "seb2": w['fs_b2'] * w['fs_g2'] + w['fs_be2'],
            "fcs2": w['fc_g2'], "fcb2": w['fc_b2'] * w['fc_g2'] + w['fc_be2']}
    for nm, v in vecs.items():
        nch = 2 if v.shape[0] == INTER else 4
        for kc in range(nch):
            putf(f"{nm}{kc}", v[kc * 128:(kc + 1) * 128])
    out['constf_base'] = constf

    for layer in range(NLAYERS):
        din_l = D0 if layer == 0 else EMB
        kcs = _KC0 if layer == 0 else _KC1
        nk = len(kcs)
        Wst = w['rgcn_Wrel0'].reshape(NREL * D0, EMB) if layer == 0 else \
            w['rgcn_Wrel'][layer - 1].reshape(NREL * EMB, EMB)
        Wself = w['rgcn_Wself0'] if layer == 0 else w['rgcn_Wself'][layer - 1]
        p = np.zeros((128, (NREL + 1) * nk * EMB), np.float32)
        for r in range(NREL):
            for si, (s0, sl) in enumerate(kcs):
                p[0:sl, (r * nk + si) * EMB:(r * nk + si + 1) * EMB] = \
                    Wst[r * din_l + s0:r * din_l + s0 + sl]
        for si, (s0, sl) in enumerate(kcs):
            p[0:sl, (NREL * nk + si) * EMB:(NREL * nk + si + 1) * EMB] = \
                Wself[s0:s0 + sl]
        out[f'wstp{layer}'] = p.astype(ml_dtypes.bfloat16)
    return out


def _prep_conv_half(w, half, constf_base):
    out = {}
    w1 = w['cr_w1'][half * 128:(half + 1) * 128]
    out['w1sb'] = np.ascontiguousarray(
        w1.transpose(1, 2, 3, 0).reshape(4, 128, 25 * 128)).astype(ml_dtypes.bfloat16)
    # conv2/conv3 weights in (own-input-half, other-input-half) chunk order
    w2 = w['cr_w2'][half * 128:(half + 1) * 128]
    w2p = w2.transpose(1, 2, 3, 0).reshape(2, 128, 25 * 128)
    order = [half, 1 - half]
    out['w2sb'] = np.ascontiguousarray(w2p[order]).astype(ml_dtypes.bfloat16)
    w3 = w['cr_w3'][half * 256:(half + 1) * 256]
    w3p = w3.transpose(1, 2, 3, 0).reshape(2, 128, 25 * 256)
    out['w3sb'] = np.ascontiguousarray(w3p[order]).astype(ml_dtypes.bfloat16)
    constf = constf_base.copy()

    def putf(nm, arr):
        c0, cols = _LAY_F[nm]
        constf[0:arr.shape[0], c0:c0 + 1] = arr.reshape(-1, 1)
    putf("b1h", w['cr_b1'][half * 128:(half + 1) * 128])
    putf("b2h", w['cr_b2'][half * 128:(half + 1) * 128])
    putf("b3h0", w['cr_b3'][half * 256:half * 256 + 128])
    putf("b3h1", w['cr_b3'][half * 256 + 128:half * 256 + 256])
    putf("mtop", np.full(128, float(half), np.float32))
    putf("mbot", np.full(128, float(1 - half), np.float32))
    c0, cols = _LAY_F["identf"]
    constf[:, c0:c0 + 128] = np.eye(128, dtype=np.float32)
    out['constf'] = constf
    return out


def _prep_doc(x, att, mi, ls):
    out = {}
    mif = mi.reshape(EM)
    attm = np.ascontiguousarray(
        att[:, mif, :].transpose(1, 0, 2).reshape(EMH, C))
    amp = np.zeros((128, 9 * C), np.float32)
    for kc in range(9):
        r = min(128, EMH - kc * 128)
        amp[0:r, kc * C:kc * C + C] = attm[kc * 128:kc * 128 + r]
    out['amp'] = amp.astype(ml_dtypes.bfloat16)
    idx = ls[:, None] + np.arange(SPAN)
    idxf = idx.reshape(LS)
    rows = att[:, idxf, :].reshape(H, L, SPAN, C)
    blocks = np.take_along_axis(rows, idx[None, :, None, :], axis=3)
    attl = blocks.transpose(0, 2, 1, 3).reshape(HS, LS)
    xmT = x[mif].T
    xspT = x[idxf].T
    actr = np.zeros((128, _CA), np.float32)

    def put(nm, arr):
        c0, cols = _LAY_A[nm]
        actr[0:arr.shape[0], c0:c0 + arr.shape[1]] = arr
    for kc in range(6):
        put(f"xmT{kc}", xmT[kc * 128:(kc + 1) * 128])
        put(f"xspT{kc}", xspT[kc * 128:(kc + 1) * 128])
    for kc in range(3):
        put(f"attl{kc}", attl[kc * 128:(kc + 1) * 128])
    out['actr'] = actr
    xpk = np.zeros((128, 8 * HID), np.float32)
    for kc in range(8):
        xpk[:, kc * HID:(kc + 1) * HID] = x[kc * 128:(kc + 1) * 128]
    out['xp'] = xpk.astype(ml_dtypes.bfloat16)
    return out


def build_in_maps(inputs):
    w = {}
    for k, v in inputs.items():
        a = np.asarray(v)
        w[k] = a if a.dtype in (np.int32, np.int64) else \
            np.asarray(a, np.float32)
    shared = _prep_shared(w)
    constf_base = shared.pop('constf_base')
    halves = [_prep_conv_half(w, h, constf_base) for h in range(2)]
    seq = np.asarray(inputs['sequence_output'], np.float32)
    att = np.asarray(inputs['attention'], np.float32)
    mi = np.asarray(inputs['mention_idx']).astype(np.int64)
    ls = np.asarray(inputs['link_start']).astype(np.int64)
    docs = [_prep_doc(seq[n], att[n], mi[n], ls[n]) for n in range(NB)]
    in_maps = []
    for core in range(N_CORES):
        n, half = core // 2, core % 2
        m = dict(shared)
        m.update(halves[half])
        m.update(docs[n])
        in_maps.append({k: (np.ascontiguousarray(v) if v.dtype == ml_dtypes.bfloat16
                            else np.ascontiguousarray(v, np.float32))
                        for k, v in m.items()})
    return in_maps


def kernel(**inputs):
    nc = _get_program()
    in_maps = build_in_maps(inputs)
    res = run_bass_kernel_spmd(nc, in_maps, list(range(N_CORES)))
    out = np.zeros((NB, EMB, E, E), np.float32)
    for core in range(N_CORES):
        n, half = core // 2, core % 2
        out[n, half * 256:(half + 1) * 256] = \
            res.results[core]["out"].reshape(256, E, E)
    return out



# revision 45
# speedup vs baseline: 1.3129x; 1.3129x over previous
"""Trainium2 Bass kernel for nn_DocREModel (DocRE: gather -> RGCN -> SE -> 5x5 convs).

Sharding: 4 documents x 2 cores each. Each pair replicates the cheap upstream
(mention/link/ea gathers -> RGCN -> fmap/SE) and splits the dominant 5x5 conv
stack by output channels, with two intra-pair AllGathers; output halves are
assembled on host. All index-driven gathers happen on host (pure data
movement; one SPMD program serves all 8 cores), all dense math on device.

Perf model notes (TimelineSim): all DMAs serialize on one ~332 GB/s pipe in
~issue order, and the PE p-state ramp rewards keeping the tensor engine
continuously fed. Hence: everything DMA'd is bf16 (f32 only for small
per-channel scale/bias vectors), tensors are issued strictly in first-use
order (amp/gTb first so the ea matmuls start ~2.5us in), RGCN + conv weights
stream just-in-time behind the compute, and h0 is assembled directly by ACT
writes into a bf16 tile instead of SBUF->SBUF DMA round trips.

Precision/layout choices:
- bf16 weights+activations everywhere on the matmul path, f32 PSUM
  accumulation throughout; per-channel BN scales/biases stay f32.
- Convs are 25 shift-tap matmuls over zero-padded 26x26 images via strided
  APs (no im2col copies). conv2/conv3 start on the locally-computed input
  half before the pair AllGather completes; the other half is extracted
  SPMD-safely with host-supplied 0/1 masks and per-core (own, other)
  weight-chunk ordering.
- RGCN folds the self-loop in as a 4th identity relation so each layer is
  one u = h^T @ [A0^T|A1^T|A2^T|I] matmul plus one PSUM accumulation over
  stacked (relation, chunk) weights -- no transposes in the loop.
"""

import numpy as np
import ml_dtypes

import concourse.bacc as bacc
import concourse.tile as tile
from concourse import mybir
from concourse.bass_utils import run_bass_kernel_spmd

F32 = mybir.dt.float32
F32R = mybir.dt.float32r
BF16 = mybir.dt.bfloat16
F8 = mybir.dt.float8e4
AF = mybir.ActivationFunctionType
ALU = mybir.AluOpType

NB, H, C, HID, EMB = 4, 12, 1024, 768, 512
E, M, L, SPAN = 22, 4, 16, 32
TD, INTER = 20, 256
NN = E + E * M + L
NREL, NLAYERS = 3, 4
EM, EMH, HS, LS = E * M, E * M * H, H * SPAN, L * SPAN
D0 = EMB + TD           # 532
EE = E * E              # 484
PADW = 26 * 26          # 676 padded 26x26 image
N_CORES = 8


def _build_adj():
    A = np.zeros((NREL, NN, NN), np.float32)
    for e in range(E):
        for m in range(M):
            mi = E + e * M + m
            A[0, e, mi] = A[0, mi, e] = 1.0
            for m2 in range(M):
                if m2 != m:
                    A[1, mi, E + e * M + m2] = 1.0
            li = E + E * M + ((e * M + m) % L)
            A[2, mi, li] = A[2, li, mi] = 1.0
    A = A / (A.sum(-1, keepdims=True) + 1e-5)
    return A


_TYPES = np.concatenate([np.zeros(E, np.int32), np.ones(EM, np.int32),
                         np.full(L, 2, np.int32)])

_KC0 = [(0, 128), (128, 128), (256, 128), (384, 128), (512, 20)]   # 532 rows
_KC1 = [(0, 128), (128, 128), (256, 128), (384, 128)]              # 512 rows


def _constb_layout():
    """Column layout of the packed bf16 constant tensor [128, CB].

    Part A (cols 0:CBA) is everything needed through stage 3's s1/c1;
    part B (fsw2T/fcw2T) is DMA'd later, after the RGCN weights.
    """
    lay = {}
    c = 0

    def add(nm, cols):
        nonlocal c
        lay[nm] = (c, cols)
        c += cols
    for kc in range(6):
        add(f"wtr{kc}", EMB)
    add("brow", EMB)
    add("onesrow", 128)
    add("onescol", 1)
    add("g2T", E)
    for kc in range(4):
        add(f"sumT{kc}", L)
    for kc in range(4):
        add(f"fsw1T{kc}", INTER)
    for kc in range(4):
        add(f"fcw1T{kc}", INTER)
    cba = c
    for kc in range(2):
        add(f"fsw2T{kc}", EMB)
    for kc in range(2):
        add(f"fcw2T{kc}", EMB)
    return lay, c, cba


def _constf_layout():
    lay = {}
    c = 0

    def add(nm, cols):
        nonlocal c
        lay[nm] = (c, cols)
        c += cols
    for nm, nch in (("ses1", 2), ("seb1", 2), ("fcs1", 2), ("fcb1", 2),
                    ("ses2", 4), ("seb2", 4), ("fcs2", 4), ("fcb2", 4)):
        for kc in range(nch):
            add(f"{nm}{kc}", 1)
    add("b1h", 1)
    add("b2h", 1)
    add("b3h0", 1)
    add("b3h1", 1)
    add("mtop", 1)
    add("mbot", 1)
    add("identf", 128)
    return lay, c


def _actb_layout():
    lay = {}
    c = 0

    def add(nm, cols):
        nonlocal c
        lay[nm] = (c, cols)
        c += cols
    for kc in range(6):
        add(f"xmT{kc}", EM)
    for kc in range(6):
        add(f"xspT{kc}", LS)
    for kc in range(3):
        add(f"attl{kc}", LS)
    return lay, c


_LAY_B, _CB, _CBA = _constb_layout()
_LAY_F, _CF = _constf_layout()
_LAY_A, _CA = _actb_layout()


def build_program(solo=False, stages=4):
    nc = bacc.Bacc("TRN2", target_bir_lowering=False, debug=False)

    def din(name, shape, dt=BF16):
        return nc.dram_tensor(name, list(shape), dt, kind="ExternalInput").ap()

    constb_d = din("constb", [128, _CB])
    constf_d = din("constf", [128, _CF], F32)
    actb_d = din("actb", [128, _CA])
    xp_d = din("xp", [128, 8 * HID])
    amp_d = din("amp", [128, 9 * C], F8)
    gTb_d = din("gTb", [128, 9 * E], F8)
    tfb_d = din("tfb", [NN, TD])
    wstp_d = [din("wstp0", [128, 20 * EMB])] + \
             [din(f"wstp{i}", [128, 16 * EMB]) for i in (1, 2, 3)]
    w1sb_d = din("w1sb", [4, 128, 25 * 128])
    w2sb_d = din("w2sb", [2, 128, 25 * 128])
    w3sb_d = din("w3sb", [2, 128, 25 * 256])
    aallTb_d = din("aallTb", [NN, (NREL + 1) * NN])
    identb_d = din("identb", [128, 128])

    out_d = nc.dram_tensor("out", [256, EE], F32, kind="ExternalOutput").ap()

    groups = [[0, 1], [2, 3], [4, 5], [6, 7]]

    with tile.TileContext(nc) as tc:
      with tc.tile_pool(name="pconst", bufs=1) as pconst, \
           tc.tile_pool(name="pwork", bufs=1) as pwork, \
           tc.tile_pool(name="pdram", bufs=1, space="DRAM") as pdram:

        constb = pconst.tile([128, _CB], BF16)
        constf = pconst.tile([128, _CF], F32)
        identb = pconst.tile([128, 128], BF16)
        aallTb = pconst.tile([NN, (NREL + 1) * NN], BF16)
        aallE = pconst.tile([E, (NREL + 1) * NN], BF16)
        aallM = pconst.tile([EM, (NREL + 1) * NN], BF16)
        aallL = pconst.tile([L, (NREL + 1) * NN], BF16)
        wstp_t = [pconst.tile([128, 20 * EMB], BF16, tag="wstp0",
                              name="wstp0")] + \
                 [pconst.tile([128, 16 * EMB], BF16, tag=f"wstp{l}",
                              name=f"wstp{l}") for l in (1, 2, 3)]
        w1 = [pconst.tile([128, 25 * 128], BF16, tag=f"w1_{kc}",
                          name=f"w1_{kc}") for kc in range(4)]

        def cb(nm, rows=128):
            c0, cols = _LAY_B[nm]
            return constb[0:rows, c0:c0 + cols]

        def cf(nm, rows=128):
            c0, cols = _LAY_F[nm]
            return constf[0:rows, c0:c0 + cols]

        wtr = [cb(f"wtr{kc}") for kc in range(6)]
        brow = cb("brow", rows=1)
        onesrow = cb("onesrow", rows=1)
        onescol = cb("onescol")
        g2T = cb("g2T", rows=EM)
        sumT = [cb(f"sumT{kc}") for kc in range(4)]
        sew = {nm: [cb(f"{nm}{kc}") for kc in range(n)]
               for nm, n in (("fsw1T", 4), ("fcw1T", 4), ("fsw2T", 2),
                             ("fcw2T", 2))}
        sev = {nm: [cf(f"{nm}{kc}") for kc in range(n)]
               for nm, n in (("ses1", 2), ("seb1", 2), ("fcs1", 2), ("fcb1", 2),
                             ("ses2", 4), ("seb2", 4), ("fcs2", 4),
                             ("fcb2", 4))}
        b1h = cf("b1h")
        b2h = cf("b2h")
        b3h = [cf("b3h0"), cf("b3h1")]
        ident = cf("identf")

        # persistent intermediates (three base-0 tiles: engines cannot
        # write SBUF at unaligned base partitions, so the node matrix is
        # kept split as [entities; mentions; links])
        h0e = pwork.tile([E, D0], BF16)
        h0m = pwork.tile([EM, D0], BF16)
        h0l = pwork.tile([L, D0], BF16)
        hfin = pwork.tile([NN, EMB], BF16)
        ectxT_sb = [pwork.tile([128, E], F32, tag=f"ectxT{i}", name=f"ectxT{i}")
                    for i in range(4)]
        ecT = [pwork.tile([128, E], F32R, tag=f"ecT{i}", name=f"ecT{i}")
               for i in range(4)]
        # PE warmup fodder: covers the head until real operands land (the
        # scheduler hoists dependency-free matmuls to the front).
        warm = pwork.tile([128, 512], BF16)
        nc.vector.memset(warm[:], 0.0)
        fusedp = [pwork.tile([128, PADW], BF16, tag=f"fusedp{i}",
                             name=f"fusedp{i}") for i in range(4)]
        g1pc = pwork.tile([128, 2 * PADW], BF16, tag="g1pc", name="g1pc")
        g2pc = pwork.tile([128, 2 * PADW], BF16, tag="g2pc", name="g2pc")
        g1p = [g1pc[:, i * PADW:(i + 1) * PADW] for i in range(2)]
        g2p = [g2pc[:, i * PADW:(i + 1) * PADW] for i in range(2)]
        for t_ in fusedp:
            nc.vector.memset(t_[:], 0.0)
        nc.vector.memset(g1pc[:], 0.0)
        nc.vector.memset(g2pc[:], 0.0)

        with tc.tile_pool(name="pbig", bufs=1) as pbig:
            gTb = pbig.tile([128, 9 * E], F8)
            amp = pbig.tile([128, 9 * C], F8)
            xp = pbig.tile([128, 8 * HID], BF16)
            actb = pbig.tile([128, _CA], BF16)

            # ---- the bulk DMA stream rides the SWDGE (gpsimd) ring in
            # first-use order; sync/scalar stay shallow for latency-
            # critical transfers later (conv exchanges, outputs) ----
            nc.scalar.dma_start(constf[:], constf_d[:])
            xm_cols = 6 * EM                      # xmT region of actb
            wtr_cols = 6 * EMB + EMB + 128 + 1    # wtr+brow+ones region
            nc.gpsimd.dma_start(actb[:, 0:xm_cols], actb_d[:, 0:xm_cols])
            nc.gpsimd.dma_start(constb[:, 0:wtr_cols], constb_d[:, 0:wtr_cols])
            se1_cols = wtr_cols + E + 4 * L   # g2T+sumT end
            nc.gpsimd.dma_start(constb[:, wtr_cols:se1_cols],
                                constb_d[:, wtr_cols:se1_cols])
            sp_cols = xm_cols + 6 * LS
            nc.gpsimd.dma_start(actb[:, xm_cols:sp_cols],
                                actb_d[:, xm_cols:sp_cols])
            nc.gpsimd.dma_start(actb[:, sp_cols:_CA], actb_d[:, sp_cols:_CA])
            nc.scalar.dma_start(h0e[:, EMB:D0], tfb_d[0:E, :])
            nc.scalar.dma_start(h0m[:, EMB:D0], tfb_d[E:E + EM, :])
            nc.scalar.dma_start(h0l[:, EMB:D0], tfb_d[E + EM:NN, :])
            nc.gpsimd.dma_start(aallTb[:], aallTb_d[:])
            nc.gpsimd.dma_start(aallE[:], aallTb_d[0:E, :])
            nc.gpsimd.dma_start(aallM[:], aallTb_d[E:E + EM, :])
            nc.gpsimd.dma_start(aallL[:], aallTb_d[E + EM:NN, :])
            # RGCN weights, chunked si-major so each layer's PSUM chain can
            # start as soon as its first chunk lands
            BL = (NREL + 1) * EMB

            def wstp_dma(layer):
                nchunks = 5 if layer == 0 else 4
                for si in range(nchunks):
                    nc.gpsimd.dma_start(
                        wstp_t[layer][:, si * BL:(si + 1) * BL],
                        wstp_d[layer][:, si * BL:(si + 1) * BL])
            wstp_dma(0)
            wstp_dma(1)
            nc.gpsimd.dma_start(gTb[:], gTb_d[:])
            for g in range(3):
                nc.gpsimd.dma_start(amp[:, g * 3 * C:(g + 1) * 3 * C],
                                    amp_d[:, g * 3 * C:(g + 1) * 3 * C])
            nc.gpsimd.dma_start(xp[:], xp_d[:])
            nc.gpsimd.dma_start(constb[:, se1_cols:_CBA],
                                constb_d[:, se1_cols:_CBA])
            wstp_dma(2)
            wstp_dma(3)
            nc.gpsimd.dma_start(constb[:, _CBA:_CB], constb_d[:, _CBA:_CB])
            nc.scalar.dma_start(identb[:], identb_d[:])
            for kc in range(4):
                nc.gpsimd.dma_start(w1[kc][:], w1sb_d[kc])

            # ========== stage 1a: mention/span/link rows -> h0b ==========
            expm = pbig.tile([EM, EMB], BF16)
            sp_ps = []
            wsb = [pbig.tile([128, 1], F32, tag=f"wsb{i}", name=f"wsb{i}")
                   for i in range(4)]
            wsp = [pbig.tile([128, EMB], BF16, tag=f"wsp{i}", name=f"wsp{i}")
                   for i in range(4)]

            def ca(nm, rows=128):
                c0, cols = _LAY_A[nm]
                return actb[0:rows, c0:c0 + cols]

            xmT = [ca(f"xmT{kc}") for kc in range(6)]
            xspT = [ca(f"xspT{kc}") for kc in range(6)]
            attl = [ca(f"attl{kc}") for kc in range(3)]

            with tc.tile_pool(name="ps1a", bufs=1, space="PSUM") as ps1a:
                jp = ps1a.tile([128, 512], F32, tag="jp", name="jp")
                for _ in range(14):
                    nc.tensor.matmul(jp[:], warm[:, 0:128], warm[:],
                                     start=True, stop=True)
                # mentions: mrep = x_m @ Wtr + b -> h0b rows + exp for pooling
                mrep_p = ps1a.tile([EM, EMB], F32, tag="mrep", name="mrep")
                for kc in range(6):
                    nc.tensor.matmul(mrep_p[:], xmT[kc][:, 0:EM], wtr[kc][:],
                                     start=(kc == 0), stop=False)
                nc.tensor.matmul(mrep_p[:], onesrow[0:1, 0:EM], brow[:],
                                 start=False, stop=True)
                nc.scalar.copy(h0m[:, 0:EMB], mrep_p[:])
                nc.scalar.activation(expm[:], mrep_p[:], AF.Exp)
                # e_rep = ln(G2 @ exp(mrep))
                ep_p = ps1a.tile([E, EMB], F32, tag="ep", name="ep")
                nc.tensor.matmul(ep_p[:], g2T[:], expm[:], start=True, stop=True)
                nc.scalar.activation(h0e[:, 0:EMB], ep_p[:], AF.Ln)
                # dummy: switch the ACT table to the sigmoid set now (exp/ln
                # are done) so stage 3's sigmoid doesn't pay the 1.3us load
                sigwarm = pbig.tile([1, 1], F32)
                nc.scalar.activation(sigwarm[:], ep_p[0:1, 0:1], AF.Sigmoid)

                # spans: sp = x_span @ Wtr + b
                for mc in range(4):
                    sp_p = ps1a.tile([128, EMB], F32, tag="sp_p", name="sp_p",
                                     bufs=3)
                    for kc in range(6):
                        nc.tensor.matmul(sp_p[:],
                                         xspT[kc][:, mc * 128:(mc + 1) * 128],
                                         wtr[kc][:], start=(kc == 0), stop=False)
                    nc.tensor.matmul(sp_p[:], onesrow[:], brow[:],
                                     start=False, stop=True)
                    spc = pbig.tile([128, EMB], BF16, tag="spc", name="spc",
                                    bufs=4)
                    nc.scalar.copy(spc[:], sp_p[:])
                    sp_ps.append(spc)
                # w = colsum(attl) / 384
                for mc in range(4):
                    w_p = ps1a.tile([128, 1], F32, tag="w_p", name="w_p", bufs=1)
                    for kc in range(3):
                        nc.tensor.matmul(w_p[:],
                                         attl[kc][:, mc * 128:(mc + 1) * 128],
                                         onescol[:],
                                         start=(kc == 0), stop=(kc == 2))
                    nc.scalar.activation(wsb[mc][:], w_p[:], AF.Copy,
                                         scale=1.0 / (H * SPAN))
                # wsp = psum(sp) * w ; link = SUM^T @ wsp
                for mc in range(4):
                    nc.vector.tensor_scalar(out=wsp[mc][:], in0=sp_ps[mc][:],
                                            scalar1=wsb[mc][:], scalar2=None,
                                            op0=ALU.mult)
                link_p = ps1a.tile([L, EMB], F32, tag="link", name="link")
                for kc in range(4):
                    nc.tensor.matmul(link_p[:], sumT[kc][:], wsp[kc][:],
                                     start=(kc == 0), stop=(kc == 3))
                nc.scalar.copy(h0l[:, 0:EMB], link_p[:])

            # ====== stage 2 + stage 1b interleaved: the ea/e_ctx latency
            # chain fills the RGCN's weight-stream stalls ======
            ea_sb = pbig.tile([E, C], F32R)
            eaT = [pbig.tile([128, E], BF16, tag=f"eaT{i}", name=f"eaT{i}")
                   for i in range(8)]
            z_sb = [pbig.tile([128, E], BF16, tag=f"z{i}", name=f"z{i}")
                    for i in range(6)]
            easumT = pbig.tile([1, E], BF16)

            if stages >= 2:
              with tc.tile_pool(name="prg", bufs=2) as prg, \
                   tc.tile_pool(name="psr", bufs=1, space="PSUM") as psr:

                def rgcn_layer(layer, h):
                    kcs = _KC0 if layer == 0 else _KC1
                    nk = len(kcs)
                    wstp = wstp_t[layer]
                    # si-major packing: block (si, r) at (si*(NREL+1)+r)*EMB
                    wst_t = [wstp[:, (si * (NREL + 1) + r) * EMB:
                                   (si * (NREL + 1) + r + 1) * EMB]
                             for r in range(NREL + 1) for si in range(nk)]
                    u_sb = []
                    for si, (s0, sl) in enumerate(kcs):
                        u_p = psr.tile([128, (NREL + 1) * NN], F32, tag="u_p",
                                       name="u_p", bufs=2)
                        if layer == 0:
                            nc.tensor.matmul(u_p[0:sl, :],
                                             h0e[:, s0:s0 + sl], aallE[:],
                                             start=True, stop=False)
                            nc.tensor.matmul(u_p[0:sl, :],
                                             h0m[:, s0:s0 + sl], aallM[:],
                                             start=False, stop=False)
                            nc.tensor.matmul(u_p[0:sl, :],
                                             h0l[:, s0:s0 + sl], aallL[:],
                                             start=False, stop=True)
                        else:
                            nc.tensor.matmul(u_p[0:sl, :], h[0:NN, s0:s0 + sl],
                                             aallTb[:], start=True, stop=True)
                        u = prg.tile([128, (NREL + 1) * NN], BF16, tag=f"u{si}",
                                     name=f"u{si}", bufs=1)
                        if si % 2 == 0:
                            nc.scalar.copy(u[0:sl, :], u_p[0:sl, :])
                        else:
                            nc.vector.tensor_copy(out=u[0:sl, :],
                                                  in_=u_p[0:sl, :])
                        u_sb.append(u)
                    y_p = psr.tile([NN, EMB], F32, tag="y_p", name="y_p")
                    n_mm = (NREL + 1) * nk
                    k_mm = 0
                    for si, (s0, sl) in enumerate(kcs):
                        for r in range(NREL + 1):
                            nc.tensor.matmul(
                                y_p[:], u_sb[si][0:sl, r * NN:(r + 1) * NN],
                                wst_t[r * nk + si][0:sl, :],
                                start=(k_mm == 0), stop=(k_mm == n_mm - 1))
                            k_mm += 1
                    hdst = hfin if layer == NLAYERS - 1 else \
                        prg.tile([NN, EMB], BF16, tag="h_next", name="h_next")
                    for (s0, sl) in _KC1:
                        nc.scalar.activation(hdst[0:NN, s0:s0 + sl],
                                             y_p[0:NN, s0:s0 + sl], AF.Relu)
                    return hdst

                h1 = rgcn_layer(0, None)

                # -- ea block (runs while wstp1 streams) --
                with tc.tile_pool(name="ps1b", bufs=1, space="PSUM") as ps1b:
                    ea_p0 = ps1b.tile([E, 512], F32, tag="ea0", name="ea0")
                    ea_p1 = ps1b.tile([E, 512], F32, tag="ea1", name="ea1")
                    for kc in range(9):
                        rows = 128 if kc < 8 else 32
                        at = amp[0:rows, kc * C:kc * C + C]
                        gt = gTb[0:rows, kc * E:(kc + 1) * E]
                        nc.tensor.matmul(ea_p0[:], gt, at[:, 0:512],
                                         start=(kc == 0), stop=(kc == 8))
                        nc.tensor.matmul(ea_p1[:], gt, at[:, 512:1024],
                                         start=(kc == 0), stop=(kc == 8))
                    r0 = pbig.tile([E, 1], F32)
                    r1 = pbig.tile([E, 1], F32)
                    nc.vector.tensor_reduce(r0[:], ea_p0[:],
                                            mybir.AxisListType.X, ALU.add)
                    nc.vector.tensor_reduce(r1[:], ea_p1[:],
                                            mybir.AxisListType.X, ALU.add)
                    rsum = pbig.tile([E, 1], F32)
                    nc.vector.tensor_tensor(out=rsum[:], in0=r0[:], in1=r1[:],
                                            op=ALU.add)
                    rsum2 = pbig.tile([E, 1], F32)
                    nc.vector.tensor_scalar(out=rsum2[:], in0=rsum[:],
                                            scalar1=1e-5, scalar2=None,
                                            op0=ALU.add)
                    rinv = pbig.tile([E, 1], F32)
                    nc.vector.reciprocal(rinv[:], rsum2[:])
                    for kc in range(4):
                        c0, c1_ = kc * 128, (kc + 1) * 128
                        if kc % 2 == 0:
                            nc.scalar.copy(ea_sb[:, c0:c1_], ea_p0[:, c0:c1_])
                            nc.scalar.copy(ea_sb[:, 512 + c0:512 + c1_],
                                           ea_p1[:, c0:c1_])
                        else:
                            nc.vector.tensor_copy(out=ea_sb[:, c0:c1_],
                                                  in_=ea_p0[:, c0:c1_])
                            nc.vector.tensor_copy(
                                out=ea_sb[:, 512 + c0:512 + c1_],
                                in_=ea_p1[:, c0:c1_])
                    easum = pbig.tile([E, 1], F32)
                    nc.vector.tensor_tensor(out=easum[:], in0=rsum[:],
                                            in1=rinv[:], op=ALU.mult)
                    # eaT transposes reuse the (now dead) ea psum banks
                    for kc in range(8):
                        tp = ps1b.tile([128, E], F32, tag=f"ea{kc % 2}",
                                       name="eaTt")
                        nc.tensor.transpose(tp[:],
                                            ea_sb[:, kc * 128:(kc + 1) * 128]
                                            .bitcast(F32), ident[0:E, 0:E])
                        if kc % 2 == 0:
                            nc.scalar.copy(eaT[kc][:], tp[:])
                        else:
                            nc.vector.tensor_copy(out=eaT[kc][:], in_=tp[:])
                    tp = ps1b.tile([1, E], F32, tag="ea1", name="easumt")
                    nc.tensor.transpose(tp[:], easum[:], ident[0:E, 0:E])
                    nc.scalar.copy(easumT[:], tp[:])

                h2 = rgcn_layer(1, h1)

                with tc.tile_pool(name="ps1c", bufs=1, space="PSUM") as ps1c:
                    # zT = ea_n @ x  [22, 768] (two 384-wide halves)
                    zt_ps = [ps1c.tile([E, 384], F32, tag="sc",
                                       name=f"zt_p{i}", bufs=2)
                             for i in range(2)]
                    for kc in range(8):
                        xt = xp[:, kc * HID:(kc + 1) * HID]
                        for hh in range(2):
                            nc.tensor.matmul(zt_ps[hh][:], eaT[kc][:],
                                             xt[:, hh * 384:(hh + 1) * 384],
                                             start=(kc == 0), stop=(kc == 7))
                    # ea was left unnormalized; fold the 1/rowsum in here
                    zt_sb = pbig.tile([E, HID], F32)
                    nc.scalar.activation(zt_sb[:, 0:384], zt_ps[0][:], AF.Copy,
                                         scale=rinv[:])
                    nc.scalar.activation(zt_sb[:, 384:768], zt_ps[1][:],
                                         AF.Copy, scale=rinv[:])
                    for kc in range(6):
                        ztp = ps1c.tile([128, E], F32, tag="tp", name="ztp",
                                        bufs=1)
                        nc.tensor.transpose(ztp[:],
                                            zt_sb[:, kc * 128:(kc + 1) * 128],
                                            ident[0:E, 0:E])
                        if kc % 2 == 0:
                            nc.scalar.copy(z_sb[kc][:], ztp[:])
                        else:
                            nc.vector.tensor_copy(out=z_sb[kc][:], in_=ztp[:])
                    # ecT2 = z^T-chunks as lhsT @ Wtr -> [22,512] + b (x) easum
                    ec2_p = ps1c.tile([E, EMB], F32, tag="sc", name="ec2",
                                      bufs=2)
                    for kc in range(6):
                        nc.tensor.matmul(ec2_p[:], z_sb[kc][:], wtr[kc][:],
                                         start=(kc == 0), stop=False)
                    nc.tensor.matmul(ec2_p[:], easumT[:], brow[:],
                                     start=False, stop=True)
                    ec2_sb = pbig.tile([E, EMB], F32)
                    nc.scalar.copy(ec2_sb[:], ec2_p[:])
                    for mc in range(4):
                        ecp = ps1c.tile([128, E], F32, tag="tp", name="ecp",
                                        bufs=1)
                        nc.tensor.transpose(ecp[:],
                                            ec2_sb[:, mc * 128:(mc + 1) * 128],
                                            ident[0:E, 0:E])
                        if mc % 2 == 0:
                            nc.scalar.copy(ectxT_sb[mc][:], ecp[:])
                        else:
                            nc.vector.tensor_copy(out=ectxT_sb[mc][:],
                                                  in_=ecp[:])

                    h3 = rgcn_layer(2, h2)
                    rgcn_layer(3, h3)

                    # entity_struT + e_ctxT -> ecT
                    for mc in range(4):
                        tp = ps1c.tile([128, E], F32,
                                       tag="tp" if mc % 2 == 0 else "sc",
                                       name="est", bufs=1 if mc % 2 == 0 else 2)
                        nc.tensor.matmul(tp[:],
                                         hfin[0:E, mc * 128:(mc + 1) * 128],
                                         identb[0:E, 0:E], start=True,
                                         stop=True)
                        nc.vector.tensor_tensor(out=ecT[mc][:], in0=tp[:],
                                                in1=ectxT_sb[mc][:],
                                                op=ALU.add)

        if stages >= 3:
          # ================= stage 3: fmap + SE =================
          fmap = [pwork.tile([128, EE], BF16, tag=f"fmap{i}", name=f"fmap{i}")
                  for i in range(4)]
          pooled = [pwork.tile([128, 1], BF16, tag=f"pool{i}", name=f"pool{i}")
                    for i in range(4)]
          for mc in range(4):
              for ee, lo, hi in ((nc.vector, 0, 11), (nc.gpsimd, 11, E)):
                  o6v = fmap[mc][:].rearrange("p (i j) -> p i j", i=E)[:, lo:hi]
                  in0 = ecT[mc][:, lo:hi].rearrange("p (i j) -> p i j", j=1) \
                      .to_broadcast([128, hi - lo, E])
                  in1 = ecT[mc][:].rearrange("p (o j) -> p o j", o=1) \
                      .to_broadcast([128, hi - lo, E])
                  ee.tensor_tensor(out=o6v, in0=in0, in1=in1, op=ALU.mult)
              rs = pwork.tile([128, 1], F32, tag=f"rs{mc}", name=f"rs{mc}")
              nc.vector.tensor_reduce(rs[:], ecT[mc][:], mybir.AxisListType.X,
                                      ALU.add)
              nc.scalar.activation(pooled[mc][:], rs[:], AF.Square,
                                   scale=1.0 / E)

          with tc.tile_pool(name="pse", bufs=1, space="PSUM") as pse:
              # channel-attention path first: its latency hides under the
              # fmap outer-product DVE chain
              c1_sb = [pwork.tile([128, 1], BF16, tag=f"c1_{i}", name=f"c1_{i}")
                       for i in range(2)]
              for oc in range(2):
                  c1_p = pse.tile([128, 1], F32, tag="c1p", name="c1p")
                  for mc in range(4):
                      nc.tensor.matmul(c1_p[:],
                                       sew["fcw1T"][mc][:, oc * 128:(oc + 1) * 128],
                                       pooled[mc][:],
                                       start=(mc == 0), stop=(mc == 3))
                  nc.scalar.activation(c1_sb[oc][:], c1_p[:], AF.Relu,
                                       bias=sev["fcb1"][oc][:],
                                       scale=sev["fcs1"][oc][:])
              # fcb2 already carries seb2 (folded on host)
              cbb = [pwork.tile([128, 1], F32, tag=f"cbb{i}", name=f"cbb{i}")
                     for i in range(4)]
              for mc in range(4):
                  c2_p = pse.tile([128, 1], F32, tag="c2p", name="c2p")
                  for kc in range(2):
                      nc.tensor.matmul(c2_p[:],
                                       sew["fcw2T"][kc][:, mc * 128:(mc + 1) * 128],
                                       c1_sb[kc][:],
                                       start=(kc == 0), stop=(kc == 1))
                  nc.scalar.activation(cbb[mc][:], c2_p[:], AF.Identity,
                                       bias=sev["fcb2"][mc][:],
                                       scale=sev["fcs2"][mc][:])
              s1_sb = [pwork.tile([128, EE], BF16, tag=f"s1_{i}", name=f"s1_{i}")
                       for i in range(2)]
              for oc in range(2):
                  s1_p = pse.tile([128, EE], F32, tag="s1p", name="s1p", bufs=2)
                  for mc in range(4):
                      nc.tensor.matmul(s1_p[:],
                                       sew["fsw1T"][mc][:, oc * 128:(oc + 1) * 128],
                                       fmap[mc][:], start=(mc == 0), stop=(mc == 3))
                  nc.scalar.activation(s1_sb[oc][:], s1_p[:], AF.Relu,
                                       bias=sev["seb1"][oc][:],
                                       scale=sev["ses1"][oc][:])
              for mc in range(4):
                  s2_p = pse.tile([128, EE], F32, tag="s2p", name="s2p", bufs=2)
                  for kc in range(2):
                      nc.tensor.matmul(s2_p[:],
                                       sew["fsw2T"][kc][:, mc * 128:(mc + 1) * 128],
                                       s1_sb[kc][:], start=(kc == 0), stop=(kc == 1))
                  sig = pwork.tile([128, EE], BF16, tag="sig", name="sig",
                                   bufs=2)
                  nc.scalar.activation(sig[:], s2_p[:], AF.Sigmoid,
                                       bias=cbb[mc][:], scale=sev["ses2"][mc][:])
                  for ee, lo, hi in ((nc.vector, 0, 11), (nc.gpsimd, 11, E)):
                      outv = fusedp[mc][:].rearrange(
                          "p (i j) -> p i j", j=26)[:, 2 + lo:2 + hi, 2:24]
                      ee.tensor_tensor(
                          out=outv,
                          in0=fmap[mc][:].rearrange("p (i j) -> p i j",
                                                    i=E)[:, lo:hi],
                          in1=sig[:].rearrange("p (i j) -> p i j",
                                               i=E)[:, lo:hi],
                          op=ALU.mult)

        if stages >= 4:
          # ================= stage 4: conv stack =================
          # Row-split pipeline: each conv computes its top (rows 0:11) and
          # bottom (rows 11:22) output halves separately; a half is relu'd
          # and AllGather'd while the next half / next conv keeps the PE
          # busy. Gathered halves land directly in zero-padded 26x26 tiles
          # in fixed rank order (weight chunks are packed in the same rank
          # order), so no masked combines are needed.
          # Row slices (0:8, 8:13, 13:22): the next conv's TOP outputs
          # (rows 0:11) only need input rows <= 12, i.e. the first two
          # slices, so they fully hide the third slice's exchange latency.
          SLICES = [(0, 8), (8, 13), (13, 17), (17, 22)]
          RH = 11 * 22

          def tap_rows(padt, tap, r0, nr):
              dy, dx = tap // 5, tap % 5
              return padt.rearrange("p (i j) -> p i j", j=26)[
                  :, dy + r0:dy + r0 + nr, dx:dx + 22]

          def rd_pair(gpc, r0, nr):
              # interior rows r0:r0+nr of both packed padded images
              return gpc[:].rearrange("p (c i j) -> p c i j", c=2, j=26)[
                  :, :, 2 + r0:2 + r0 + nr, 2:24]

          with tc.tile_pool(name="pcw", bufs=1) as pcw, \
               tc.tile_pool(name="psc", bufs=1, space="PSUM") as psc:
              w2 = []
              for kc in range(2):
                  t = pcw.tile([128, 25 * 128], BF16, tag=f"w2_{kc}",
                               name=f"w2_{kc}")
                  for ch in range(2):
                      nc.gpsimd.dma_start(t[:, ch * 1600:(ch + 1) * 1600],
                                          w2sb_d[kc][:, ch * 1600:(ch + 1) * 1600])
                  w2.append(t)
              w3 = []
              for kc in range(2):
                  t = pcw.tile([128, 25 * 256], BF16, tag=f"w3_{kc}",
                               name=f"w3_{kc}")
                  for ch in range(4):
                      nc.gpsimd.dma_start(t[:, ch * 1600:(ch + 1) * 1600],
                                          w3sb_d[kc][:, ch * 1600:(ch + 1) * 1600])
                  w3.append(t)

              def exchange_slice(stage_sb, dram_pre, gpc, slices, sl_i):
                  """Relu'd slice -> DRAM -> AllGather over the pair -> both
                  packed padded tiles via one 4D-AP read, in fixed rank
                  order. Solo emulates the gather with two direct writes."""
                  r0, r1_ = slices[sl_i]
                  nr = r1_ - r0
                  seg = stage_sb[:, r0 * 22:r1_ * 22]
                  gseg = pdram.tile([256, nr * 22], BF16,
                                    tag=f"{dram_pre}g{sl_i}",
                                    name=f"{dram_pre}g{sl_i}")
                  if solo:
                      nc.sync.dma_start(gseg[0:128, :], seg)
                      nc.scalar.dma_start(gseg[128:256, :], seg)
                  else:
                      bseg = pdram.tile([128, nr * 22], BF16,
                                        tag=f"{dram_pre}b{sl_i}",
                                        name=f"{dram_pre}b{sl_i}")
                      nc.sync.dma_start(bseg[:], seg)
                      nc.gpsimd.collective_compute(
                          "AllGather", ALU.bypass, replica_groups=groups,
                          ins=[bseg[:].opt()], outs=[gseg[:].opt()])
                  gv = gpc[:].rearrange("p (c i j) -> p c i j", c=2, j=26)
                  nc.gpsimd.dma_start(gv[:, 0, 2 + r0:2 + r0 + nr, 2:24],
                                      gseg[0:128, :])
                  nc.gpsimd.dma_start(gv[:, 1, 2 + r0:2 + r0 + nr, 2:24],
                                      gseg[128:256, :])

              def conv_sliced(wsel, srcs, nkc, stage_sb, bias, dram_pre,
                              gpc, slices):
                  """One conv layer: compute the row slices, relu each into
                  stage_sb and exchange it as soon as it's ready."""
                  for sl_i, (r0, r1_) in enumerate(slices):
                      nr = r1_ - r0
                      cp = psc.tile([128, RH], F32, tag="cp", name="cp",
                                    bufs=4)
                      cpv = cp[:, 0:nr * 22]
                      k = 0
                      for tap in range(25):
                          for kc in range(nkc):
                              nc.tensor.matmul(
                                  cpv, wsel(kc, tap),
                                  tap_rows(srcs[kc], tap, r0, nr),
                                  start=(k == 0), stop=(k == 25 * nkc - 1))
                              k += 1
                      nc.scalar.activation(stage_sb[:, r0 * 22:r1_ * 22], cpv,
                                           AF.Relu, bias=bias)
                      exchange_slice(stage_sb, dram_pre, gpc, slices, sl_i)

              # ---- conv1: fusedp -> 128 out-ch (my half) ----
              r1s = pcw.tile([128, EE], BF16, tag="r1s", name="r1s")
              conv_sliced(
                  lambda kc, tap: w1[kc][:, tap * 128:(tap + 1) * 128],
                  [t[:] for t in fusedp], 4, r1s, b1h[:], "r1", g1pc,
                  [(0, 8), (8, 13), (13, 17), (17, 22)])

              # ---- conv2: g1p -> 128 out-ch (my half) ----
              r2s = pcw.tile([128, EE], BF16, tag="r2s", name="r2s")
              conv_sliced(
                  lambda kc, tap: w2[kc][:, tap * 128:(tap + 1) * 128],
                  g1p, 2, r2s, b2h[:], "r2", g2pc,
                  [(0, 8), (8, 13), (13, 22)])

              # ---- conv3: g2p -> 256 out-ch (my half), two half-rows per
              # out chunk; both top chunks first (they only need conv2's
              # first two slices), hiding the last conv2 exchange ----
              for (oc, hh) in ((0, 0), (1, 0), (0, 1), (1, 1)):
                  cp = psc.tile([128, RH], F32, tag="cp", name="cp", bufs=4)
                  k = 0
                  for tap in range(25):
                      for kc in range(2):
                          nc.tensor.matmul(
                              cp[:],
                              w3[kc][:, tap * 256 + oc * 128:
                                     tap * 256 + (oc + 1) * 128],
                              tap_rows(g2p[kc], tap, hh * 11, 11),
                              start=(k == 0), stop=(k == 49))
                          k += 1
                  # finer output slices at the very end shorten the tail
                  nslice = 2 if (oc == 1 and hh == 1) else 1
                  rows = [(0, 6), (6, 11)] if nslice == 2 else [(0, 11)]
                  for ri, (ra, rb) in enumerate(rows):
                      o_sb = pcw.tile([128, RH], F32, tag="osb",
                                      name="osb", bufs=3)
                      ov = o_sb[:, 0:(rb - ra) * 22]
                      nc.scalar.activation(ov, cp[:, ra * 22:rb * 22],
                                           AF.Relu, bias=b3h[oc][:])
                      eng = nc.sync if (oc + hh + ri) % 2 == 0 else nc.scalar
                      eng.dma_start(
                          out_d[oc * 128:(oc + 1) * 128,
                                hh * RH + ra * 22:hh * RH + rb * 22], ov)

    nc.compile()
    return nc


_NC_CACHE = None


def _get_program():
    global _NC_CACHE
    if _NC_CACHE is None:
        _NC_CACHE = build_program()
    return _NC_CACHE


def _bf(a):
    return np.ascontiguousarray(a.astype(ml_dtypes.bfloat16))


def _prep_shared(w):
    """Packed weights/constants identical on every core."""
    ADJ = _build_adj()
    out = {}
    constb = np.zeros((128, _CB), np.float32)

    def put(nm, arr):
        c0, cols = _LAY_B[nm]
        r, cc = arr.shape
        constb[0:r, c0:c0 + cc] = arr
    wt = w['W_trans']
    for kc in range(6):
        put(f"wtr{kc}", wt[kc * 128:(kc + 1) * 128])
    put("brow", w['b_trans'].reshape(1, EMB))
    put("onesrow", np.ones((1, 128), np.float32))
    put("onescol", np.ones((128, 1), np.float32))
    g2T = np.zeros((EM, E), np.float32)
    for e in range(E):
        g2T[e * M:(e + 1) * M, e] = 1.0
    put("g2T", g2T)
    sumT = np.kron(np.eye(L, dtype=np.float32), np.ones((SPAN, 1), np.float32))
    for kc in range(4):
        put(f"sumT{kc}", sumT[kc * 128:(kc + 1) * 128])
    for nm, arr, nch in (("fsw1T", w['fs_w1'].T, 4), ("fcw1T", w['fc_w1'].T, 4),
                         ("fsw2T", w['fs_w2'].T, 2), ("fcw2T", w['fc_w2'].T, 2)):
        for kc in range(nch):
            put(f"{nm}{kc}", np.ascontiguousarray(arr[kc * 128:(kc + 1) * 128]))
    out['constb'] = _bf(constb)

    gT = np.zeros((EMH, E), np.float32)
    for e in range(E):
        gT[e * M * H:(e + 1) * M * H, e] = 1.0 / (M * H)
    gTb = np.zeros((128, 9 * E), np.float32)
    for kc in range(9):
        r = min(128, EMH - kc * 128)
        gTb[0:r, kc * E:(kc + 1) * E] = gT[kc * 128:kc * 128 + r]
    out['gTb'] = np.ascontiguousarray(gTb.astype(ml_dtypes.float8_e4m3))
    out['aallTb'] = _bf(np.concatenate(
        [ADJ[r].T for r in range(NREL)] + [np.eye(NN, dtype=np.float32)],
        axis=1))
    out['tfb'] = _bf(np.ascontiguousarray(w['type_embed'][_TYPES]))
    out['identb'] = _bf(np.eye(128, dtype=np.float32))

    constf = np.zeros((128, _CF), np.float32)

    def putf(nm, arr):
        c0, cols = _LAY_F[nm]
        constf[0:arr.shape[0], c0:c0 + 1] = arr.reshape(-1, 1)
    vecs = {"ses1": w['fs_g1'], "seb1": w['fs_b1'] * w['fs_g1'] + w['fs_be1'],
            "fcs1": w['fc_g1'], "fcb1": w['fc_b1'] * w['fc_g1'] + w['fc_be1'],
            "ses2": w['fs_g2'], "seb2": w['fs_b2'] * w['fs_g2'] + w['fs_be2'],
            "fcs2": w['fc_g2'],
            "fcb2": w['fc_b2'] * w['fc_g2'] + w['fc_be2'] +
                    w['fs_b2'] * w['fs_g2'] + w['fs_be2']}
    for nm, v in vecs.items():
        nch = 2 if v.shape[0] == INTER else 4
        for kc in range(nch):
            putf(f"{nm}{kc}", v[kc * 128:(kc + 1) * 128])
    out['constf_base'] = constf

    for layer in range(NLAYERS):
        din_l = D0 if layer == 0 else EMB
        kcs = _KC0 if layer == 0 else _KC1
        nk = len(kcs)
        Wst = w['rgcn_Wrel0'].reshape(NREL * D0, EMB) if layer == 0 else \
            w['rgcn_Wrel'][layer - 1].reshape(NREL * EMB, EMB)
        Wself = w['rgcn_Wself0'] if layer == 0 else w['rgcn_Wself'][layer - 1]
        p = np.zeros((128, (NREL + 1) * nk * EMB), np.float32)
        for si, (s0, sl) in enumerate(kcs):
            for r in range(NREL):
                b = si * (NREL + 1) + r
                p[0:sl, b * EMB:(b + 1) * EMB] = \
                    Wst[r * din_l + s0:r * din_l + s0 + sl]
            b = si * (NREL + 1) + NREL
            p[0:sl, b * EMB:(b + 1) * EMB] = Wself[s0:s0 + sl]
        out[f'wstp{layer}'] = _bf(p)
    return out


def _prep_conv_half(w, half, constf_base):
    out = {}
    w1 = w['cr_w1'][half * 128:(half + 1) * 128]
    out['w1sb'] = _bf(np.ascontiguousarray(
        w1.transpose(1, 2, 3, 0).reshape(4, 128, 25 * 128)))
    # conv2/conv3 weight chunks in natural (rank-ordered) input-half order
    w2 = w['cr_w2'][half * 128:(half + 1) * 128]
    out['w2sb'] = _bf(np.ascontiguousarray(
        w2.transpose(1, 2, 3, 0).reshape(2, 128, 25 * 128)))
    w3 = w['cr_w3'][half * 256:(half + 1) * 256]
    out['w3sb'] = _bf(np.ascontiguousarray(
        w3.transpose(1, 2, 3, 0).reshape(2, 128, 25 * 256)))
    constf = constf_base.copy()

    def putf(nm, arr):
        c0, cols = _LAY_F[nm]
        constf[0:arr.shape[0], c0:c0 + 1] = arr.reshape(-1, 1)
    putf("b1h", w['cr_b1'][half * 128:(half + 1) * 128])
    putf("b2h", w['cr_b2'][half * 128:(half + 1) * 128])
    putf("b3h0", w['cr_b3'][half * 256:half * 256 + 128])
    putf("b3h1", w['cr_b3'][half * 256 + 128:half * 256 + 256])
    putf("mtop", np.full(128, float(half), np.float32))
    putf("mbot", np.full(128, float(1 - half), np.float32))
    c0, cols = _LAY_F["identf"]
    constf[:, c0:c0 + 128] = np.eye(128, dtype=np.float32)
    out['constf'] = constf
    return out


def _prep_doc(x, att, mi, ls):
    out = {}
    mif = mi.reshape(EM)
    attm = np.ascontiguousarray(
        att[:, mif, :].transpose(1, 0, 2).reshape(EMH, C))
    amp = np.zeros((128, 9 * C), np.float32)
    for kc in range(9):
        r = min(128, EMH - kc * 128)
        amp[0:r, kc * C:kc * C + C] = attm[kc * 128:kc * 128 + r]
    out['amp'] = np.ascontiguousarray(amp.astype(ml_dtypes.float8_e4m3))
    idx = ls[:, None] + np.arange(SPAN)
    idxf = idx.reshape(LS)
    rows = att[:, idxf, :].reshape(H, L, SPAN, C)
    blocks = np.take_along_axis(rows, idx[None, :, None, :], axis=3)
    attl = blocks.transpose(0, 2, 1, 3).reshape(HS, LS)
    xmT = x[mif].T
    xspT = x[idxf].T
    actb = np.zeros((128, _CA), np.float32)

    def put(nm, arr):
        c0, cols = _LAY_A[nm]
        actb[0:arr.shape[0], c0:c0 + arr.shape[1]] = arr
    for kc in range(6):
        put(f"xmT{kc}", xmT[kc * 128:(kc + 1) * 128])
        put(f"xspT{kc}", xspT[kc * 128:(kc + 1) * 128])
    for kc in range(3):
        put(f"attl{kc}", attl[kc * 128:(kc + 1) * 128])
    out['actb'] = _bf(actb)
    xpk = np.zeros((128, 8 * HID), np.float32)
    for kc in range(8):
        xpk[:, kc * HID:(kc + 1) * HID] = x[kc * 128:(kc + 1) * 128]
    out['xp'] = _bf(xpk)
    return out


def build_in_maps(inputs):
    w = {}
    for k, v in inputs.items():
        a = np.asarray(v)
        w[k] = a if a.dtype in (np.int32, np.int64) else \
            np.asarray(a, np.float32)
    shared = _prep_shared(w)
    constf_base = shared.pop('constf_base')
    halves = [_prep_conv_half(w, h, constf_base) for h in range(2)]
    seq = np.asarray(inputs['sequence_output'], np.float32)
    att = np.asarray(inputs['attention'], np.float32)
    mi = np.asarray(inputs['mention_idx']).astype(np.int64)
    ls = np.asarray(inputs['link_start']).astype(np.int64)
    docs = [_prep_doc(seq[n], att[n], mi[n], ls[n]) for n in range(NB)]
    in_maps = []
    for core in range(N_CORES):
        n, half = core // 2, core % 2
        m = dict(shared)
        m.update(halves[half])
        m.update(docs[n])
        in_maps.append({k: (np.ascontiguousarray(v)
                            if v.dtype in (ml_dtypes.bfloat16,
                                           ml_dtypes.float8_e4m3)
                            else np.ascontiguousarray(v, np.float32))
                        for k, v in m.items()})
    return in_maps


def kernel(**inputs):
    nc = _get_program()
    in_maps = build_in_maps(inputs)
    res = run_bass_kernel_spmd(nc, in_maps, list(range(N_CORES)))
    out = np.zeros((NB, EMB, E, E), np.float32)
    for core in range(N_CORES):
        n, half = core // 2, core % 2
        out[n, half * 256:(half + 1) * 256] = \
            res.results[core]["out"].reshape(256, E, E)
    return out


# revision 51
# speedup vs baseline: 1.3461x; 1.0253x over previous
"""Trainium2 Bass kernel for nn_DocREModel (DocRE: gather -> RGCN -> SE -> 5x5 convs).

Sharding: 4 documents x 2 cores each. Each pair replicates the cheap upstream
(mention/link/ea gathers -> RGCN -> fmap/SE) and splits the dominant 5x5 conv
stack by output channels, with two intra-pair AllGathers; output halves are
assembled on host. All index-driven gathers happen on host (pure data
movement; one SPMD program serves all 8 cores), all dense math on device.

Perf model notes (TimelineSim): all DMAs serialize on one ~332 GB/s pipe in
~issue order, and the PE p-state ramp rewards keeping the tensor engine
continuously fed. Hence: everything DMA'd is bf16 (f32 only for small
per-channel scale/bias vectors), tensors are issued strictly in first-use
order (amp/gTb first so the ea matmuls start ~2.5us in), RGCN + conv weights
stream just-in-time behind the compute, and h0 is assembled directly by ACT
writes into a bf16 tile instead of SBUF->SBUF DMA round trips.

Precision/layout choices:
- bf16 weights+activations everywhere on the matmul path, f32 PSUM
  accumulation throughout; per-channel BN scales/biases stay f32.
- Convs are 25 shift-tap matmuls over zero-padded 26x26 images via strided
  APs (no im2col copies). conv2/conv3 start on the locally-computed input
  half before the pair AllGather completes; the other half is extracted
  SPMD-safely with host-supplied 0/1 masks and per-core (own, other)
  weight-chunk ordering.
- RGCN folds the self-loop in as a 4th identity relation so each layer is
  one u = h^T @ [A0^T|A1^T|A2^T|I] matmul plus one PSUM accumulation over
  stacked (relation, chunk) weights -- no transposes in the loop.
"""

import numpy as np
import ml_dtypes

import concourse.bacc as bacc
import concourse.tile as tile
from concourse import mybir
from concourse.bass_utils import run_bass_kernel_spmd

F32 = mybir.dt.float32
F32R = mybir.dt.float32r
BF16 = mybir.dt.bfloat16
F8 = mybir.dt.float8e4
AF = mybir.ActivationFunctionType
ALU = mybir.AluOpType

NB, H, C, HID, EMB = 4, 12, 1024, 768, 512
E, M, L, SPAN = 22, 4, 16, 32
TD, INTER = 20, 256
NN = E + E * M + L
NREL, NLAYERS = 3, 4
EM, EMH, HS, LS = E * M, E * M * H, H * SPAN, L * SPAN
D0 = EMB + TD           # 532
EE = E * E              # 484
PADW = 26 * 26          # 676 padded 26x26 image
N_CORES = 8


def _build_adj():
    A = np.zeros((NREL, NN, NN), np.float32)
    for e in range(E):
        for m in range(M):
            mi = E + e * M + m
            A[0, e, mi] = A[0, mi, e] = 1.0
            for m2 in range(M):
                if m2 != m:
                    A[1, mi, E + e * M + m2] = 1.0
            li = E + E * M + ((e * M + m) % L)
            A[2, mi, li] = A[2, li, mi] = 1.0
    A = A / (A.sum(-1, keepdims=True) + 1e-5)
    return A


_TYPES = np.concatenate([np.zeros(E, np.int32), np.ones(EM, np.int32),
                         np.full(L, 2, np.int32)])

_KC0 = [(0, 128), (128, 128), (256, 128), (384, 128), (512, 20)]   # 532 rows
_KC1 = [(0, 128), (128, 128), (256, 128), (384, 128)]              # 512 rows


def _constb_layout():
    """Column layout of the packed bf16 constant tensor [128, CB].

    Part A (cols 0:CBA) is everything needed through stage 3's s1/c1;
    part B (fsw2T/fcw2T) is DMA'd later, after the RGCN weights.
    """
    lay = {}
    c = 0

    def add(nm, cols):
        nonlocal c
        lay[nm] = (c, cols)
        c += cols
    for kc in range(6):
        add(f"wtr{kc}", EMB)
    add("brow", EMB)
    add("onesrow", 128)
    add("onescol", 1)
    add("g2T", E)
    for kc in range(4):
        add(f"sumT{kc}", L)
    for kc in range(4):
        add(f"fsw1T{kc}", INTER)
    for kc in range(4):
        add(f"fcw1T{kc}", INTER)
    cba = c
    for kc in range(2):
        add(f"fsw2T{kc}", EMB)
    for kc in range(2):
        add(f"fcw2T{kc}", EMB)
    return lay, c, cba


def _constf_layout():
    lay = {}
    c = 0

    def add(nm, cols):
        nonlocal c
        lay[nm] = (c, cols)
        c += cols
    for nm, nch in (("ses1", 2), ("seb1", 2), ("fcs1", 2), ("fcb1", 2),
                    ("ses2", 4), ("seb2", 4), ("fcs2", 4), ("fcb2", 4)):
        for kc in range(nch):
            add(f"{nm}{kc}", 1)
    add("b1h", 1)
    add("b2h", 1)
    add("b3h0", 1)
    add("b3h1", 1)
    add("mtop", 1)
    add("mbot", 1)
    add("identf", 128)
    return lay, c


def _actb_layout():
    lay = {}
    c = 0

    def add(nm, cols):
        nonlocal c
        lay[nm] = (c, cols)
        c += cols
    for kc in range(6):
        add(f"xmT{kc}", EM)
    for kc in range(6):
        add(f"xspT{kc}", LS)
    for kc in range(3):
        add(f"attl{kc}", LS)
    return lay, c


_LAY_B, _CB, _CBA = _constb_layout()
_LAY_F, _CF = _constf_layout()
_LAY_A, _CA = _actb_layout()


def build_program(solo=False, stages=4):
    nc = bacc.Bacc("TRN2", target_bir_lowering=False, debug=False)

    def din(name, shape, dt=BF16):
        return nc.dram_tensor(name, list(shape), dt, kind="ExternalInput").ap()

    constb_d = din("constb", [128, _CB])
    constf_d = din("constf", [128, _CF], F32)
    actb_d = din("actb", [128, _CA])
    xp_d = din("xp", [128, 8 * HID])
    amp_d = din("amp", [128, 9 * C], F8)
    gTb_d = din("gTb", [128, 9 * E], F8)
    tfb_d = din("tfb", [NN, TD])
    wstp_d = [din("wstp0", [128, 20 * EMB])] + \
             [din(f"wstp{i}", [128, 16 * EMB]) for i in (1, 2, 3)]
    w1sb_d = din("w1sb", [4, 128, 25 * 128])
    w2sb_d = din("w2sb", [2, 128, 25 * 128])
    w3sb_d = din("w3sb", [2, 128, 25 * 256])
    aallTb_d = din("aallTb", [NN, (NREL + 1) * NN])
    identb_d = din("identb", [128, 128])

    out_d = nc.dram_tensor("out", [256, EE], F32, kind="ExternalOutput").ap()

    groups = [[0, 1], [2, 3], [4, 5], [6, 7]]

    with tile.TileContext(nc) as tc:
      with tc.tile_pool(name="pconst", bufs=1) as pconst, \
           tc.tile_pool(name="pwork", bufs=1) as pwork, \
           tc.tile_pool(name="pdram", bufs=1, space="DRAM") as pdram:

        constb = pconst.tile([128, _CB], BF16)
        constf = pconst.tile([128, _CF], F32)
        identb = pconst.tile([128, 128], BF16)
        aallTb = pconst.tile([NN, (NREL + 1) * NN], BF16)
        aallE = pconst.tile([E, (NREL + 1) * NN], BF16)
        aallM = pconst.tile([EM, (NREL + 1) * NN], BF16)
        aallL = pconst.tile([L, (NREL + 1) * NN], BF16)
        wstp_t = [pconst.tile([128, 20 * EMB], BF16, tag="wstp0",
                              name="wstp0")] + \
                 [pconst.tile([128, 16 * EMB], BF16, tag=f"wstp{l}",
                              name=f"wstp{l}") for l in (1, 2, 3)]
        w1 = [pconst.tile([128, 25 * 128], BF16, tag=f"w1_{kc}",
                          name=f"w1_{kc}") for kc in range(4)]

        def cb(nm, rows=128):
            c0, cols = _LAY_B[nm]
            return constb[0:rows, c0:c0 + cols]

        def cf(nm, rows=128):
            c0, cols = _LAY_F[nm]
            return constf[0:rows, c0:c0 + cols]

        wtr = [cb(f"wtr{kc}") for kc in range(6)]
        brow = cb("brow", rows=1)
        onesrow = cb("onesrow", rows=1)
        onescol = cb("onescol")
        g2T = cb("g2T", rows=EM)
        sumT = [cb(f"sumT{kc}") for kc in range(4)]
        sew = {nm: [cb(f"{nm}{kc}") for kc in range(n)]
               for nm, n in (("fsw1T", 4), ("fcw1T", 4), ("fsw2T", 2),
                             ("fcw2T", 2))}
        sev = {nm: [cf(f"{nm}{kc}") for kc in range(n)]
               for nm, n in (("ses1", 2), ("seb1", 2), ("fcs1", 2), ("fcb1", 2),
                             ("ses2", 4), ("seb2", 4), ("fcs2", 4),
                             ("fcb2", 4))}
        b1h = cf("b1h")
        b2h = cf("b2h")
        b3h = [cf("b3h0"), cf("b3h1")]
        ident = cf("identf")

        # persistent intermediates (three base-0 tiles: engines cannot
        # write SBUF at unaligned base partitions, so the node matrix is
        # kept split as [entities; mentions; links])
        h0e = pwork.tile([E, D0], BF16)
        h0m = pwork.tile([EM, D0], BF16)
        h0l = pwork.tile([L, D0], BF16)
        hfin = pwork.tile([NN, EMB], BF16)
        ectxT_sb = [pwork.tile([128, E], F32, tag=f"ectxT{i}", name=f"ectxT{i}")
                    for i in range(4)]
        ecT = [pwork.tile([128, E], F32R, tag=f"ecT{i}", name=f"ecT{i}")
               for i in range(4)]
        # PE warmup fodder: covers the head until real operands land (the
        # scheduler hoists dependency-free matmuls to the front).
        warm = pwork.tile([128, 512], BF16)
        nc.vector.memset(warm[:], 0.0)
        fusedp = [pwork.tile([128, PADW], BF16, tag=f"fusedp{i}",
                             name=f"fusedp{i}") for i in range(4)]
        g1pc = pwork.tile([128, 2 * PADW], BF16, tag="g1pc", name="g1pc")
        g2pc = pwork.tile([128, 2 * PADW], BF16, tag="g2pc", name="g2pc")
        g1p = [g1pc[:, i * PADW:(i + 1) * PADW] for i in range(2)]
        g2p = [g2pc[:, i * PADW:(i + 1) * PADW] for i in range(2)]
        for t_ in fusedp:
            nc.vector.memset(t_[:], 0.0)
        nc.vector.memset(g1pc[:], 0.0)
        nc.vector.memset(g2pc[:], 0.0)

        with tc.tile_pool(name="pbig", bufs=1) as pbig:
            gTb = pbig.tile([128, 9 * E], F8)
            amp = pbig.tile([128, 9 * C], F8)
            xp = pbig.tile([128, 8 * HID], BF16)
            actb = pbig.tile([128, _CA], BF16)

            # ---- the bulk DMA stream rides the SWDGE (gpsimd) ring in
            # first-use order; sync/scalar stay shallow for latency-
            # critical transfers later (conv exchanges, outputs) ----
            nc.scalar.dma_start(constf[:], constf_d[:])
            xm_cols = 6 * EM                      # xmT region of actb
            wtr_cols = 6 * EMB + EMB + 128 + 1    # wtr+brow+ones region
            nc.gpsimd.dma_start(actb[:, 0:xm_cols], actb_d[:, 0:xm_cols])
            nc.gpsimd.dma_start(constb[:, 0:wtr_cols], constb_d[:, 0:wtr_cols])
            se1_cols = wtr_cols + E + 4 * L   # g2T+sumT end
            nc.gpsimd.dma_start(constb[:, wtr_cols:se1_cols],
                                constb_d[:, wtr_cols:se1_cols])
            sp_cols = xm_cols + 6 * LS
            nc.gpsimd.dma_start(actb[:, xm_cols:sp_cols],
                                actb_d[:, xm_cols:sp_cols])
            nc.gpsimd.dma_start(actb[:, sp_cols:_CA], actb_d[:, sp_cols:_CA])
            nc.scalar.dma_start(h0e[:, EMB:D0], tfb_d[0:E, :])
            nc.scalar.dma_start(h0m[:, EMB:D0], tfb_d[E:E + EM, :])
            nc.scalar.dma_start(h0l[:, EMB:D0], tfb_d[E + EM:NN, :])
            nc.gpsimd.dma_start(aallTb[:], aallTb_d[:])
            nc.gpsimd.dma_start(aallE[:], aallTb_d[0:E, :])
            nc.gpsimd.dma_start(aallM[:], aallTb_d[E:E + EM, :])
            nc.gpsimd.dma_start(aallL[:], aallTb_d[E + EM:NN, :])
            # RGCN weights, chunked si-major so each layer's PSUM chain can
            # start as soon as its first chunk lands
            BL = (NREL + 1) * EMB

            def wstp_dma(layer):
                nchunks = 5 if layer == 0 else 4
                for si in range(nchunks):
                    nc.gpsimd.dma_start(
                        wstp_t[layer][:, si * BL:(si + 1) * BL],
                        wstp_d[layer][:, si * BL:(si + 1) * BL])
            wstp_dma(0)
            wstp_dma(1)
            nc.gpsimd.dma_start(gTb[:], gTb_d[:])
            for g in range(3):
                nc.gpsimd.dma_start(amp[:, g * 3 * C:(g + 1) * 3 * C],
                                    amp_d[:, g * 3 * C:(g + 1) * 3 * C])
            nc.gpsimd.dma_start(xp[:], xp_d[:])
            nc.gpsimd.dma_start(constb[:, se1_cols:_CBA],
                                constb_d[:, se1_cols:_CBA])
            wstp_dma(2)
            wstp_dma(3)
            nc.gpsimd.dma_start(constb[:, _CBA:_CB], constb_d[:, _CBA:_CB])
            nc.scalar.dma_start(identb[:], identb_d[:])
            for kc in range(4):
                nc.gpsimd.dma_start(w1[kc][:], w1sb_d[kc])

            # ========== stage 1a: mention/span/link rows -> h0b ==========
            expm = pbig.tile([EM, EMB], BF16)
            sp_ps = []
            wsb = [pbig.tile([128, 1], F32, tag=f"wsb{i}", name=f"wsb{i}")
                   for i in range(4)]
            wsp = [pbig.tile([128, EMB], BF16, tag=f"wsp{i}", name=f"wsp{i}")
                   for i in range(4)]

            def ca(nm, rows=128):
                c0, cols = _LAY_A[nm]
                return actb[0:rows, c0:c0 + cols]

            xmT = [ca(f"xmT{kc}") for kc in range(6)]
            xspT = [ca(f"xspT{kc}") for kc in range(6)]
            attl = [ca(f"attl{kc}") for kc in range(3)]

            with tc.tile_pool(name="ps1a", bufs=1, space="PSUM") as ps1a:
                jp = ps1a.tile([128, 512], F32, tag="jp", name="jp")
                for _ in range(14):
                    nc.tensor.matmul(jp[:], warm[:, 0:128], warm[:],
                                     start=True, stop=True)
                # mentions: mrep = x_m @ Wtr + b -> h0b rows + exp for pooling
                mrep_p = ps1a.tile([EM, EMB], F32, tag="mrep", name="mrep")
                for kc in range(6):
                    nc.tensor.matmul(mrep_p[:], xmT[kc][:, 0:EM], wtr[kc][:],
                                     start=(kc == 0), stop=False)
                nc.tensor.matmul(mrep_p[:], onesrow[0:1, 0:EM], brow[:],
                                 start=False, stop=True)
                nc.scalar.copy(h0m[:, 0:EMB], mrep_p[:])
                nc.scalar.activation(expm[:], mrep_p[:], AF.Exp)
                # e_rep = ln(G2 @ exp(mrep))
                ep_p = ps1a.tile([E, EMB], F32, tag="ep", name="ep")
                nc.tensor.matmul(ep_p[:], g2T[:], expm[:], start=True, stop=True)
                nc.scalar.activation(h0e[:, 0:EMB], ep_p[:], AF.Ln)
                # dummy: switch the ACT table to the sigmoid set now (exp/ln
                # are done) so stage 3's sigmoid doesn't pay the 1.3us load
                sigwarm = pbig.tile([1, 1], F32)
                nc.scalar.activation(sigwarm[:], ep_p[0:1, 0:1], AF.Sigmoid)

                # spans: sp = x_span @ Wtr + b
                for mc in range(4):
                    sp_p = ps1a.tile([128, EMB], F32, tag="sp_p", name="sp_p",
                                     bufs=3)
                    for kc in range(6):
                        nc.tensor.matmul(sp_p[:],
                                         xspT[kc][:, mc * 128:(mc + 1) * 128],
                                         wtr[kc][:], start=(kc == 0), stop=False)
                    nc.tensor.matmul(sp_p[:], onesrow[:], brow[:],
                                     start=False, stop=True)
                    spc = pbig.tile([128, EMB], BF16, tag="spc", name="spc",
                                    bufs=4)
                    nc.scalar.copy(spc[:], sp_p[:])
                    sp_ps.append(spc)
                # w = colsum(attl) / 384
                for mc in range(4):
                    w_p = ps1a.tile([128, 1], F32, tag="w_p", name="w_p", bufs=1)
                    for kc in range(3):
                        nc.tensor.matmul(w_p[:],
                                         attl[kc][:, mc * 128:(mc + 1) * 128],
                                         onescol[:],
                                         start=(kc == 0), stop=(kc == 2))
                    nc.scalar.activation(wsb[mc][:], w_p[:], AF.Copy,
                                         scale=1.0 / (H * SPAN))
                # wsp = psum(sp) * w ; link = SUM^T @ wsp
                for mc in range(4):
                    nc.vector.tensor_scalar(out=wsp[mc][:], in0=sp_ps[mc][:],
                                            scalar1=wsb[mc][:], scalar2=None,
                                            op0=ALU.mult)
                link_p = ps1a.tile([L, EMB], F32, tag="link", name="link")
                for kc in range(4):
                    nc.tensor.matmul(link_p[:], sumT[kc][:], wsp[kc][:],
                                     start=(kc == 0), stop=(kc == 3))
                nc.scalar.copy(h0l[:, 0:EMB], link_p[:])

            # ====== stage 2 + stage 1b interleaved: the ea/e_ctx latency
            # chain fills the RGCN's weight-stream stalls ======
            ea_sb = pbig.tile([E, C], F32R)
            eaT = [pbig.tile([128, E], BF16, tag=f"eaT{i}", name=f"eaT{i}")
                   for i in range(8)]
            z_sb = [pbig.tile([128, E], BF16, tag=f"z{i}", name=f"z{i}")
                    for i in range(6)]
            easumT = pbig.tile([1, E], BF16)

            if stages >= 2:
              with tc.tile_pool(name="prg", bufs=2) as prg, \
                   tc.tile_pool(name="psr", bufs=1, space="PSUM") as psr:

                def rgcn_layer(layer, h):
                    kcs = _KC0 if layer == 0 else _KC1
                    nk = len(kcs)
                    wstp = wstp_t[layer]
                    # si-major packing: block (si, r) at (si*(NREL+1)+r)*EMB
                    wst_t = [wstp[:, (si * (NREL + 1) + r) * EMB:
                                   (si * (NREL + 1) + r + 1) * EMB]
                             for r in range(NREL + 1) for si in range(nk)]
                    u_sb = []
                    for si, (s0, sl) in enumerate(kcs):
                        u_p = psr.tile([128, (NREL + 1) * NN], F32, tag="u_p",
                                       name="u_p", bufs=2)
                        if layer == 0:
                            nc.tensor.matmul(u_p[0:sl, :],
                                             h0e[:, s0:s0 + sl], aallE[:],
                                             start=True, stop=False)
                            nc.tensor.matmul(u_p[0:sl, :],
                                             h0m[:, s0:s0 + sl], aallM[:],
                                             start=False, stop=False)
                            nc.tensor.matmul(u_p[0:sl, :],
                                             h0l[:, s0:s0 + sl], aallL[:],
                                             start=False, stop=True)
                        else:
                            nc.tensor.matmul(u_p[0:sl, :], h[0:NN, s0:s0 + sl],
                                             aallTb[:], start=True, stop=True)
                        u = prg.tile([128, (NREL + 1) * NN], BF16, tag=f"u{si}",
                                     name=f"u{si}", bufs=1)
                        if si % 2 == 0:
                            nc.scalar.copy(u[0:sl, :], u_p[0:sl, :])
                        else:
                            nc.vector.tensor_copy(out=u[0:sl, :],
                                                  in_=u_p[0:sl, :])
                        u_sb.append(u)
                    y_p = psr.tile([NN, EMB], F32, tag="y_p", name="y_p")
                    n_mm = (NREL + 1) * nk
                    k_mm = 0
                    for si, (s0, sl) in enumerate(kcs):
                        for r in range(NREL + 1):
                            nc.tensor.matmul(
                                y_p[:], u_sb[si][0:sl, r * NN:(r + 1) * NN],
                                wst_t[r * nk + si][0:sl, :],
                                start=(k_mm == 0), stop=(k_mm == n_mm - 1))
                            k_mm += 1
                    hdst = hfin if layer == NLAYERS - 1 else \
                        prg.tile([NN, EMB], BF16, tag="h_next", name="h_next")
                    for (s0, sl) in _KC1:
                        nc.scalar.activation(hdst[0:NN, s0:s0 + sl],
                                             y_p[0:NN, s0:s0 + sl], AF.Relu)
                    return hdst

                h1 = rgcn_layer(0, None)

                # -- ea block (runs while wstp1 streams) --
                with tc.tile_pool(name="ps1b", bufs=1, space="PSUM") as ps1b:
                    ea_p0 = ps1b.tile([E, 512], F32, tag="ea0", name="ea0")
                    ea_p1 = ps1b.tile([E, 512], F32, tag="ea1", name="ea1")
                    for kc in range(9):
                        rows = 128 if kc < 8 else 32
                        at = amp[0:rows, kc * C:kc * C + C]
                        gt = gTb[0:rows, kc * E:(kc + 1) * E]
                        nc.tensor.matmul(ea_p0[:], gt, at[:, 0:512],
                                         start=(kc == 0), stop=(kc == 8))
                        nc.tensor.matmul(ea_p1[:], gt, at[:, 512:1024],
                                         start=(kc == 0), stop=(kc == 8))
                    r0 = pbig.tile([E, 1], F32)
                    r1 = pbig.tile([E, 1], F32)
                    nc.vector.tensor_reduce(r0[:], ea_p0[:],
                                            mybir.AxisListType.X, ALU.add)
                    nc.vector.tensor_reduce(r1[:], ea_p1[:],
                                            mybir.AxisListType.X, ALU.add)
                    rsum = pbig.tile([E, 1], F32)
                    nc.vector.tensor_tensor(out=rsum[:], in0=r0[:], in1=r1[:],
                                            op=ALU.add)
                    rsum2 = pbig.tile([E, 1], F32)
                    nc.vector.tensor_scalar(out=rsum2[:], in0=rsum[:],
                                            scalar1=1e-5, scalar2=None,
                                            op0=ALU.add)
                    rinv = pbig.tile([E, 1], F32)
                    nc.vector.reciprocal(rinv[:], rsum2[:])
                    for kc in range(4):
                        c0, c1_ = kc * 128, (kc + 1) * 128
                        if kc % 2 == 0:
                            nc.scalar.copy(ea_sb[:, c0:c1_], ea_p0[:, c0:c1_])
                            nc.scalar.copy(ea_sb[:, 512 + c0:512 + c1_],
                                           ea_p1[:, c0:c1_])
                        else:
                            nc.vector.tensor_copy(out=ea_sb[:, c0:c1_],
                                                  in_=ea_p0[:, c0:c1_])
                            nc.vector.tensor_copy(
                                out=ea_sb[:, 512 + c0:512 + c1_],
                                in_=ea_p1[:, c0:c1_])
                    easum = pbig.tile([E, 1], F32)
                    nc.vector.tensor_tensor(out=easum[:], in0=rsum[:],
                                            in1=rinv[:], op=ALU.mult)
                    # eaT transposes reuse the (now dead) ea psum banks
                    for kc in range(8):
                        tp = ps1b.tile([128, E], F32, tag=f"ea{kc % 2}",
                                       name="eaTt")
                        nc.tensor.transpose(tp[:],
                                            ea_sb[:, kc * 128:(kc + 1) * 128]
                                            .bitcast(F32), ident[0:E, 0:E])
                        if kc % 2 == 0:
                            nc.scalar.copy(eaT[kc][:], tp[:])
                        else:
                            nc.vector.tensor_copy(out=eaT[kc][:], in_=tp[:])
                    tp = ps1b.tile([1, E], F32, tag="ea1", name="easumt")
                    nc.tensor.transpose(tp[:], easum[:], ident[0:E, 0:E])
                    nc.scalar.copy(easumT[:], tp[:])

                h2 = rgcn_layer(1, h1)

                with tc.tile_pool(name="ps1c", bufs=1, space="PSUM") as ps1c:
                    # zT = ea_n @ x  [22, 768] (two 384-wide halves)
                    zt_ps = [ps1c.tile([E, 384], F32, tag="sc",
                                       name=f"zt_p{i}", bufs=2)
                             for i in range(2)]
                    for kc in range(8):
                        xt = xp[:, kc * HID:(kc + 1) * HID]
                        for hh in range(2):
                            nc.tensor.matmul(zt_ps[hh][:], eaT[kc][:],
                                             xt[:, hh * 384:(hh + 1) * 384],
                                             start=(kc == 0), stop=(kc == 7))
                    # ea was left unnormalized; fold the 1/rowsum in here
                    zt_sb = pbig.tile([E, HID], F32)
                    nc.scalar.activation(zt_sb[:, 0:384], zt_ps[0][:], AF.Copy,
                                         scale=rinv[:])
                    nc.scalar.activation(zt_sb[:, 384:768], zt_ps[1][:],
                                         AF.Copy, scale=rinv[:])
                    for kc in range(6):
                        ztp = ps1c.tile([128, E], F32, tag="tp", name="ztp",
                                        bufs=1)
                        nc.tensor.transpose(ztp[:],
                                            zt_sb[:, kc * 128:(kc + 1) * 128],
                                            ident[0:E, 0:E])
                        if kc % 2 == 0:
                            nc.scalar.copy(z_sb[kc][:], ztp[:])
                        else:
                            nc.vector.tensor_copy(out=z_sb[kc][:], in_=ztp[:])
                    # ecT2 = z^T-chunks as lhsT @ Wtr -> [22,512] + b (x) easum
                    ec2_p = ps1c.tile([E, EMB], F32, tag="sc", name="ec2",
                                      bufs=2)
                    for kc in range(6):
                        nc.tensor.matmul(ec2_p[:], z_sb[kc][:], wtr[kc][:],
                                         start=(kc == 0), stop=False)
                    nc.tensor.matmul(ec2_p[:], easumT[:], brow[:],
                                     start=False, stop=True)
                    ec2_sb = pbig.tile([E, EMB], F32)
                    nc.scalar.copy(ec2_sb[:], ec2_p[:])
                    for mc in range(4):
                        ecp = ps1c.tile([128, E], F32, tag="tp", name="ecp",
                                        bufs=1)
                        nc.tensor.transpose(ecp[:],
                                            ec2_sb[:, mc * 128:(mc + 1) * 128],
                                            ident[0:E, 0:E])
                        if mc % 2 == 0:
                            nc.scalar.copy(ectxT_sb[mc][:], ecp[:])
                        else:
                            nc.vector.tensor_copy(out=ectxT_sb[mc][:],
                                                  in_=ecp[:])

                    h3 = rgcn_layer(2, h2)
                    rgcn_layer(3, h3)

                    # entity_struT + e_ctxT -> ecT
                    for mc in range(4):
                        tp = ps1c.tile([128, E], F32,
                                       tag="tp" if mc % 2 == 0 else "sc",
                                       name="est", bufs=1 if mc % 2 == 0 else 2)
                        nc.tensor.matmul(tp[:],
                                         hfin[0:E, mc * 128:(mc + 1) * 128],
                                         identb[0:E, 0:E], start=True,
                                         stop=True)
                        nc.vector.tensor_tensor(out=ecT[mc][:], in0=tp[:],
                                                in1=ectxT_sb[mc][:],
                                                op=ALU.add)

        if stages >= 3:
          # ================= stage 3: fmap + SE =================
          fmap = [pwork.tile([128, EE], BF16, tag=f"fmap{i}", name=f"fmap{i}")
                  for i in range(4)]
          pooled = [pwork.tile([128, 1], BF16, tag=f"pool{i}", name=f"pool{i}")
                    for i in range(4)]
          for mc in range(4):
              for ee, lo, hi in ((nc.vector, 0, 11), (nc.gpsimd, 11, E)):
                  o6v = fmap[mc][:].rearrange("p (i j) -> p i j", i=E)[:, lo:hi]
                  in0 = ecT[mc][:, lo:hi].rearrange("p (i j) -> p i j", j=1) \
                      .to_broadcast([128, hi - lo, E])
                  in1 = ecT[mc][:].rearrange("p (o j) -> p o j", o=1) \
                      .to_broadcast([128, hi - lo, E])
                  ee.tensor_tensor(out=o6v, in0=in0, in1=in1, op=ALU.mult)
              rs = pwork.tile([128, 1], F32, tag=f"rs{mc}", name=f"rs{mc}")
              nc.vector.tensor_reduce(rs[:], ecT[mc][:], mybir.AxisListType.X,
                                      ALU.add)
              nc.scalar.activation(pooled[mc][:], rs[:], AF.Square,
                                   scale=1.0 / E)

          pse_cm = tc.tile_pool(name="pse", bufs=1, space="PSUM")
          pse = pse_cm.__enter__()
          if True:
              # channel-attention path first: its latency hides under the
              # fmap outer-product DVE chain
              c1_sb = [pwork.tile([128, 1], BF16, tag=f"c1_{i}", name=f"c1_{i}")
                       for i in range(2)]
              for oc in range(2):
                  c1_p = pse.tile([128, 1], F32, tag="c1p", name="c1p")
                  for mc in range(4):
                      nc.tensor.matmul(c1_p[:],
                                       sew["fcw1T"][mc][:, oc * 128:(oc + 1) * 128],
                                       pooled[mc][:],
                                       start=(mc == 0), stop=(mc == 3))
                  nc.scalar.activation(c1_sb[oc][:], c1_p[:], AF.Relu,
                                       bias=sev["fcb1"][oc][:],
                                       scale=sev["fcs1"][oc][:])
              # fcb2 already carries seb2 (folded on host)
              cbb = [pwork.tile([128, 1], F32, tag=f"cbb{i}", name=f"cbb{i}")
                     for i in range(4)]
              for mc in range(4):
                  c2_p = pse.tile([128, 1], F32, tag="c2p", name="c2p")
                  for kc in range(2):
                      nc.tensor.matmul(c2_p[:],
                                       sew["fcw2T"][kc][:, mc * 128:(mc + 1) * 128],
                                       c1_sb[kc][:],
                                       start=(kc == 0), stop=(kc == 1))
                  nc.scalar.activation(cbb[mc][:], c2_p[:], AF.Identity,
                                       bias=sev["fcb2"][mc][:],
                                       scale=sev["fcs2"][mc][:])
              s1_sb = [pwork.tile([128, EE], BF16, tag=f"s1_{i}", name=f"s1_{i}")
                       for i in range(2)]
              for oc in range(2):
                  s1_p = pse.tile([128, EE], F32, tag="s1p", name="s1p", bufs=2)
                  for mc in range(4):
                      nc.tensor.matmul(s1_p[:],
                                       sew["fsw1T"][mc][:, oc * 128:(oc + 1) * 128],
                                       fmap[mc][:], start=(mc == 0), stop=(mc == 3))
                  nc.scalar.activation(s1_sb[oc][:], s1_p[:], AF.Relu,
                                       bias=sev["seb1"][oc][:],
                                       scale=sev["ses1"][oc][:])
              for mc in range(4):
                  s2_p = pse.tile([128, EE], F32, tag="s2p", name="s2p", bufs=2)
                  for kc in range(2):
                      nc.tensor.matmul(s2_p[:],
                                       sew["fsw2T"][kc][:, mc * 128:(mc + 1) * 128],
                                       s1_sb[kc][:], start=(kc == 0), stop=(kc == 1))
                  sig = pwork.tile([128, EE], BF16, tag="sig", name="sig",
                                   bufs=2)
                  nc.scalar.activation(sig[:], s2_p[:], AF.Sigmoid,
                                       bias=cbb[mc][:], scale=sev["ses2"][mc][:])
                  for ee, lo, hi in ((nc.vector, 0, 11), (nc.gpsimd, 11, E)):
                      outv = fusedp[mc][:].rearrange(
                          "p (i j) -> p i j", j=26)[:, 2 + lo:2 + hi, 2:24]
                      ee.tensor_tensor(
                          out=outv,
                          in0=fmap[mc][:].rearrange("p (i j) -> p i j",
                                                    i=E)[:, lo:hi],
                          in1=sig[:].rearrange("p (i j) -> p i j",
                                               i=E)[:, lo:hi],
                          op=ALU.mult)

        if stages >= 4:
          # ================= stage 4: conv stack =================
          # Row-split pipeline: each conv computes its top (rows 0:11) and
          # bottom (rows 11:22) output halves separately; a half is relu'd
          # and AllGather'd while the next half / next conv keeps the PE
          # busy. Gathered halves land directly in zero-padded 26x26 tiles
          # in fixed rank order (weight chunks are packed in the same rank
          # order), so no masked combines are needed.
          # Row slices (0:8, 8:13, 13:22): the next conv's TOP outputs
          # (rows 0:11) only need input rows <= 12, i.e. the first two
          # slices, so they fully hide the third slice's exchange latency.
          SLICES = [(0, 8), (8, 13), (13, 17), (17, 22)]
          RH = 11 * 22

          def tap_rows(padt, tap, r0, nr):
              dy, dx = tap // 5, tap % 5
              return padt.rearrange("p (i j) -> p i j", j=26)[
                  :, dy + r0:dy + r0 + nr, dx:dx + 22]

          def rd_pair(gpc, r0, nr):
              # interior rows r0:r0+nr of both packed padded images
              return gpc[:].rearrange("p (c i j) -> p c i j", c=2, j=26)[
                  :, :, 2 + r0:2 + r0 + nr, 2:24]

          with tc.tile_pool(name="pcw", bufs=1) as pcw:
              psc = pse
              w2 = []
              for kc in range(2):
                  t = pcw.tile([128, 25 * 128], BF16, tag=f"w2_{kc}",
                               name=f"w2_{kc}")
                  for ch in range(2):
                      nc.gpsimd.dma_start(t[:, ch * 1600:(ch + 1) * 1600],
                                          w2sb_d[kc][:, ch * 1600:(ch + 1) * 1600])
                  w2.append(t)
              w3 = []
              for kc in range(2):
                  t = pcw.tile([128, 25 * 256], BF16, tag=f"w3_{kc}",
                               name=f"w3_{kc}")
                  for ch in range(4):
                      nc.gpsimd.dma_start(t[:, ch * 1600:(ch + 1) * 1600],
                                          w3sb_d[kc][:, ch * 1600:(ch + 1) * 1600])
                  w3.append(t)

              def exchange_slice(stage_sb, dram_pre, gpc, slices, sl_i):
                  """Relu'd slice -> DRAM -> AllGather over the pair -> both
                  packed padded tiles via one 4D-AP read, in fixed rank
                  order. Solo emulates the gather with two direct writes."""
                  r0, r1_ = slices[sl_i]
                  nr = r1_ - r0
                  seg = stage_sb[:, r0 * 22:r1_ * 22]
                  gseg = pdram.tile([256, nr * 22], BF16,
                                    tag=f"{dram_pre}g{sl_i}",
                                    name=f"{dram_pre}g{sl_i}")
                  if solo:
                      nc.sync.dma_start(gseg[0:128, :], seg)
                      nc.sync.dma_start(gseg[128:256, :], seg)
                  else:
                      bseg = pdram.tile([128, nr * 22], BF16,
                                        tag=f"{dram_pre}b{sl_i}",
                                        name=f"{dram_pre}b{sl_i}")
                      nc.sync.dma_start(bseg[:], seg)
                      nc.gpsimd.collective_compute(
                          "AllGather", ALU.bypass, replica_groups=groups,
                          ins=[bseg[:].opt()], outs=[gseg[:].opt()])
                  gv = gpc[:].rearrange("p (c i j) -> p c i j", c=2, j=26)
                  nc.scalar.dma_start(gv[:, 0, 2 + r0:2 + r0 + nr, 2:24],
                                      gseg[0:128, :])
                  nc.gpsimd.dma_start(gv[:, 1, 2 + r0:2 + r0 + nr, 2:24],
                                      gseg[128:256, :])

              def conv_sliced(wsel, srcs, nkc, stage_sb, bias, dram_pre,
                              gpc, slices):
                  """One conv layer: compute the row slices, relu each into
                  stage_sb and exchange it as soon as it's ready."""
                  for sl_i, (r0, r1_) in enumerate(slices):
                      nr = r1_ - r0
                      cp = psc.tile([128, RH], F32, tag="cp", name="cp",
                                    bufs=2)
                      cpv = cp[:, 0:nr * 22]
                      k = 0
                      for kc in range(nkc):
                          for tap in range(25):
                              nc.tensor.matmul(
                                  cpv, wsel(kc, tap),
                                  tap_rows(srcs[kc], tap, r0, nr),
                                  start=(k == 0), stop=(k == 25 * nkc - 1))
                              k += 1
                      nc.scalar.activation(stage_sb[:, r0 * 22:r1_ * 22], cpv,
                                           AF.Relu, bias=bias)
                      exchange_slice(stage_sb, dram_pre, gpc, slices, sl_i)

              # ---- conv1: fusedp -> 128 out-ch (my half) ----
              r1s = pcw.tile([128, EE], BF16, tag="r1s", name="r1s")
              conv_sliced(
                  lambda kc, tap: w1[kc][:, tap * 128:(tap + 1) * 128],
                  [t[:] for t in fusedp], 4, r1s, b1h[:], "r1", g1pc,
                  [(0, 8), (8, 13), (13, 17), (17, 22)])

              # ---- conv2: g1p -> 128 out-ch (my half) ----
              r2s = pcw.tile([128, EE], BF16, tag="r2s", name="r2s")
              conv_sliced(
                  lambda kc, tap: w2[kc][:, tap * 128:(tap + 1) * 128],
                  g1p, 2, r2s, b2h[:], "r2", g2pc,
                  [(0, 8), (8, 13), (13, 22)])

              # ---- conv3: g2p -> 256 out-ch (my half), two half-rows per
              # out chunk; both top chunks first (they only need conv2's
              # first two slices), hiding the last conv2 exchange ----
              for (oc, hh) in ((0, 0), (1, 0), (0, 1), (1, 1)):
                  # the final chunk runs as two independent PSUM chains so
                  # the first half's relu+output DMA overlaps the second's
                  last = (oc == 1 and hh == 1)
                  rows = [(0, 6), (6, 11)] if last else [(0, 11)]
                  for ri, (ra, rb) in enumerate(rows):
                      nr = rb - ra
                      cp = psc.tile([128, RH], F32, tag="cp", name="cp",
                                    bufs=2)
                      cpv = cp[:, 0:nr * 22]
                      k = 0
                      for tap in range(25):
                          for kc in range(2):
                              nc.tensor.matmul(
                                  cpv,
                                  w3[kc][:, tap * 256 + oc * 128:
                                         tap * 256 + (oc + 1) * 128],
                                  tap_rows(g2p[kc], tap, hh * 11 + ra, nr),
                                  start=(k == 0), stop=(k == 49))
                              k += 1
                      o_sb = pcw.tile([128, RH], F32, tag="osb",
                                      name="osb", bufs=3)
                      ov = o_sb[:, 0:nr * 22]
                      nc.scalar.activation(ov, cpv, AF.Relu, bias=b3h[oc][:])
                      eng = nc.sync if (oc + hh + ri) % 2 == 0 else nc.scalar
                      eng.dma_start(
                          out_d[oc * 128:(oc + 1) * 128,
                                hh * RH + ra * 22:hh * RH + rb * 22], ov)

        if stages >= 3:
            pse_cm.__exit__(None, None, None)

    nc.compile()
    return nc


_NC_CACHE = None


def _get_program():
    global _NC_CACHE
    if _NC_CACHE is None:
        _NC_CACHE = build_program()
    return _NC_CACHE


def _bf(a):
    return np.ascontiguousarray(a.astype(ml_dtypes.bfloat16))


def _prep_shared(w):
    """Packed weights/constants identical on every core."""
    ADJ = _build_adj()
    out = {}
    constb = np.zeros((128, _CB), np.float32)

    def put(nm, arr):
        c0, cols = _LAY_B[nm]
        r, cc = arr.shape
        constb[0:r, c0:c0 + cc] = arr
    wt = w['W_trans']
    for kc in range(6):
        put(f"wtr{kc}", wt[kc * 128:(kc + 1) * 128])
    put("brow", w['b_trans'].reshape(1, EMB))
    put("onesrow", np.ones((1, 128), np.float32))
    put("onescol", np.ones((128, 1), np.float32))
    g2T = np.zeros((EM, E), np.float32)
    for e in range(E):
        g2T[e * M:(e + 1) * M, e] = 1.0
    put("g2T", g2T)
    sumT = np.kron(np.eye(L, dtype=np.float32), np.ones((SPAN, 1), np.float32))
    for kc in range(4):
        put(f"sumT{kc}", sumT[kc * 128:(kc + 1) * 128])
    for nm, arr, nch in (("fsw1T", w['fs_w1'].T, 4), ("fcw1T", w['fc_w1'].T, 4),
                         ("fsw2T", w['fs_w2'].T, 2), ("fcw2T", w['fc_w2'].T, 2)):
        for kc in range(nch):
            put(f"{nm}{kc}", np.ascontiguousarray(arr[kc * 128:(kc + 1) * 128]))
    out['constb'] = _bf(constb)

    gT = np.zeros((EMH, E), np.float32)
    for e in range(E):
        gT[e * M * H:(e + 1) * M * H, e] = 1.0 / (M * H)
    gTb = np.zeros((128, 9 * E), np.float32)
    for kc in range(9):
        r = min(128, EMH - kc * 128)
        gTb[0:r, kc * E:(kc + 1) * E] = gT[kc * 128:kc * 128 + r]
    out['gTb'] = np.ascontiguousarray(gTb.astype(ml_dtypes.float8_e4m3))
    out['aallTb'] = _bf(np.concatenate(
        [ADJ[r].T for r in range(NREL)] + [np.eye(NN, dtype=np.float32)],
        axis=1))
    out['tfb'] = _bf(np.ascontiguousarray(w['type_embed'][_TYPES]))
    out['identb'] = _bf(np.eye(128, dtype=np.float32))

    constf = np.zeros((128, _CF), np.float32)

    def putf(nm, arr):
        c0, cols = _LAY_F[nm]
        constf[0:arr.shape[0], c0:c0 + 1] = arr.reshape(-1, 1)
    vecs = {"ses1": w['fs_g1'], "seb1": w['fs_b1'] * w['fs_g1'] + w['fs_be1'],
            "fcs1": w['fc_g1'], "fcb1": w['fc_b1'] * w['fc_g1'] + w['fc_be1'],
            "ses2": w['fs_g2'], "seb2": w['fs_b2'] * w['fs_g2'] + w['fs_be2'],
            "fcs2": w['fc_g2'],
            "fcb2": w['fc_b2'] * w['fc_g2'] + w['fc_be2'] +
                    w['fs_b2'] * w['fs_g2'] + w['fs_be2']}
    for nm, v in vecs.items():
        nch = 2 if v.shape[0] == INTER else 4
        for kc in range(nch):
            putf(f"{nm}{kc}", v[kc * 128:(kc + 1) * 128])
    out['constf_base'] = constf

    for layer in range(NLAYERS):
        din_l = D0 if layer == 0 else EMB
        kcs = _KC0 if layer == 0 else _KC1
        nk = len(kcs)
        Wst = w['rgcn_Wrel0'].reshape(NREL * D0, EMB) if layer == 0 else \
            w['rgcn_Wrel'][layer - 1].reshape(NREL * EMB, EMB)
        Wself = w['rgcn_Wself0'] if layer == 0 else w['rgcn_Wself'][layer - 1]
        p = np.zeros((128, (NREL + 1) * nk * EMB), np.float32)
        for si, (s0, sl) in enumerate(kcs):
            for r in range(NREL):
                b = si * (NREL + 1) + r
                p[0:sl, b * EMB:(b + 1) * EMB] = \
                    Wst[r * din_l + s0:r * din_l + s0 + sl]
            b = si * (NREL + 1) + NREL
            p[0:sl, b * EMB:(b + 1) * EMB] = Wself[s0:s0 + sl]
        out[f'wstp{layer}'] = _bf(p)
    return out


def _prep_conv_half(w, half, constf_base):
    out = {}
    w1 = w['cr_w1'][half * 128:(half + 1) * 128]
    out['w1sb'] = _bf(np.ascontiguousarray(
        w1.transpose(1, 2, 3, 0).reshape(4, 128, 25 * 128)))
    # conv2/conv3 weight chunks in natural (rank-ordered) input-half order
    w2 = w['cr_w2'][half * 128:(half + 1) * 128]
    out['w2sb'] = _bf(np.ascontiguousarray(
        w2.transpose(1, 2, 3, 0).reshape(2, 128, 25 * 128)))
    w3 = w['cr_w3'][half * 256:(half + 1) * 256]
    out['w3sb'] = _bf(np.ascontiguousarray(
        w3.transpose(1, 2, 3, 0).reshape(2, 128, 25 * 256)))
    constf = constf_base.copy()

    def putf(nm, arr):
        c0, cols = _LAY_F[nm]
        constf[0:arr.shape[0], c0:c0 + 1] = arr.reshape(-1, 1)
    putf("b1h", w['cr_b1'][half * 128:(half + 1) * 128])
    putf("b2h", w['cr_b2'][half * 128:(half + 1) * 128])
    putf("b3h0", w['cr_b3'][half * 256:half * 256 + 128])
    putf("b3h1", w['cr_b3'][half * 256 + 128:half * 256 + 256])
    putf("mtop", np.full(128, float(half), np.float32))
    putf("mbot", np.full(128, float(1 - half), np.float32))
    c0, cols = _LAY_F["identf"]
    constf[:, c0:c0 + 128] = np.eye(128, dtype=np.float32)
    out['constf'] = constf
    return out


def _prep_doc(x, att, mi, ls):
    out = {}
    mif = mi.reshape(EM)
    attm = np.ascontiguousarray(
        att[:, mif, :].transpose(1, 0, 2).reshape(EMH, C))
    amp = np.zeros((128, 9 * C), np.float32)
    for kc in range(9):
        r = min(128, EMH - kc * 128)
        amp[0:r, kc * C:kc * C + C] = attm[kc * 128:kc * 128 + r]
    out['amp'] = np.ascontiguousarray(amp.astype(ml_dtypes.float8_e4m3))
    idx = ls[:, None] + np.arange(SPAN)
    idxf = idx.reshape(LS)
    rows = att[:, idxf, :].reshape(H, L, SPAN, C)
    blocks = np.take_along_axis(rows, idx[None, :, None, :], axis=3)
    attl = blocks.transpose(0, 2, 1, 3).reshape(HS, LS)
    xmT = x[mif].T
    xspT = x[idxf].T
    actb = np.zeros((128, _CA), np.float32)

    def put(nm, arr):
        c0, cols = _LAY_A[nm]
        actb[0:arr.shape[0], c0:c0 + arr.shape[1]] = arr
    for kc in range(6):
        put(f"xmT{kc}", xmT[kc * 128:(kc + 1) * 128])
        put(f"xspT{kc}", xspT[kc * 128:(kc + 1) * 128])
    for kc in range(3):
        put(f"attl{kc}", attl[kc * 128:(kc + 1) * 128])
    out['actb'] = _bf(actb)
    xpk = np.zeros((128, 8 * HID), np.float32)
    for kc in range(8):
        xpk[:, kc * HID:(kc + 1) * HID] = x[kc * 128:(kc + 1) * 128]
    out['xp'] = _bf(xpk)
    return out


def build_in_maps(inputs):
    w = {}
    for k, v in inputs.items():
        a = np.asarray(v)
        w[k] = a if a.dtype in (np.int32, np.int64) else \
            np.asarray(a, np.float32)
    shared = _prep_shared(w)
    constf_base = shared.pop('constf_base')
    halves = [_prep_conv_half(w, h, constf_base) for h in range(2)]
    seq = np.asarray(inputs['sequence_output'], np.float32)
    att = np.asarray(inputs['attention'], np.float32)
    mi = np.asarray(inputs['mention_idx']).astype(np.int64)
    ls = np.asarray(inputs['link_start']).astype(np.int64)
    docs = [_prep_doc(seq[n], att[n], mi[n], ls[n]) for n in range(NB)]
    in_maps = []
    for core in range(N_CORES):
        n, half = core // 2, core % 2
        m = dict(shared)
        m.update(halves[half])
        m.update(docs[n])
        in_maps.append({k: (np.ascontiguousarray(v)
                            if v.dtype in (ml_dtypes.bfloat16,
                                           ml_dtypes.float8_e4m3)
                            else np.ascontiguousarray(v, np.float32))
                        for k, v in m.items()})
    return in_maps


def kernel(**inputs):
    nc = _get_program()
    in_maps = build_in_maps(inputs)
    res = run_bass_kernel_spmd(nc, in_maps, list(range(N_CORES)))
    out = np.zeros((NB, EMB, E, E), np.float32)
    for core in range(N_CORES):
        n, half = core // 2, core % 2
        out[n, half * 256:(half + 1) * 256] = \
            res.results[core]["out"].reshape(256, E, E)
    return out


# revision 52
# speedup vs baseline: 1.3614x; 1.0113x over previous
"""Trainium2 Bass kernel for nn_DocREModel (DocRE: gather -> RGCN -> SE -> 5x5 convs).

Sharding: 4 documents x 2 cores each. Each pair replicates the cheap upstream
(mention/link/ea gathers -> RGCN -> fmap/SE) and splits the dominant 5x5 conv
stack by output channels, with two intra-pair AllGathers; output halves are
assembled on host. All index-driven gathers happen on host (pure data
movement; one SPMD program serves all 8 cores), all dense math on device.

Perf model notes (TimelineSim): all DMAs serialize on one ~332 GB/s pipe in
~issue order, and the PE p-state ramp rewards keeping the tensor engine
continuously fed. Hence: everything DMA'd is bf16 (f32 only for small
per-channel scale/bias vectors), tensors are issued strictly in first-use
order (amp/gTb first so the ea matmuls start ~2.5us in), RGCN + conv weights
stream just-in-time behind the compute, and h0 is assembled directly by ACT
writes into a bf16 tile instead of SBUF->SBUF DMA round trips.

Precision/layout choices:
- bf16 weights+activations everywhere on the matmul path, f32 PSUM
  accumulation throughout; per-channel BN scales/biases stay f32.
- Convs are 25 shift-tap matmuls over zero-padded 26x26 images via strided
  APs (no im2col copies). conv2/conv3 start on the locally-computed input
  half before the pair AllGather completes; the other half is extracted
  SPMD-safely with host-supplied 0/1 masks and per-core (own, other)
  weight-chunk ordering.
- RGCN folds the self-loop in as a 4th identity relation so each layer is
  one u = h^T @ [A0^T|A1^T|A2^T|I] matmul plus one PSUM accumulation over
  stacked (relation, chunk) weights -- no transposes in the loop.
"""

import numpy as np
import ml_dtypes

import concourse.bacc as bacc
import concourse.tile as tile
from concourse import mybir
from concourse.bass_utils import run_bass_kernel_spmd

F32 = mybir.dt.float32
F32R = mybir.dt.float32r
BF16 = mybir.dt.bfloat16
F8 = mybir.dt.float8e4
AF = mybir.ActivationFunctionType
ALU = mybir.AluOpType

NB, H, C, HID, EMB = 4, 12, 1024, 768, 512
E, M, L, SPAN = 22, 4, 16, 32
TD, INTER = 20, 256
NN = E + E * M + L
NREL, NLAYERS = 3, 4
EM, EMH, HS, LS = E * M, E * M * H, H * SPAN, L * SPAN
D0 = EMB + TD           # 532
EE = E * E              # 484
PADW = 26 * 26          # 676 padded 26x26 image
N_CORES = 8


def _build_adj():
    A = np.zeros((NREL, NN, NN), np.float32)
    for e in range(E):
        for m in range(M):
            mi = E + e * M + m
            A[0, e, mi] = A[0, mi, e] = 1.0
            for m2 in range(M):
                if m2 != m:
                    A[1, mi, E + e * M + m2] = 1.0
            li = E + E * M + ((e * M + m) % L)
            A[2, mi, li] = A[2, li, mi] = 1.0
    A = A / (A.sum(-1, keepdims=True) + 1e-5)
    return A


_TYPES = np.concatenate([np.zeros(E, np.int32), np.ones(EM, np.int32),
                         np.full(L, 2, np.int32)])

_KC0 = [(0, 128), (128, 128), (256, 128), (384, 128), (512, 20)]   # 532 rows
_KC1 = [(0, 128), (128, 128), (256, 128), (384, 128)]              # 512 rows


def _constb_layout():
    """Column layout of the packed bf16 constant tensor [128, CB].

    Part A (cols 0:CBA) is everything needed through stage 3's s1/c1;
    part B (fsw2T/fcw2T) is DMA'd later, after the RGCN weights.
    """
    lay = {}
    c = 0

    def add(nm, cols):
        nonlocal c
        lay[nm] = (c, cols)
        c += cols
    for kc in range(6):
        add(f"wtr{kc}", EMB)
    add("brow", EMB)
    add("onesrow", 128)
    add("onescol", 1)
    add("g2T", E)
    for kc in range(4):
        add(f"sumT{kc}", L)
    for kc in range(4):
        add(f"fsw1T{kc}", INTER)
    for kc in range(4):
        add(f"fcw1T{kc}", INTER)
    cba = c
    for kc in range(2):
        add(f"fsw2T{kc}", EMB)
    for kc in range(2):
        add(f"fcw2T{kc}", EMB)
    return lay, c, cba


def _constf_layout():
    lay = {}
    c = 0

    def add(nm, cols):
        nonlocal c
        lay[nm] = (c, cols)
        c += cols
    for nm, nch in (("ses1", 2), ("seb1", 2), ("fcs1", 2), ("fcb1", 2),
                    ("ses2", 4), ("seb2", 4), ("fcs2", 4), ("fcb2", 4)):
        for kc in range(nch):
            add(f"{nm}{kc}", 1)
    add("b1h", 1)
    add("b2h", 1)
    add("b3h0", 1)
    add("b3h1", 1)
    add("mtop", 1)
    add("mbot", 1)
    add("identf", 128)
    return lay, c


def _actb_layout():
    lay = {}
    c = 0

    def add(nm, cols):
        nonlocal c
        lay[nm] = (c, cols)
        c += cols
    for kc in range(6):
        add(f"xmT{kc}", EM)
    for kc in range(6):
        add(f"xspT{kc}", LS)
    for kc in range(3):
        add(f"attl{kc}", LS)
    return lay, c


_LAY_B, _CB, _CBA = _constb_layout()
_LAY_F, _CF = _constf_layout()
_LAY_A, _CA = _actb_layout()


def build_program(solo=False, stages=4):
    nc = bacc.Bacc("TRN2", target_bir_lowering=False, debug=False)

    def din(name, shape, dt=BF16):
        return nc.dram_tensor(name, list(shape), dt, kind="ExternalInput").ap()

    constb_d = din("constb", [128, _CB])
    constf_d = din("constf", [128, _CF], F32)
    actb_d = din("actb", [128, _CA])
    xp_d = din("xp", [128, 8 * HID])
    amp_d = din("amp", [128, 9 * C], F8)
    gTb_d = din("gTb", [128, 9 * E], F8)
    tfb_d = din("tfb", [NN, TD])
    wstp_d = [din("wstp0", [128, 20 * EMB])] + \
             [din(f"wstp{i}", [128, 16 * EMB]) for i in (1, 2, 3)]
    w1sb_d = din("w1sb", [4, 128, 25 * 128])
    w2sb_d = din("w2sb", [2, 128, 25 * 128])
    w3sb_d = din("w3sb", [2, 128, 25 * 256])
    aallTb_d = din("aallTb", [NN, (NREL + 1) * NN])
    identb_d = din("identb", [128, 128])

    out_d = nc.dram_tensor("out", [256, EE], F32, kind="ExternalOutput").ap()

    groups = [[0, 1], [2, 3], [4, 5], [6, 7]]

    with tile.TileContext(nc) as tc:
      with tc.tile_pool(name="pconst", bufs=1) as pconst, \
           tc.tile_pool(name="pwork", bufs=1) as pwork, \
           tc.tile_pool(name="pdram", bufs=1, space="DRAM") as pdram:

        constb = pconst.tile([128, _CB], BF16)
        constf = pconst.tile([128, _CF], F32)
        identb = pconst.tile([128, 128], BF16)
        aallTb = pconst.tile([NN, (NREL + 1) * NN], BF16)
        aallE = pconst.tile([E, (NREL + 1) * NN], BF16)
        aallM = pconst.tile([EM, (NREL + 1) * NN], BF16)
        aallL = pconst.tile([L, (NREL + 1) * NN], BF16)
        wstp_t = [pconst.tile([128, 20 * EMB], BF16, tag="wstp0",
                              name="wstp0")] + \
                 [pconst.tile([128, 16 * EMB], BF16, tag=f"wstp{l}",
                              name=f"wstp{l}") for l in (1, 2, 3)]
        w1 = [pconst.tile([128, 25 * 128], BF16, tag=f"w1_{kc}",
                          name=f"w1_{kc}") for kc in range(4)]

        def cb(nm, rows=128):
            c0, cols = _LAY_B[nm]
            return constb[0:rows, c0:c0 + cols]

        def cf(nm, rows=128):
            c0, cols = _LAY_F[nm]
            return constf[0:rows, c0:c0 + cols]

        wtr = [cb(f"wtr{kc}") for kc in range(6)]
        brow = cb("brow", rows=1)
        onesrow = cb("onesrow", rows=1)
        onescol = cb("onescol")
        g2T = cb("g2T", rows=EM)
        sumT = [cb(f"sumT{kc}") for kc in range(4)]
        sew = {nm: [cb(f"{nm}{kc}") for kc in range(n)]
               for nm, n in (("fsw1T", 4), ("fcw1T", 4), ("fsw2T", 2),
                             ("fcw2T", 2))}
        sev = {nm: [cf(f"{nm}{kc}") for kc in range(n)]
               for nm, n in (("ses1", 2), ("seb1", 2), ("fcs1", 2), ("fcb1", 2),
                             ("ses2", 4), ("seb2", 4), ("fcs2", 4),
                             ("fcb2", 4))}
        b1h = cf("b1h")
        b2h = cf("b2h")
        b3h = [cf("b3h0"), cf("b3h1")]
        ident = cf("identf")

        # persistent intermediates (three base-0 tiles: engines cannot
        # write SBUF at unaligned base partitions, so the node matrix is
        # kept split as [entities; mentions; links])
        h0e = pwork.tile([E, D0], BF16)
        h0m = pwork.tile([EM, D0], BF16)
        h0l = pwork.tile([L, D0], BF16)
        hfin = pwork.tile([NN, EMB], BF16)
        ectxT_sb = [pwork.tile([128, E], F32, tag=f"ectxT{i}", name=f"ectxT{i}")
                    for i in range(4)]
        ecT = [pwork.tile([128, E], F32R, tag=f"ecT{i}", name=f"ecT{i}")
               for i in range(4)]
        # PE warmup fodder: covers the head until real operands land (the
        # scheduler hoists dependency-free matmuls to the front).
        warm = pwork.tile([128, 512], BF16)
        nc.vector.memset(warm[:], 0.0)
        fusedp = [pwork.tile([128, PADW], BF16, tag=f"fusedp{i}",
                             name=f"fusedp{i}") for i in range(4)]
        g1pc = pwork.tile([128, 2 * PADW], BF16, tag="g1pc", name="g1pc")
        g2pc = pwork.tile([128, 2 * PADW], BF16, tag="g2pc", name="g2pc")
        g1p = [g1pc[:, i * PADW:(i + 1) * PADW] for i in range(2)]
        g2p = [g2pc[:, i * PADW:(i + 1) * PADW] for i in range(2)]
        for t_ in fusedp:
            nc.vector.memset(t_[:], 0.0)
        nc.vector.memset(g1pc[:], 0.0)
        nc.vector.memset(g2pc[:], 0.0)

        with tc.tile_pool(name="pbig", bufs=1) as pbig:
            gTb = pbig.tile([128, 9 * E], F8)
            amp = pbig.tile([128, 9 * C], F8)
            xp = pbig.tile([128, 8 * HID], BF16)
            actb = pbig.tile([128, _CA], BF16)

            # ---- the bulk DMA stream rides the SWDGE (gpsimd) ring in
            # first-use order; sync/scalar stay shallow for latency-
            # critical transfers later (conv exchanges, outputs) ----
            nc.scalar.dma_start(constf[:], constf_d[:])
            xm_cols = 6 * EM                      # xmT region of actb
            wtr_cols = 6 * EMB + EMB + 128 + 1    # wtr+brow+ones region
            nc.gpsimd.dma_start(actb[:, 0:xm_cols], actb_d[:, 0:xm_cols])
            nc.gpsimd.dma_start(constb[:, 0:wtr_cols], constb_d[:, 0:wtr_cols])
            se1_cols = wtr_cols + E + 4 * L   # g2T+sumT end
            nc.gpsimd.dma_start(constb[:, wtr_cols:se1_cols],
                                constb_d[:, wtr_cols:se1_cols])
            sp_cols = xm_cols + 6 * LS
            nc.gpsimd.dma_start(actb[:, xm_cols:sp_cols],
                                actb_d[:, xm_cols:sp_cols])
            nc.gpsimd.dma_start(actb[:, sp_cols:_CA], actb_d[:, sp_cols:_CA])
            nc.scalar.dma_start(h0e[:, EMB:D0], tfb_d[0:E, :])
            nc.scalar.dma_start(h0m[:, EMB:D0], tfb_d[E:E + EM, :])
            nc.scalar.dma_start(h0l[:, EMB:D0], tfb_d[E + EM:NN, :])
            nc.gpsimd.dma_start(aallTb[:], aallTb_d[:])
            nc.gpsimd.dma_start(aallE[:], aallTb_d[0:E, :])
            nc.gpsimd.dma_start(aallM[:], aallTb_d[E:E + EM, :])
            nc.gpsimd.dma_start(aallL[:], aallTb_d[E + EM:NN, :])
            # RGCN weights, chunked si-major so each layer's PSUM chain can
            # start as soon as its first chunk lands
            BL = (NREL + 1) * EMB

            def wstp_dma(layer):
                nchunks = 5 if layer == 0 else 4
                for si in range(nchunks):
                    nc.gpsimd.dma_start(
                        wstp_t[layer][:, si * BL:(si + 1) * BL],
                        wstp_d[layer][:, si * BL:(si + 1) * BL])
            wstp_dma(0)
            wstp_dma(1)
            nc.gpsimd.dma_start(gTb[:], gTb_d[:])
            for g in range(3):
                nc.gpsimd.dma_start(amp[:, g * 3 * C:(g + 1) * 3 * C],
                                    amp_d[:, g * 3 * C:(g + 1) * 3 * C])
            nc.gpsimd.dma_start(xp[:], xp_d[:])
            nc.gpsimd.dma_start(constb[:, se1_cols:_CBA],
                                constb_d[:, se1_cols:_CBA])
            wstp_dma(2)
            wstp_dma(3)
            nc.gpsimd.dma_start(constb[:, _CBA:_CB], constb_d[:, _CBA:_CB])
            nc.scalar.dma_start(identb[:], identb_d[:])
            for kc in range(4):
                nc.gpsimd.dma_start(w1[kc][:], w1sb_d[kc])

            # ========== stage 1a: mention/span/link rows -> h0b ==========
            expm = pbig.tile([EM, EMB], BF16)
            sp_ps = []
            wsb = [pbig.tile([128, 1], F32, tag=f"wsb{i}", name=f"wsb{i}")
                   for i in range(4)]
            wsp = [pbig.tile([128, EMB], BF16, tag=f"wsp{i}", name=f"wsp{i}")
                   for i in range(4)]

            def ca(nm, rows=128):
                c0, cols = _LAY_A[nm]
                return actb[0:rows, c0:c0 + cols]

            xmT = [ca(f"xmT{kc}") for kc in range(6)]
            xspT = [ca(f"xspT{kc}") for kc in range(6)]
            attl = [ca(f"attl{kc}") for kc in range(3)]

            with tc.tile_pool(name="ps1a", bufs=1, space="PSUM") as ps1a:
                jp = ps1a.tile([128, 512], F32, tag="jp", name="jp")
                for _ in range(14):
                    nc.tensor.matmul(jp[:], warm[:, 0:128], warm[:],
                                     start=True, stop=True)
                # mentions: mrep = x_m @ Wtr + b -> h0b rows + exp for pooling
                mrep_p = ps1a.tile([EM, EMB], F32, tag="mrep", name="mrep")
                for kc in range(6):
                    nc.tensor.matmul(mrep_p[:], xmT[kc][:, 0:EM], wtr[kc][:],
                                     start=(kc == 0), stop=False)
                nc.tensor.matmul(mrep_p[:], onesrow[0:1, 0:EM], brow[:],
                                 start=False, stop=True)
                nc.scalar.copy(h0m[:, 0:EMB], mrep_p[:])
                nc.scalar.activation(expm[:], mrep_p[:], AF.Exp)
                # e_rep = ln(G2 @ exp(mrep))
                ep_p = ps1a.tile([E, EMB], F32, tag="ep", name="ep")
                nc.tensor.matmul(ep_p[:], g2T[:], expm[:], start=True, stop=True)
                nc.scalar.activation(h0e[:, 0:EMB], ep_p[:], AF.Ln)
                # dummy: switch the ACT table to the sigmoid set now (exp/ln
                # are done) so stage 3's sigmoid doesn't pay the 1.3us load
                sigwarm = pbig.tile([1, 1], F32)
                nc.scalar.activation(sigwarm[:], ep_p[0:1, 0:1], AF.Sigmoid)

                # spans: sp = x_span @ Wtr + b
                for mc in range(4):
                    sp_p = ps1a.tile([128, EMB], F32, tag="sp_p", name="sp_p",
                                     bufs=3)
                    for kc in range(6):
                        nc.tensor.matmul(sp_p[:],
                                         xspT[kc][:, mc * 128:(mc + 1) * 128],
                                         wtr[kc][:], start=(kc == 0), stop=False)
                    nc.tensor.matmul(sp_p[:], onesrow[:], brow[:],
                                     start=False, stop=True)
                    spc = pbig.tile([128, EMB], BF16, tag="spc", name="spc",
                                    bufs=4)
                    nc.scalar.copy(spc[:], sp_p[:])
                    sp_ps.append(spc)
                # w = colsum(attl) / 384
                for mc in range(4):
                    w_p = ps1a.tile([128, 1], F32, tag="w_p", name="w_p", bufs=1)
                    for kc in range(3):
                        nc.tensor.matmul(w_p[:],
                                         attl[kc][:, mc * 128:(mc + 1) * 128],
                                         onescol[:],
                                         start=(kc == 0), stop=(kc == 2))
                    nc.scalar.activation(wsb[mc][:], w_p[:], AF.Copy,
                                         scale=1.0 / (H * SPAN))
                # wsp = psum(sp) * w ; link = SUM^T @ wsp
                for mc in range(4):
                    nc.vector.tensor_scalar(out=wsp[mc][:], in0=sp_ps[mc][:],
                                            scalar1=wsb[mc][:], scalar2=None,
                                            op0=ALU.mult)
                link_p = ps1a.tile([L, EMB], F32, tag="link", name="link")
                for kc in range(4):
                    nc.tensor.matmul(link_p[:], sumT[kc][:], wsp[kc][:],
                                     start=(kc == 0), stop=(kc == 3))
                nc.scalar.copy(h0l[:, 0:EMB], link_p[:])

            # ====== stage 2 + stage 1b interleaved: the ea/e_ctx latency
            # chain fills the RGCN's weight-stream stalls ======
            ea_sb = pbig.tile([E, C], F32R)
            eaT = [pbig.tile([128, E], BF16, tag=f"eaT{i}", name=f"eaT{i}")
                   for i in range(8)]
            z_sb = [pbig.tile([128, E], BF16, tag=f"z{i}", name=f"z{i}")
                    for i in range(6)]
            easumT = pbig.tile([1, E], BF16)

            if stages >= 2:
              with tc.tile_pool(name="prg", bufs=2) as prg, \
                   tc.tile_pool(name="psr", bufs=1, space="PSUM") as psr:

                def rgcn_layer(layer, h):
                    kcs = _KC0 if layer == 0 else _KC1
                    nk = len(kcs)
                    wstp = wstp_t[layer]
                    # si-major packing: block (si, r) at (si*(NREL+1)+r)*EMB
                    wst_t = [wstp[:, (si * (NREL + 1) + r) * EMB:
                                   (si * (NREL + 1) + r + 1) * EMB]
                             for r in range(NREL + 1) for si in range(nk)]
                    u_sb = []
                    for si, (s0, sl) in enumerate(kcs):
                        u_p = psr.tile([128, (NREL + 1) * NN], F32, tag="u_p",
                                       name="u_p", bufs=2)
                        if layer == 0:
                            nc.tensor.matmul(u_p[0:sl, :],
                                             h0e[:, s0:s0 + sl], aallE[:],
                                             start=True, stop=False)
                            nc.tensor.matmul(u_p[0:sl, :],
                                             h0m[:, s0:s0 + sl], aallM[:],
                                             start=False, stop=False)
                            nc.tensor.matmul(u_p[0:sl, :],
                                             h0l[:, s0:s0 + sl], aallL[:],
                                             start=False, stop=True)
                        else:
                            nc.tensor.matmul(u_p[0:sl, :], h[0:NN, s0:s0 + sl],
                                             aallTb[:], start=True, stop=True)
                        u = prg.tile([128, (NREL + 1) * NN], BF16, tag=f"u{si}",
                                     name=f"u{si}", bufs=1)
                        if si % 2 == 0:
                            nc.scalar.copy(u[0:sl, :], u_p[0:sl, :])
                        else:
                            nc.vector.tensor_copy(out=u[0:sl, :],
                                                  in_=u_p[0:sl, :])
                        u_sb.append(u)
                    y_p = psr.tile([NN, EMB], F32, tag="y_p", name="y_p")
                    n_mm = (NREL + 1) * nk
                    k_mm = 0
                    for si, (s0, sl) in enumerate(kcs):
                        for r in range(NREL + 1):
                            nc.tensor.matmul(
                                y_p[:], u_sb[si][0:sl, r * NN:(r + 1) * NN],
                                wst_t[r * nk + si][0:sl, :],
                                start=(k_mm == 0), stop=(k_mm == n_mm - 1))
                            k_mm += 1
                    hdst = hfin if layer == NLAYERS - 1 else \
                        prg.tile([NN, EMB], BF16, tag="h_next", name="h_next")
                    for (s0, sl) in _KC1:
                        nc.scalar.activation(hdst[0:NN, s0:s0 + sl],
                                             y_p[0:NN, s0:s0 + sl], AF.Relu)
                    return hdst

                h1 = rgcn_layer(0, None)

                # -- ea block (runs while wstp1 streams) --
                with tc.tile_pool(name="ps1b", bufs=1, space="PSUM") as ps1b:
                    ea_p0 = ps1b.tile([E, 512], F32, tag="ea0", name="ea0")
                    ea_p1 = ps1b.tile([E, 512], F32, tag="ea1", name="ea1")
                    for kc in range(9):
                        rows = 128 if kc < 8 else 32
                        at = amp[0:rows, kc * C:kc * C + C]
                        gt = gTb[0:rows, kc * E:(kc + 1) * E]
                        nc.tensor.matmul(ea_p0[:], gt, at[:, 0:512],
                                         start=(kc == 0), stop=(kc == 8))
                        nc.tensor.matmul(ea_p1[:], gt, at[:, 512:1024],
                                         start=(kc == 0), stop=(kc == 8))
                    r0 = pbig.tile([E, 1], F32)
                    r1 = pbig.tile([E, 1], F32)
                    nc.vector.tensor_reduce(r0[:], ea_p0[:],
                                            mybir.AxisListType.X, ALU.add)
                    nc.vector.tensor_reduce(r1[:], ea_p1[:],
                                            mybir.AxisListType.X, ALU.add)
                    rsum = pbig.tile([E, 1], F32)
                    nc.vector.tensor_tensor(out=rsum[:], in0=r0[:], in1=r1[:],
                                            op=ALU.add)
                    rsum2 = pbig.tile([E, 1], F32)
                    nc.vector.tensor_scalar(out=rsum2[:], in0=rsum[:],
                                            scalar1=1e-5, scalar2=None,
                                            op0=ALU.add)
                    rinv = pbig.tile([E, 1], F32)
                    nc.vector.reciprocal(rinv[:], rsum2[:])
                    for kc in range(4):
                        c0, c1_ = kc * 128, (kc + 1) * 128
                        if kc % 2 == 0:
                            nc.scalar.copy(ea_sb[:, c0:c1_], ea_p0[:, c0:c1_])
                            nc.scalar.copy(ea_sb[:, 512 + c0:512 + c1_],
                                           ea_p1[:, c0:c1_])
                        else:
                            nc.vector.tensor_copy(out=ea_sb[:, c0:c1_],
                                                  in_=ea_p0[:, c0:c1_])
                            nc.vector.tensor_copy(
                                out=ea_sb[:, 512 + c0:512 + c1_],
                                in_=ea_p1[:, c0:c1_])
                    easum = pbig.tile([E, 1], F32)
                    nc.vector.tensor_tensor(out=easum[:], in0=rsum[:],
                                            in1=rinv[:], op=ALU.mult)
                    # eaT transposes reuse the (now dead) ea psum banks
                    for kc in range(8):
                        tp = ps1b.tile([128, E], F32, tag=f"ea{kc % 2}",
                                       name="eaTt")
                        nc.tensor.transpose(tp[:],
                                            ea_sb[:, kc * 128:(kc + 1) * 128]
                                            .bitcast(F32), ident[0:E, 0:E])
                        if kc % 2 == 0:
                            nc.scalar.copy(eaT[kc][:], tp[:])
                        else:
                            nc.vector.tensor_copy(out=eaT[kc][:], in_=tp[:])
                    tp = ps1b.tile([1, E], F32, tag="ea1", name="easumt")
                    nc.tensor.transpose(tp[:], easum[:], ident[0:E, 0:E])
                    nc.scalar.copy(easumT[:], tp[:])

                h2 = rgcn_layer(1, h1)

                with tc.tile_pool(name="ps1c", bufs=1, space="PSUM") as ps1c:
                    # zT = ea_n @ x  [22, 768] (two 384-wide halves)
                    zt_ps = [ps1c.tile([E, 384], F32, tag="sc",
                                       name=f"zt_p{i}", bufs=2)
                             for i in range(2)]
                    for kc in range(8):
                        xt = xp[:, kc * HID:(kc + 1) * HID]
                        for hh in range(2):
                            nc.tensor.matmul(zt_ps[hh][:], eaT[kc][:],
                                             xt[:, hh * 384:(hh + 1) * 384],
                                             start=(kc == 0), stop=(kc == 7))
                    # ea was left unnormalized; fold the 1/rowsum in here
                    zt_sb = pbig.tile([E, HID], F32)
                    nc.scalar.activation(zt_sb[:, 0:384], zt_ps[0][:], AF.Copy,
                                         scale=rinv[:])
                    nc.scalar.activation(zt_sb[:, 384:768], zt_ps[1][:],
                                         AF.Copy, scale=rinv[:])
                    for kc in range(6):
                        ztp = ps1c.tile([128, E], F32, tag="tp", name="ztp",
                                        bufs=1)
                        nc.tensor.transpose(ztp[:],
                                            zt_sb[:, kc * 128:(kc + 1) * 128],
                                            ident[0:E, 0:E])
                        if kc % 2 == 0:
                            nc.scalar.copy(z_sb[kc][:], ztp[:])
                        else:
                            nc.vector.tensor_copy(out=z_sb[kc][:], in_=ztp[:])
                    # ecT2 = z^T-chunks as lhsT @ Wtr -> [22,512] + b (x) easum
                    ec2_p = ps1c.tile([E, EMB], F32, tag="sc", name="ec2",
                                      bufs=2)
                    for kc in range(6):
                        nc.tensor.matmul(ec2_p[:], z_sb[kc][:], wtr[kc][:],
                                         start=(kc == 0), stop=False)
                    nc.tensor.matmul(ec2_p[:], easumT[:], brow[:],
                                     start=False, stop=True)
                    ec2_sb = pbig.tile([E, EMB], F32)
                    nc.scalar.copy(ec2_sb[:], ec2_p[:])
                    for mc in range(4):
                        ecp = ps1c.tile([128, E], F32, tag="tp", name="ecp",
                                        bufs=1)
                        nc.tensor.transpose(ecp[:],
                                            ec2_sb[:, mc * 128:(mc + 1) * 128],
                                            ident[0:E, 0:E])
                        if mc % 2 == 0:
                            nc.scalar.copy(ectxT_sb[mc][:], ecp[:])
                        else:
                            nc.vector.tensor_copy(out=ectxT_sb[mc][:],
                                                  in_=ecp[:])

                    h3 = rgcn_layer(2, h2)
                    rgcn_layer(3, h3)

                    # entity_struT + e_ctxT -> ecT
                    for mc in range(4):
                        tp = ps1c.tile([128, E], F32,
                                       tag="tp" if mc % 2 == 0 else "sc",
                                       name="est", bufs=1 if mc % 2 == 0 else 2)
                        nc.tensor.matmul(tp[:],
                                         hfin[0:E, mc * 128:(mc + 1) * 128],
                                         identb[0:E, 0:E], start=True,
                                         stop=True)
                        nc.vector.tensor_tensor(out=ecT[mc][:], in0=tp[:],
                                                in1=ectxT_sb[mc][:],
                                                op=ALU.add)

        if stages >= 3:
          # ================= stage 3: fmap + SE =================
          fmap = [pwork.tile([128, EE], BF16, tag=f"fmap{i}", name=f"fmap{i}")
                  for i in range(4)]
          pooled = [pwork.tile([128, 1], BF16, tag=f"pool{i}", name=f"pool{i}")
                    for i in range(4)]
          for mc in range(4):
              for ee, lo, hi in ((nc.vector, 0, 11), (nc.gpsimd, 11, E)):
                  o6v = fmap[mc][:].rearrange("p (i j) -> p i j", i=E)[:, lo:hi]
                  in0 = ecT[mc][:, lo:hi].rearrange("p (i j) -> p i j", j=1) \
                      .to_broadcast([128, hi - lo, E])
                  in1 = ecT[mc][:].rearrange("p (o j) -> p o j", o=1) \
                      .to_broadcast([128, hi - lo, E])
                  ee.tensor_tensor(out=o6v, in0=in0, in1=in1, op=ALU.mult)
              rs = pwork.tile([128, 1], F32, tag=f"rs{mc}", name=f"rs{mc}")
              nc.vector.tensor_reduce(rs[:], ecT[mc][:], mybir.AxisListType.X,
                                      ALU.add)
              nc.scalar.activation(pooled[mc][:], rs[:], AF.Square,
                                   scale=1.0 / E)

          pse_cm = tc.tile_pool(name="pse", bufs=1, space="PSUM")
          pse = pse_cm.__enter__()
          if True:
              # channel-attention path first: its latency hides under the
              # fmap outer-product DVE chain
              c1_sb = [pwork.tile([128, 1], BF16, tag=f"c1_{i}", name=f"c1_{i}")
                       for i in range(2)]
              for oc in range(2):
                  c1_p = pse.tile([128, 1], F32, tag="c1p", name="c1p")
                  for mc in range(4):
                      nc.tensor.matmul(c1_p[:],
                                       sew["fcw1T"][mc][:, oc * 128:(oc + 1) * 128],
                                       pooled[mc][:],
                                       start=(mc == 0), stop=(mc == 3))
                  nc.scalar.activation(c1_sb[oc][:], c1_p[:], AF.Relu,
                                       bias=sev["fcb1"][oc][:],
                                       scale=sev["fcs1"][oc][:])
              # fcb2 already carries seb2 (folded on host)
              cbb = [pwork.tile([128, 1], F32, tag=f"cbb{i}", name=f"cbb{i}")
                     for i in range(4)]
              for mc in range(4):
                  c2_p = pse.tile([128, 1], F32, tag="c2p", name="c2p")
                  for kc in range(2):
                      nc.tensor.matmul(c2_p[:],
                                       sew["fcw2T"][kc][:, mc * 128:(mc + 1) * 128],
                                       c1_sb[kc][:],
                                       start=(kc == 0), stop=(kc == 1))
                  nc.scalar.activation(cbb[mc][:], c2_p[:], AF.Identity,
                                       bias=sev["fcb2"][mc][:],
                                       scale=sev["fcs2"][mc][:])
              s1_sb = [pwork.tile([128, EE], BF16, tag=f"s1_{i}", name=f"s1_{i}")
                       for i in range(2)]
              for oc in range(2):
                  s1_p = pse.tile([128, EE], F32, tag="s1p", name="s1p", bufs=2)
                  for mc in range(4):
                      nc.tensor.matmul(s1_p[:],
                                       sew["fsw1T"][mc][:, oc * 128:(oc + 1) * 128],
                                       fmap[mc][:], start=(mc == 0), stop=(mc == 3))
                  nc.scalar.activation(s1_sb[oc][:], s1_p[:], AF.Relu,
                                       bias=sev["seb1"][oc][:],
                                       scale=sev["ses1"][oc][:])
              for mc in range(4):
                  s2_p = pse.tile([128, EE], F32, tag="s2p", name="s2p", bufs=2)
                  for kc in range(2):
                      nc.tensor.matmul(s2_p[:],
                                       sew["fsw2T"][kc][:, mc * 128:(mc + 1) * 128],
                                       s1_sb[kc][:], start=(kc == 0), stop=(kc == 1))
                  sig = pwork.tile([128, EE], BF16, tag="sig", name="sig",
                                   bufs=2)
                  nc.scalar.activation(sig[:], s2_p[:], AF.Sigmoid,
                                       bias=cbb[mc][:], scale=sev["ses2"][mc][:])
                  for ee, lo, hi in ((nc.vector, 0, 11), (nc.gpsimd, 11, E)):
                      outv = fusedp[mc][:].rearrange(
                          "p (i j) -> p i j", j=26)[:, 2 + lo:2 + hi, 2:24]
                      ee.tensor_tensor(
                          out=outv,
                          in0=fmap[mc][:].rearrange("p (i j) -> p i j",
                                                    i=E)[:, lo:hi],
                          in1=sig[:].rearrange("p (i j) -> p i j",
                                               i=E)[:, lo:hi],
                          op=ALU.mult)

        if stages >= 4:
          # ================= stage 4: conv stack =================
          # Row-split pipeline: each conv computes its top (rows 0:11) and
          # bottom (rows 11:22) output halves separately; a half is relu'd
          # and AllGather'd while the next half / next conv keeps the PE
          # busy. Gathered halves land directly in zero-padded 26x26 tiles
          # in fixed rank order (weight chunks are packed in the same rank
          # order), so no masked combines are needed.
          # Row slices (0:8, 8:13, 13:22): the next conv's TOP outputs
          # (rows 0:11) only need input rows <= 12, i.e. the first two
          # slices, so they fully hide the third slice's exchange latency.
          SLICES = [(0, 8), (8, 13), (13, 17), (17, 22)]
          RH = 11 * 22

          def tap_rows(padt, tap, r0, nr):
              dy, dx = tap // 5, tap % 5
              return padt.rearrange("p (i j) -> p i j", j=26)[
                  :, dy + r0:dy + r0 + nr, dx:dx + 22]

          def rd_pair(gpc, r0, nr):
              # interior rows r0:r0+nr of both packed padded images
              return gpc[:].rearrange("p (c i j) -> p c i j", c=2, j=26)[
                  :, :, 2 + r0:2 + r0 + nr, 2:24]

          with tc.tile_pool(name="pcw", bufs=1) as pcw:
              psc = pse
              w2 = []
              for kc in range(2):
                  t = pcw.tile([128, 25 * 128], BF16, tag=f"w2_{kc}",
                               name=f"w2_{kc}")
                  for ch in range(2):
                      nc.gpsimd.dma_start(t[:, ch * 1600:(ch + 1) * 1600],
                                          w2sb_d[kc][:, ch * 1600:(ch + 1) * 1600])
                  w2.append(t)
              w3 = []
              for kc in range(2):
                  t = pcw.tile([128, 25 * 256], BF16, tag=f"w3_{kc}",
                               name=f"w3_{kc}")
                  for ch in range(4):
                      nc.gpsimd.dma_start(t[:, ch * 1600:(ch + 1) * 1600],
                                          w3sb_d[kc][:, ch * 1600:(ch + 1) * 1600])
                  w3.append(t)

              def exchange_slice(stage_sb, dram_pre, gpc, slices, sl_i):
                  """Relu'd slice -> DRAM -> AllGather over the pair -> both
                  packed padded tiles via one 4D-AP read, in fixed rank
                  order. Solo emulates the gather with two direct writes."""
                  r0, r1_ = slices[sl_i]
                  nr = r1_ - r0
                  seg = stage_sb[:, r0 * 22:r1_ * 22]
                  gseg = pdram.tile([256, nr * 22], BF16,
                                    tag=f"{dram_pre}g{sl_i}",
                                    name=f"{dram_pre}g{sl_i}")
                  if solo:
                      nc.sync.dma_start(gseg[0:128, :], seg)
                      nc.sync.dma_start(gseg[128:256, :], seg)
                  else:
                      bseg = pdram.tile([128, nr * 22], BF16,
                                        tag=f"{dram_pre}b{sl_i}",
                                        name=f"{dram_pre}b{sl_i}")
                      nc.sync.dma_start(bseg[:], seg)
                      nc.gpsimd.collective_compute(
                          "AllGather", ALU.bypass, replica_groups=groups,
                          ins=[bseg[:].opt()], outs=[gseg[:].opt()])
                  gv = gpc[:].rearrange("p (c i j) -> p c i j", c=2, j=26)
                  nc.scalar.dma_start(gv[:, 0, 2 + r0:2 + r0 + nr, 2:24],
                                      gseg[0:128, :])
                  nc.gpsimd.dma_start(gv[:, 1, 2 + r0:2 + r0 + nr, 2:24],
                                      gseg[128:256, :])

              def conv_sliced(wsel, srcs, nkc, stage_sb, bias, dram_pre,
                              gpc, slices):
                  """One conv layer: compute the row slices, relu each into
                  stage_sb and exchange it as soon as it's ready."""
                  for sl_i, (r0, r1_) in enumerate(slices):
                      nr = r1_ - r0
                      cp = psc.tile([128, RH], F32, tag="cp", name="cp",
                                    bufs=2)
                      cpv = cp[:, 0:nr * 22]
                      k = 0
                      for kc in range(nkc):
                          for tap in range(25):
                              nc.tensor.matmul(
                                  cpv, wsel(kc, tap),
                                  tap_rows(srcs[kc], tap, r0, nr),
                                  start=(k == 0), stop=(k == 25 * nkc - 1))
                              k += 1
                      nc.scalar.activation(stage_sb[:, r0 * 22:r1_ * 22], cpv,
                                           AF.Relu, bias=bias)
                      exchange_slice(stage_sb, dram_pre, gpc, slices, sl_i)

              # ---- conv1: fusedp -> 128 out-ch (my half) ----
              r1s = pcw.tile([128, EE], BF16, tag="r1s", name="r1s")
              conv_sliced(
                  lambda kc, tap: w1[kc][:, tap * 128:(tap + 1) * 128],
                  [t[:] for t in fusedp], 4, r1s, b1h[:], "r1", g1pc,
                  [(0, 8), (8, 13), (13, 17), (17, 22)])

              # ---- conv2: g1p -> 128 out-ch (my half) ----
              r2s = pcw.tile([128, EE], BF16, tag="r2s", name="r2s")
              conv_sliced(
                  lambda kc, tap: w2[kc][:, tap * 128:(tap + 1) * 128],
                  g1p, 2, r2s, b2h[:], "r2", g2pc,
                  [(0, 10), (10, 13), (13, 22)])

              # ---- conv3: g2p -> 256 out-ch (my half), two half-rows per
              # out chunk; both top chunks first (they only need conv2's
              # first two slices), hiding the last conv2 exchange ----
              for (oc, hh) in ((0, 0), (1, 0), (0, 1), (1, 1)):
                  # the final chunk runs as two independent PSUM chains so
                  # the first half's relu+output DMA overlaps the second's
                  last = (oc == 1 and hh == 1)
                  rows = [(0, 6), (6, 11)] if last else [(0, 11)]
                  for ri, (ra, rb) in enumerate(rows):
                      nr = rb - ra
                      cp = psc.tile([128, RH], F32, tag="cp", name="cp",
                                    bufs=2)
                      cpv = cp[:, 0:nr * 22]
                      order = ([t for t in range(25) if t // 5 <= 1] +
                               [t for t in range(25) if t // 5 > 1]) \
                          if hh == 0 else list(range(25))
                      k = 0
                      for tap in order:
                          for kc in range(2):
                              nc.tensor.matmul(
                                  cpv,
                                  w3[kc][:, tap * 256 + oc * 128:
                                         tap * 256 + (oc + 1) * 128],
                                  tap_rows(g2p[kc], tap, hh * 11 + ra, nr),
                                  start=(k == 0), stop=(k == 49))
                              k += 1
                      o_sb = pcw.tile([128, RH], F32, tag="osb",
                                      name="osb", bufs=3)
                      ov = o_sb[:, 0:nr * 22]
                      nc.scalar.activation(ov, cpv, AF.Relu, bias=b3h[oc][:])
                      eng = nc.sync if (oc + hh + ri) % 2 == 0 else nc.scalar
                      eng.dma_start(
                          out_d[oc * 128:(oc + 1) * 128,
                                hh * RH + ra * 22:hh * RH + rb * 22], ov)

        if stages >= 3:
            pse_cm.__exit__(None, None, None)

    nc.compile()
    return nc


_NC_CACHE = None


def _get_program():
    global _NC_CACHE
    if _NC_CACHE is None:
        _NC_CACHE = build_program()
    return _NC_CACHE


def _bf(a):
    return np.ascontiguousarray(a.astype(ml_dtypes.bfloat16))


def _prep_shared(w):
    """Packed weights/constants identical on every core."""
    ADJ = _build_adj()
    out = {}
    constb = np.zeros((128, _CB), np.float32)

    def put(nm, arr):
        c0, cols = _LAY_B[nm]
        r, cc = arr.shape
        constb[0:r, c0:c0 + cc] = arr
    wt = w['W_trans']
    for kc in range(6):
        put(f"wtr{kc}", wt[kc * 128:(kc + 1) * 128])
    put("brow", w['b_trans'].reshape(1, EMB))
    put("onesrow", np.ones((1, 128), np.float32))
    put("onescol", np.ones((128, 1), np.float32))
    g2T = np.zeros((EM, E), np.float32)
    for e in range(E):
        g2T[e * M:(e + 1) * M, e] = 1.0
    put("g2T", g2T)
    sumT = np.kron(np.eye(L, dtype=np.float32), np.ones((SPAN, 1), np.float32))
    for kc in range(4):
        put(f"sumT{kc}", sumT[kc * 128:(kc + 1) * 128])
    for nm, arr, nch in (("fsw1T", w['fs_w1'].T, 4), ("fcw1T", w['fc_w1'].T, 4),
                         ("fsw2T", w['fs_w2'].T, 2), ("fcw2T", w['fc_w2'].T, 2)):
        for kc in range(nch):
            put(f"{nm}{kc}", np.ascontiguousarray(arr[kc * 128:(kc + 1) * 128]))
    out['constb'] = _bf(constb)

    gT = np.zeros((EMH, E), np.float32)
    for e in range(E):
        gT[e * M * H:(e + 1) * M * H, e] = 1.0 / (M * H)
    gTb = np.zeros((128, 9 * E), np.float32)
    for kc in range(9):
        r = min(128, EMH - kc * 128)
        gTb[0:r, kc * E:(kc + 1) * E] = gT[kc * 128:kc * 128 + r]
    out['gTb'] = np.ascontiguousarray(gTb.astype(ml_dtypes.float8_e4m3))
    out['aallTb'] = _bf(np.concatenate(
        [ADJ[r].T for r in range(NREL)] + [np.eye(NN, dtype=np.float32)],
        axis=1))
    out['tfb'] = _bf(np.ascontiguousarray(w['type_embed'][_TYPES]))
    out['identb'] = _bf(np.eye(128, dtype=np.float32))

    constf = np.zeros((128, _CF), np.float32)

    def putf(nm, arr):
        c0, cols = _LAY_F[nm]
        constf[0:arr.shape[0], c0:c0 + 1] = arr.reshape(-1, 1)
    vecs = {"ses1": w['fs_g1'], "seb1": w['fs_b1'] * w['fs_g1'] + w['fs_be1'],
            "fcs1": w['fc_g1'], "fcb1": w['fc_b1'] * w['fc_g1'] + w['fc_be1'],
            "ses2": w['fs_g2'], "seb2": w['fs_b2'] * w['fs_g2'] + w['fs_be2'],
            "fcs2": w['fc_g2'],
            "fcb2": w['fc_b2'] * w['fc_g2'] + w['fc_be2'] +
                    w['fs_b2'] * w['fs_g2'] + w['fs_be2']}
    for nm, v in vecs.items():
        nch = 2 if v.shape[0] == INTER else 4
        for kc in range(nch):
            putf(f"{nm}{kc}", v[kc * 128:(kc + 1) * 128])
    out['constf_base'] = constf

    for layer in range(NLAYERS):
        din_l = D0 if layer == 0 else EMB
        kcs = _KC0 if layer == 0 else _KC1
        nk = len(kcs)
        Wst = w['rgcn_Wrel0'].reshape(NREL * D0, EMB) if layer == 0 else \
            w['rgcn_Wrel'][layer - 1].reshape(NREL * EMB, EMB)
        Wself = w['rgcn_Wself0'] if layer == 0 else w['rgcn_Wself'][layer - 1]
        p = np.zeros((128, (NREL + 1) * nk * EMB), np.float32)
        for si, (s0, sl) in enumerate(kcs):
            for r in range(NREL):
                b = si * (NREL + 1) + r
                p[0:sl, b * EMB:(b + 1) * EMB] = \
                    Wst[r * din_l + s0:r * din_l + s0 + sl]
            b = si * (NREL + 1) + NREL
            p[0:sl, b * EMB:(b + 1) * EMB] = Wself[s0:s0 + sl]
        out[f'wstp{layer}'] = _bf(p)
    return out


def _prep_conv_half(w, half, constf_base):
    out = {}
    w1 = w['cr_w1'][half * 128:(half + 1) * 128]
    out['w1sb'] = _bf(np.ascontiguousarray(
        w1.transpose(1, 2, 3, 0).reshape(4, 128, 25 * 128)))
    # conv2/conv3 weight chunks in natural (rank-ordered) input-half order
    w2 = w['cr_w2'][half * 128:(half + 1) * 128]
    out['w2sb'] = _bf(np.ascontiguousarray(
        w2.transpose(1, 2, 3, 0).reshape(2, 128, 25 * 128)))
    w3 = w['cr_w3'][half * 256:(half + 1) * 256]
    out['w3sb'] = _bf(np.ascontiguousarray(
        w3.transpose(1, 2, 3, 0).reshape(2, 128, 25 * 256)))
    constf = constf_base.copy()

    def putf(nm, arr):
        c0, cols = _LAY_F[nm]
        constf[0:arr.shape[0], c0:c0 + 1] = arr.reshape(-1, 1)
    putf("b1h", w['cr_b1'][half * 128:(half + 1) * 128])
    putf("b2h", w['cr_b2'][half * 128:(half + 1) * 128])
    putf("b3h0", w['cr_b3'][half * 256:half * 256 + 128])
    putf("b3h1", w['cr_b3'][half * 256 + 128:half * 256 + 256])
    putf("mtop", np.full(128, float(half), np.float32))
    putf("mbot", np.full(128, float(1 - half), np.float32))
    c0, cols = _LAY_F["identf"]
    constf[:, c0:c0 + 128] = np.eye(128, dtype=np.float32)
    out['constf'] = constf
    return out


def _prep_doc(x, att, mi, ls):
    out = {}
    mif = mi.reshape(EM)
    attm = np.ascontiguousarray(
        att[:, mif, :].transpose(1, 0, 2).reshape(EMH, C))
    amp = np.zeros((128, 9 * C), np.float32)
    for kc in range(9):
        r = min(128, EMH - kc * 128)
        amp[0:r, kc * C:kc * C + C] = attm[kc * 128:kc * 128 + r]
    out['amp'] = np.ascontiguousarray(amp.astype(ml_dtypes.float8_e4m3))
    idx = ls[:, None] + np.arange(SPAN)
    idxf = idx.reshape(LS)
    rows = att[:, idxf, :].reshape(H, L, SPAN, C)
    blocks = np.take_along_axis(rows, idx[None, :, None, :], axis=3)
    attl = blocks.transpose(0, 2, 1, 3).reshape(HS, LS)
    xmT = x[mif].T
    xspT = x[idxf].T
    actb = np.zeros((128, _CA), np.float32)

    def put(nm, arr):
        c0, cols = _LAY_A[nm]
        actb[0:arr.shape[0], c0:c0 + arr.shape[1]] = arr
    for kc in range(6):
        put(f"xmT{kc}", xmT[kc * 128:(kc + 1) * 128])
        put(f"xspT{kc}", xspT[kc * 128:(kc + 1) * 128])
    for kc in range(3):
        put(f"attl{kc}", attl[kc * 128:(kc + 1) * 128])
    out['actb'] = _bf(actb)
    xpk = np.zeros((128, 8 * HID), np.float32)
    for kc in range(8):
        xpk[:, kc * HID:(kc + 1) * HID] = x[kc * 128:(kc + 1) * 128]
    out['xp'] = _bf(xpk)
    return out


def build_in_maps(inputs):
    w = {}
    for k, v in inputs.items():
        a = np.asarray(v)
        w[k] = a if a.dtype in (np.int32, np.int64) else \
            np.asarray(a, np.float32)
    shared = _prep_shared(w)
    constf_base = shared.pop('constf_base')
    halves = [_prep_conv_half(w, h, constf_base) for h in range(2)]
    seq = np.asarray(inputs['sequence_output'], np.float32)
    att = np.asarray(inputs['attention'], np.float32)
    mi = np.asarray(inputs['mention_idx']).astype(np.int64)
    ls = np.asarray(inputs['link_start']).astype(np.int64)
    docs = [_prep_doc(seq[n], att[n], mi[n], ls[n]) for n in range(NB)]
    in_maps = []
    for core in range(N_CORES):
        n, half = core // 2, core % 2
        m = dict(shared)
        m.update(halves[half])
        m.update(docs[n])
        in_maps.append({k: (np.ascontiguousarray(v)
                            if v.dtype in (ml_dtypes.bfloat16,
                                           ml_dtypes.float8_e4m3)
                            else np.ascontiguousarray(v, np.float32))
                        for k, v in m.items()})
    return in_maps


def kernel(**inputs):
    nc = _get_program()
    in_maps = build_in_maps(inputs)
    res = run_bass_kernel_spmd(nc, in_maps, list(range(N_CORES)))
    out = np.zeros((NB, EMB, E, E), np.float32)
    for core in range(N_CORES):
        n, half = core // 2, core % 2
        out[n, half * 256:(half + 1) * 256] = \
            res.results[core]["out"].reshape(256, E, E)
    return out


# revision 55
# speedup vs baseline: 1.3627x; 1.0010x over previous
"""Trainium2 Bass kernel for nn_DocREModel (DocRE: gather -> RGCN -> SE -> 5x5 convs).

Sharding: 4 documents x 2 cores each. Each pair replicates the cheap upstream
(mention/link/ea gathers -> RGCN -> fmap/SE) and splits the dominant 5x5 conv
stack by output channels, with two intra-pair AllGathers; output halves are
assembled on host. All index-driven gathers happen on host (pure data
movement; one SPMD program serves all 8 cores), all dense math on device.

Perf model notes (TimelineSim): all DMAs serialize on one ~332 GB/s pipe in
~issue order, and the PE p-state ramp rewards keeping the tensor engine
continuously fed. Hence: everything DMA'd is bf16 (f32 only for small
per-channel scale/bias vectors), tensors are issued strictly in first-use
order (amp/gTb first so the ea matmuls start ~2.5us in), RGCN + conv weights
stream just-in-time behind the compute, and h0 is assembled directly by ACT
writes into a bf16 tile instead of SBUF->SBUF DMA round trips.

Precision/layout choices:
- bf16 weights+activations everywhere on the matmul path, f32 PSUM
  accumulation throughout; per-channel BN scales/biases stay f32.
- Convs are 25 shift-tap matmuls over zero-padded 26x26 images via strided
  APs (no im2col copies). conv2/conv3 start on the locally-computed input
  half before the pair AllGather completes; the other half is extracted
  SPMD-safely with host-supplied 0/1 masks and per-core (own, other)
  weight-chunk ordering.
- RGCN folds the self-loop in as a 4th identity relation so each layer is
  one u = h^T @ [A0^T|A1^T|A2^T|I] matmul plus one PSUM accumulation over
  stacked (relation, chunk) weights -- no transposes in the loop.
"""

import numpy as np
import ml_dtypes

import concourse.bacc as bacc
import concourse.tile as tile
from concourse import mybir
from concourse.bass_utils import run_bass_kernel_spmd

F32 = mybir.dt.float32
F32R = mybir.dt.float32r
BF16 = mybir.dt.bfloat16
F8 = mybir.dt.float8e4
AF = mybir.ActivationFunctionType
ALU = mybir.AluOpType

NB, H, C, HID, EMB = 4, 12, 1024, 768, 512
E, M, L, SPAN = 22, 4, 16, 32
TD, INTER = 20, 256
NN = E + E * M + L
NREL, NLAYERS = 3, 4
EM, EMH, HS, LS = E * M, E * M * H, H * SPAN, L * SPAN
D0 = EMB + TD           # 532
EE = E * E              # 484
PADW = 26 * 26          # 676 padded 26x26 image
N_CORES = 8


def _build_adj():
    A = np.zeros((NREL, NN, NN), np.float32)
    for e in range(E):
        for m in range(M):
            mi = E + e * M + m
            A[0, e, mi] = A[0, mi, e] = 1.0
            for m2 in range(M):
                if m2 != m:
                    A[1, mi, E + e * M + m2] = 1.0
            li = E + E * M + ((e * M + m) % L)
            A[2, mi, li] = A[2, li, mi] = 1.0
    A = A / (A.sum(-1, keepdims=True) + 1e-5)
    return A


_TYPES = np.concatenate([np.zeros(E, np.int32), np.ones(EM, np.int32),
                         np.full(L, 2, np.int32)])

_KC0 = [(0, 128), (128, 128), (256, 128), (384, 128), (512, 20)]   # 532 rows
_KC1 = [(0, 128), (128, 128), (256, 128), (384, 128)]              # 512 rows


def _constb_layout():
    """Column layout of the packed bf16 constant tensor [128, CB].

    Part A (cols 0:CBA) is everything needed through stage 3's s1/c1;
    part B (fsw2T/fcw2T) is DMA'd later, after the RGCN weights.
    """
    lay = {}
    c = 0

    def add(nm, cols):
        nonlocal c
        lay[nm] = (c, cols)
        c += cols
    for kc in range(6):
        add(f"wtr{kc}", EMB)
    add("brow", EMB)
    add("onesrow", 128)
    add("onescol", 1)
    add("g2T", E)
    for kc in range(4):
        add(f"sumT{kc}", L)
    for kc in range(4):
        add(f"fsw1T{kc}", INTER)
    for kc in range(4):
        add(f"fcw1T{kc}", INTER)
    cba = c
    for kc in range(2):
        add(f"fsw2T{kc}", EMB)
    for kc in range(2):
        add(f"fcw2T{kc}", EMB)
    return lay, c, cba


def _constf_layout():
    lay = {}
    c = 0

    def add(nm, cols):
        nonlocal c
        lay[nm] = (c, cols)
        c += cols
    for nm, nch in (("ses1", 2), ("seb1", 2), ("fcs1", 2), ("fcb1", 2),
                    ("ses2", 4), ("seb2", 4), ("fcs2", 4), ("fcb2", 4)):
        for kc in range(nch):
            add(f"{nm}{kc}", 1)
    add("b1h", 1)
    add("b2h", 1)
    add("b3h0", 1)
    add("b3h1", 1)
    add("mtop", 1)
    add("mbot", 1)
    add("identf", 128)
    return lay, c


def _actb_layout():
    lay = {}
    c = 0

    def add(nm, cols):
        nonlocal c
        lay[nm] = (c, cols)
        c += cols
    for kc in range(6):
        add(f"xmT{kc}", EM)
    for kc in range(6):
        add(f"xspT{kc}", LS)
    for kc in range(3):
        add(f"attl{kc}", LS)
    return lay, c


_LAY_B, _CB, _CBA = _constb_layout()
_LAY_F, _CF = _constf_layout()
_LAY_A, _CA = _actb_layout()


def build_program(solo=False, stages=4):
    nc = bacc.Bacc("TRN2", target_bir_lowering=False, debug=False)

    def din(name, shape, dt=BF16):
        return nc.dram_tensor(name, list(shape), dt, kind="ExternalInput").ap()

    constb_d = din("constb", [128, _CB])
    constf_d = din("constf", [128, _CF], F32)
    actb_d = din("actb", [128, _CA])
    xp_d = din("xp", [128, 8 * HID])
    amp_d = din("amp", [128, 9 * C], F8)
    gTb_d = din("gTb", [128, 9 * E], F8)
    tfb_d = din("tfb", [NN, TD])
    wstp_d = [din("wstp0", [128, 20 * EMB])] + \
             [din(f"wstp{i}", [128, 16 * EMB]) for i in (1, 2, 3)]
    w1sb_d = din("w1sb", [4, 128, 25 * 128])
    w2sb_d = din("w2sb", [2, 128, 25 * 128])
    w3sb_d = din("w3sb", [2, 128, 25 * 256])
    aallTb_d = din("aallTb", [NN, (NREL + 1) * NN])
    identb_d = din("identb", [128, 128])

    out_d = nc.dram_tensor("out", [256, EE], F32, kind="ExternalOutput").ap()

    groups = [[0, 1], [2, 3], [4, 5], [6, 7]]

    with tile.TileContext(nc) as tc:
      with tc.tile_pool(name="pconst", bufs=1) as pconst, \
           tc.tile_pool(name="pwork", bufs=1) as pwork, \
           tc.tile_pool(name="pdram", bufs=1, space="DRAM") as pdram:

        constb = pconst.tile([128, _CB], BF16)
        constf = pconst.tile([128, _CF], F32)
        identb = pconst.tile([128, 128], BF16)
        aallTb = pconst.tile([NN, (NREL + 1) * NN], BF16)
        aallE = pconst.tile([E, (NREL + 1) * NN], BF16)
        aallM = pconst.tile([EM, (NREL + 1) * NN], BF16)
        aallL = pconst.tile([L, (NREL + 1) * NN], BF16)
        wstp_t = [pconst.tile([128, 20 * EMB], BF16, tag="wstp0",
                              name="wstp0")] + \
                 [pconst.tile([128, 16 * EMB], BF16, tag=f"wstp{l}",
                              name=f"wstp{l}") for l in (1, 2, 3)]
        w1 = [pconst.tile([128, 25 * 128], BF16, tag=f"w1_{kc}",
                          name=f"w1_{kc}") for kc in range(4)]

        def cb(nm, rows=128):
            c0, cols = _LAY_B[nm]
            return constb[0:rows, c0:c0 + cols]

        def cf(nm, rows=128):
            c0, cols = _LAY_F[nm]
            return constf[0:rows, c0:c0 + cols]

        wtr = [cb(f"wtr{kc}") for kc in range(6)]
        brow = cb("brow", rows=1)
        onesrow = cb("onesrow", rows=1)
        onescol = cb("onescol")
        g2T = cb("g2T", rows=EM)
        sumT = [cb(f"sumT{kc}") for kc in range(4)]
        sew = {nm: [cb(f"{nm}{kc}") for kc in range(n)]
               for nm, n in (("fsw1T", 4), ("fcw1T", 4), ("fsw2T", 2),
                             ("fcw2T", 2))}
        sev = {nm: [cf(f"{nm}{kc}") for kc in range(n)]
               for nm, n in (("ses1", 2), ("seb1", 2), ("fcs1", 2), ("fcb1", 2),
                             ("ses2", 4), ("seb2", 4), ("fcs2", 4),
                             ("fcb2", 4))}
        b1h = cf("b1h")
        b2h = cf("b2h")
        b3h = [cf("b3h0"), cf("b3h1")]
        ident = cf("identf")

        # persistent intermediates (three base-0 tiles: engines cannot
        # write SBUF at unaligned base partitions, so the node matrix is
        # kept split as [entities; mentions; links])
        h0e = pwork.tile([E, D0], BF16)
        h0m = pwork.tile([EM, D0], BF16)
        h0l = pwork.tile([L, D0], BF16)
        hfin = pwork.tile([NN, EMB], BF16)
        ectxT_sb = [pwork.tile([128, E], F32, tag=f"ectxT{i}", name=f"ectxT{i}")
                    for i in range(4)]
        ecT = [pwork.tile([128, E], F32R, tag=f"ecT{i}", name=f"ecT{i}")
               for i in range(4)]
        # PE warmup fodder: covers the head until real operands land (the
        # scheduler hoists dependency-free matmuls to the front).
        warm = pwork.tile([128, 512], BF16)
        nc.vector.memset(warm[:], 0.0)
        fusedp = [pwork.tile([128, PADW], BF16, tag=f"fusedp{i}",
                             name=f"fusedp{i}") for i in range(4)]
        g1pc = pwork.tile([128, 2 * PADW], BF16, tag="g1pc", name="g1pc")
        g2pc = pwork.tile([128, 2 * PADW], BF16, tag="g2pc", name="g2pc")
        g1p = [g1pc[:, i * PADW:(i + 1) * PADW] for i in range(2)]
        g2p = [g2pc[:, i * PADW:(i + 1) * PADW] for i in range(2)]
        for t_ in fusedp:
            nc.vector.memset(t_[:], 0.0)
        nc.vector.memset(g1pc[:], 0.0)
        nc.vector.memset(g2pc[:], 0.0)

        with tc.tile_pool(name="pbig", bufs=1) as pbig:
            gTb = pbig.tile([128, 9 * E], F8)
            amp = pbig.tile([128, 9 * C], F8)
            xp = pbig.tile([128, 8 * HID], BF16)
            actb = pbig.tile([128, _CA], BF16)

            # ---- the bulk DMA stream rides the SWDGE (gpsimd) ring in
            # first-use order; sync/scalar stay shallow for latency-
            # critical transfers later (conv exchanges, outputs) ----
            nc.scalar.dma_start(constf[:], constf_d[:])
            xm_cols = 6 * EM                      # xmT region of actb
            wtr_cols = 6 * EMB + EMB + 128 + 1    # wtr+brow+ones region
            nc.gpsimd.dma_start(actb[:, 0:xm_cols], actb_d[:, 0:xm_cols])
            nc.gpsimd.dma_start(constb[:, 0:wtr_cols], constb_d[:, 0:wtr_cols])
            se1_cols = wtr_cols + E + 4 * L   # g2T+sumT end
            nc.gpsimd.dma_start(constb[:, wtr_cols:se1_cols],
                                constb_d[:, wtr_cols:se1_cols])
            sp_cols = xm_cols + 6 * LS
            nc.gpsimd.dma_start(actb[:, xm_cols:sp_cols],
                                actb_d[:, xm_cols:sp_cols])
            nc.gpsimd.dma_start(actb[:, sp_cols:_CA], actb_d[:, sp_cols:_CA])
            nc.scalar.dma_start(h0e[:, EMB:D0], tfb_d[0:E, :])
            nc.scalar.dma_start(h0m[:, EMB:D0], tfb_d[E:E + EM, :])
            nc.scalar.dma_start(h0l[:, EMB:D0], tfb_d[E + EM:NN, :])
            nc.gpsimd.dma_start(aallTb[:], aallTb_d[:])
            nc.gpsimd.dma_start(aallE[:], aallTb_d[0:E, :])
            nc.gpsimd.dma_start(aallM[:], aallTb_d[E:E + EM, :])
            nc.gpsimd.dma_start(aallL[:], aallTb_d[E + EM:NN, :])
            # RGCN weights, chunked si-major so each layer's PSUM chain can
            # start as soon as its first chunk lands
            BL = (NREL + 1) * EMB

            def wstp_dma(layer):
                nchunks = 5 if layer == 0 else 4
                for si in range(nchunks):
                    nc.gpsimd.dma_start(
                        wstp_t[layer][:, si * BL:(si + 1) * BL],
                        wstp_d[layer][:, si * BL:(si + 1) * BL])
            wstp_dma(0)
            wstp_dma(1)
            nc.gpsimd.dma_start(gTb[:], gTb_d[:])
            for g in range(3):
                nc.gpsimd.dma_start(amp[:, g * 3 * C:(g + 1) * 3 * C],
                                    amp_d[:, g * 3 * C:(g + 1) * 3 * C])
            nc.gpsimd.dma_start(xp[:], xp_d[:])
            nc.gpsimd.dma_start(constb[:, se1_cols:_CBA],
                                constb_d[:, se1_cols:_CBA])
            wstp_dma(2)
            wstp_dma(3)
            nc.gpsimd.dma_start(constb[:, _CBA:_CB], constb_d[:, _CBA:_CB])
            nc.scalar.dma_start(identb[:], identb_d[:])
            for kc in range(4):
                nc.gpsimd.dma_start(w1[kc][:], w1sb_d[kc])

            # ========== stage 1a: mention/span/link rows -> h0b ==========
            expm = pbig.tile([EM, EMB], BF16)
            sp_ps = []
            wsb = [pbig.tile([128, 1], F32, tag=f"wsb{i}", name=f"wsb{i}")
                   for i in range(4)]
            wsp = [pbig.tile([128, EMB], BF16, tag=f"wsp{i}", name=f"wsp{i}")
                   for i in range(4)]

            def ca(nm, rows=128):
                c0, cols = _LAY_A[nm]
                return actb[0:rows, c0:c0 + cols]

            xmT = [ca(f"xmT{kc}") for kc in range(6)]
            xspT = [ca(f"xspT{kc}") for kc in range(6)]
            attl = [ca(f"attl{kc}") for kc in range(3)]

            with tc.tile_pool(name="ps1a", bufs=1, space="PSUM") as ps1a:
                jp = ps1a.tile([128, 512], F32, tag="jp", name="jp")
                for _ in range(14):
                    nc.tensor.matmul(jp[:], warm[:, 0:128], warm[:],
                                     start=True, stop=True)
                # mentions: mrep = x_m @ Wtr + b -> h0b rows + exp for pooling
                mrep_p = ps1a.tile([EM, EMB], F32, tag="mrep", name="mrep")
                for kc in range(6):
                    nc.tensor.matmul(mrep_p[:], xmT[kc][:, 0:EM], wtr[kc][:],
                                     start=(kc == 0), stop=False)
                nc.tensor.matmul(mrep_p[:], onesrow[0:1, 0:EM], brow[:],
                                 start=False, stop=True)
                nc.scalar.copy(h0m[:, 0:EMB], mrep_p[:])
                nc.scalar.activation(expm[:], mrep_p[:], AF.Exp)
                # e_rep = ln(G2 @ exp(mrep))
                ep_p = ps1a.tile([E, EMB], F32, tag="ep", name="ep")
                nc.tensor.matmul(ep_p[:], g2T[:], expm[:], start=True, stop=True)
                nc.scalar.activation(h0e[:, 0:EMB], ep_p[:], AF.Ln)
                # dummy: switch the ACT table to the sigmoid set now (exp/ln
                # are done) so stage 3's sigmoid doesn't pay the 1.3us load
                sigwarm = pbig.tile([1, 1], F32)
                nc.scalar.activation(sigwarm[:], ep_p[0:1, 0:1], AF.Sigmoid)

                # spans: sp = x_span @ Wtr + b
                for mc in range(4):
                    sp_p = ps1a.tile([128, EMB], F32, tag="sp_p", name="sp_p",
                                     bufs=3)
                    for kc in range(6):
                        nc.tensor.matmul(sp_p[:],
                                         xspT[kc][:, mc * 128:(mc + 1) * 128],
                                         wtr[kc][:], start=(kc == 0), stop=False)
                    nc.tensor.matmul(sp_p[:], onesrow[:], brow[:],
                                     start=False, stop=True)
                    spc = pbig.tile([128, EMB], BF16, tag="spc", name="spc",
                                    bufs=4)
                    nc.scalar.copy(spc[:], sp_p[:])
                    sp_ps.append(spc)
                # w = colsum(attl) / 384
                for mc in range(4):
                    w_p = ps1a.tile([128, 1], F32, tag="w_p", name="w_p", bufs=1)
                    for kc in range(3):
                        nc.tensor.matmul(w_p[:],
                                         attl[kc][:, mc * 128:(mc + 1) * 128],
                                         onescol[:],
                                         start=(kc == 0), stop=(kc == 2))
                    nc.scalar.activation(wsb[mc][:], w_p[:], AF.Copy,
                                         scale=1.0 / (H * SPAN))
                # wsp = psum(sp) * w ; link = SUM^T @ wsp
                for mc in range(4):
                    nc.vector.tensor_scalar(out=wsp[mc][:], in0=sp_ps[mc][:],
                                            scalar1=wsb[mc][:], scalar2=None,
                                            op0=ALU.mult)
                link_p = ps1a.tile([L, EMB], F32, tag="link", name="link")
                for kc in range(4):
                    nc.tensor.matmul(link_p[:], sumT[kc][:], wsp[kc][:],
                                     start=(kc == 0), stop=(kc == 3))
                nc.scalar.copy(h0l[:, 0:EMB], link_p[:])

            # ====== stage 2 + stage 1b interleaved: the ea/e_ctx latency
            # chain fills the RGCN's weight-stream stalls ======
            ea_sb = pbig.tile([E, C], F32R)
            eaT = [pbig.tile([128, E], BF16, tag=f"eaT{i}", name=f"eaT{i}")
                   for i in range(8)]
            z_sb = [pbig.tile([128, E], BF16, tag=f"z{i}", name=f"z{i}")
                    for i in range(6)]
            easumT = pbig.tile([1, E], BF16)

            if stages >= 2:
              with tc.tile_pool(name="prg", bufs=2) as prg, \
                   tc.tile_pool(name="psr", bufs=1, space="PSUM") as psr:

                def rgcn_layer(layer, h):
                    kcs = _KC0 if layer == 0 else _KC1
                    nk = len(kcs)
                    wstp = wstp_t[layer]
                    # si-major packing: block (si, r) at (si*(NREL+1)+r)*EMB
                    wst_t = [wstp[:, (si * (NREL + 1) + r) * EMB:
                                   (si * (NREL + 1) + r + 1) * EMB]
                             for r in range(NREL + 1) for si in range(nk)]
                    u_sb = []
                    for si, (s0, sl) in enumerate(kcs):
                        u_p = psr.tile([128, (NREL + 1) * NN], F32, tag="u_p",
                                       name="u_p", bufs=2)
                        if layer == 0:
                            nc.tensor.matmul(u_p[0:sl, :],
                                             h0e[:, s0:s0 + sl], aallE[:],
                                             start=True, stop=False)
                            nc.tensor.matmul(u_p[0:sl, :],
                                             h0m[:, s0:s0 + sl], aallM[:],
                                             start=False, stop=False)
                            nc.tensor.matmul(u_p[0:sl, :],
                                             h0l[:, s0:s0 + sl], aallL[:],
                                             start=False, stop=True)
                        else:
                            nc.tensor.matmul(u_p[0:sl, :], h[0:NN, s0:s0 + sl],
                                             aallTb[:], start=True, stop=True)
                        u = prg.tile([128, (NREL + 1) * NN], BF16, tag=f"u{si}",
                                     name=f"u{si}", bufs=1)
                        if si % 2 == 0:
                            nc.scalar.copy(u[0:sl, :], u_p[0:sl, :])
                        else:
                            nc.vector.tensor_copy(out=u[0:sl, :],
                                                  in_=u_p[0:sl, :])
                        u_sb.append(u)
                    y_p = psr.tile([NN, EMB], F32, tag="y_p", name="y_p")
                    n_mm = (NREL + 1) * nk
                    k_mm = 0
                    for si, (s0, sl) in enumerate(kcs):
                        for r in range(NREL + 1):
                            nc.tensor.matmul(
                                y_p[:], u_sb[si][0:sl, r * NN:(r + 1) * NN],
                                wst_t[r * nk + si][0:sl, :],
                                start=(k_mm == 0), stop=(k_mm == n_mm - 1))
                            k_mm += 1
                    hdst = hfin if layer == NLAYERS - 1 else \
                        prg.tile([NN, EMB], BF16, tag="h_next", name="h_next")
                    for (s0, sl) in _KC1:
                        nc.scalar.activation(hdst[0:NN, s0:s0 + sl],
                                             y_p[0:NN, s0:s0 + sl], AF.Relu)
                    return hdst

                h1 = rgcn_layer(0, None)

                # -- ea block (runs while wstp1 streams) --
                with tc.tile_pool(name="ps1b", bufs=1, space="PSUM") as ps1b:
                    ea_p0 = ps1b.tile([E, 512], F32, tag="ea0", name="ea0")
                    ea_p1 = ps1b.tile([E, 512], F32, tag="ea1", name="ea1")
                    for kc in range(9):
                        rows = 128 if kc < 8 else 32
                        at = amp[0:rows, kc * C:kc * C + C]
                        gt = gTb[0:rows, kc * E:(kc + 1) * E]
                        nc.tensor.matmul(ea_p0[:], gt, at[:, 0:512],
                                         start=(kc == 0), stop=(kc == 8))
                        nc.tensor.matmul(ea_p1[:], gt, at[:, 512:1024],
                                         start=(kc == 0), stop=(kc == 8))
                    r0 = pbig.tile([E, 1], F32)
                    r1 = pbig.tile([E, 1], F32)
                    nc.vector.tensor_reduce(r0[:], ea_p0[:],
                                            mybir.AxisListType.X, ALU.add)
                    nc.vector.tensor_reduce(r1[:], ea_p1[:],
                                            mybir.AxisListType.X, ALU.add)
                    rsum = pbig.tile([E, 1], F32)
                    nc.vector.tensor_tensor(out=rsum[:], in0=r0[:], in1=r1[:],
                                            op=ALU.add)
                    rsum2 = pbig.tile([E, 1], F32)
                    nc.vector.tensor_scalar(out=rsum2[:], in0=rsum[:],
                                            scalar1=1e-5, scalar2=None,
                                            op0=ALU.add)
                    rinv = pbig.tile([E, 1], F32)
                    nc.vector.reciprocal(rinv[:], rsum2[:])
                    for kc in range(4):
                        c0, c1_ = kc * 128, (kc + 1) * 128
                        if kc % 2 == 0:
                            nc.scalar.copy(ea_sb[:, c0:c1_], ea_p0[:, c0:c1_])
                            nc.scalar.copy(ea_sb[:, 512 + c0:512 + c1_],
                                           ea_p1[:, c0:c1_])
                        else:
                            nc.vector.tensor_copy(out=ea_sb[:, c0:c1_],
                                                  in_=ea_p0[:, c0:c1_])
                            nc.vector.tensor_copy(
                                out=ea_sb[:, 512 + c0:512 + c1_],
                                in_=ea_p1[:, c0:c1_])
                    easum = pbig.tile([E, 1], F32)
                    nc.vector.tensor_tensor(out=easum[:], in0=rsum[:],
                                            in1=rinv[:], op=ALU.mult)
                    # eaT transposes reuse the (now dead) ea psum banks
                    for kc in range(8):
                        tp = ps1b.tile([128, E], F32, tag=f"ea{kc % 2}",
                                       name="eaTt")
                        nc.tensor.transpose(tp[:],
                                            ea_sb[:, kc * 128:(kc + 1) * 128]
                                            .bitcast(F32), ident[0:E, 0:E])
                        if kc % 2 == 0:
                            nc.scalar.copy(eaT[kc][:], tp[:])
                        else:
                            nc.vector.tensor_copy(out=eaT[kc][:], in_=tp[:])
                    tp = ps1b.tile([1, E], F32, tag="ea1", name="easumt")
                    nc.tensor.transpose(tp[:], easum[:], ident[0:E, 0:E])
                    nc.scalar.copy(easumT[:], tp[:])

                h2 = rgcn_layer(1, h1)

                with tc.tile_pool(name="ps1c", bufs=1, space="PSUM") as ps1c:
                    # zT = ea_n @ x  [22, 768] (two 384-wide halves)
                    zt_ps = [ps1c.tile([E, 384], F32, tag="sc",
                                       name=f"zt_p{i}", bufs=2)
                             for i in range(2)]
                    for kc in range(8):
                        xt = xp[:, kc * HID:(kc + 1) * HID]
                        for hh in range(2):
                            nc.tensor.matmul(zt_ps[hh][:], eaT[kc][:],
                                             xt[:, hh * 384:(hh + 1) * 384],
                                             start=(kc == 0), stop=(kc == 7))
                    # ea was left unnormalized; fold the 1/rowsum in here
                    zt_sb = pbig.tile([E, HID], F32)
                    nc.scalar.activation(zt_sb[:, 0:384], zt_ps[0][:], AF.Copy,
                                         scale=rinv[:])
                    nc.scalar.activation(zt_sb[:, 384:768], zt_ps[1][:],
                                         AF.Copy, scale=rinv[:])
                    for kc in range(6):
                        ztp = ps1c.tile([128, E], F32, tag="tp", name="ztp",
                                        bufs=1)
                        nc.tensor.transpose(ztp[:],
                                            zt_sb[:, kc * 128:(kc + 1) * 128],
                                            ident[0:E, 0:E])
                        if kc % 2 == 0:
                            nc.scalar.copy(z_sb[kc][:], ztp[:])
                        else:
                            nc.vector.tensor_copy(out=z_sb[kc][:], in_=ztp[:])
                    # ecT2 = z^T-chunks as lhsT @ Wtr -> [22,512] + b (x) easum
                    ec2_p = ps1c.tile([E, EMB], F32, tag="sc", name="ec2",
                                      bufs=2)
                    for kc in range(6):
                        nc.tensor.matmul(ec2_p[:], z_sb[kc][:], wtr[kc][:],
                                         start=(kc == 0), stop=False)
                    nc.tensor.matmul(ec2_p[:], easumT[:], brow[:],
                                     start=False, stop=True)
                    ec2_sb = pbig.tile([E, EMB], F32)
                    nc.scalar.copy(ec2_sb[:], ec2_p[:])
                    for mc in range(4):
                        ecp = ps1c.tile([128, E], F32, tag="tp", name="ecp",
                                        bufs=1)
                        nc.tensor.transpose(ecp[:],
                                            ec2_sb[:, mc * 128:(mc + 1) * 128],
                                            ident[0:E, 0:E])
                        if mc % 2 == 0:
                            nc.scalar.copy(ectxT_sb[mc][:], ecp[:])
                        else:
                            nc.vector.tensor_copy(out=ectxT_sb[mc][:],
                                                  in_=ecp[:])

                    h3 = rgcn_layer(2, h2)
                    rgcn_layer(3, h3)

                    # entity_struT + e_ctxT -> ecT
                    for mc in range(4):
                        tp = ps1c.tile([128, E], F32,
                                       tag="tp" if mc % 2 == 0 else "sc",
                                       name="est", bufs=1 if mc % 2 == 0 else 2)
                        nc.tensor.matmul(tp[:],
                                         hfin[0:E, mc * 128:(mc + 1) * 128],
                                         identb[0:E, 0:E], start=True,
                                         stop=True)
                        nc.vector.tensor_tensor(out=ecT[mc][:], in0=tp[:],
                                                in1=ectxT_sb[mc][:],
                                                op=ALU.add)

        if stages >= 3:
          # ================= stage 3: fmap + SE =================
          fmap = [pwork.tile([128, EE], BF16, tag=f"fmap{i}", name=f"fmap{i}")
                  for i in range(4)]
          pooled = [pwork.tile([128, 1], BF16, tag=f"pool{i}", name=f"pool{i}")
                    for i in range(4)]
          for mc in range(4):
              for ee, lo, hi in ((nc.vector, 0, 11), (nc.gpsimd, 11, E)):
                  o6v = fmap[mc][:].rearrange("p (i j) -> p i j", i=E)[:, lo:hi]
                  in0 = ecT[mc][:, lo:hi].rearrange("p (i j) -> p i j", j=1) \
                      .to_broadcast([128, hi - lo, E])
                  in1 = ecT[mc][:].rearrange("p (o j) -> p o j", o=1) \
                      .to_broadcast([128, hi - lo, E])
                  ee.tensor_tensor(out=o6v, in0=in0, in1=in1, op=ALU.mult)
              rs = pwork.tile([128, 1], F32, tag=f"rs{mc}", name=f"rs{mc}")
              nc.vector.tensor_reduce(rs[:], ecT[mc][:], mybir.AxisListType.X,
                                      ALU.add)
              nc.scalar.activation(pooled[mc][:], rs[:], AF.Square,
                                   scale=1.0 / E)

          pse_cm = tc.tile_pool(name="pse", bufs=1, space="PSUM")
          pse = pse_cm.__enter__()
          if True:
              # channel-attention path first: its latency hides under the
              # fmap outer-product DVE chain
              c1_sb = [pwork.tile([128, 1], BF16, tag=f"c1_{i}", name=f"c1_{i}")
                       for i in range(2)]
              for oc in range(2):
                  c1_p = pse.tile([128, 1], F32, tag="c1p", name="c1p")
                  for mc in range(4):
                      nc.tensor.matmul(c1_p[:],
                                       sew["fcw1T"][mc][:, oc * 128:(oc + 1) * 128],
                                       pooled[mc][:],
                                       start=(mc == 0), stop=(mc == 3))
                  nc.scalar.activation(c1_sb[oc][:], c1_p[:], AF.Relu,
                                       bias=sev["fcb1"][oc][:],
                                       scale=sev["fcs1"][oc][:])
              # fcb2 already carries seb2 (folded on host)
              cbb = [pwork.tile([128, 1], F32, tag=f"cbb{i}", name=f"cbb{i}")
                     for i in range(4)]
              for mc in range(4):
                  c2_p = pse.tile([128, 1], F32, tag="c2p", name="c2p")
                  for kc in range(2):
                      nc.tensor.matmul(c2_p[:],
                                       sew["fcw2T"][kc][:, mc * 128:(mc + 1) * 128],
                                       c1_sb[kc][:],
                                       start=(kc == 0), stop=(kc == 1))
                  nc.scalar.activation(cbb[mc][:], c2_p[:], AF.Identity,
                                       bias=sev["fcb2"][mc][:],
                                       scale=sev["fcs2"][mc][:])
              s1_sb = [pwork.tile([128, EE], BF16, tag=f"s1_{i}", name=f"s1_{i}")
                       for i in range(2)]
              for oc in range(2):
                  s1_p = pse.tile([128, EE], F32, tag="s1p", name="s1p", bufs=2)
                  for mc in range(4):
                      nc.tensor.matmul(s1_p[:],
                                       sew["fsw1T"][mc][:, oc * 128:(oc + 1) * 128],
                                       fmap[mc][:], start=(mc == 0), stop=(mc == 3))
                  nc.scalar.activation(s1_sb[oc][:], s1_p[:], AF.Relu,
                                       bias=sev["seb1"][oc][:],
                                       scale=sev["ses1"][oc][:])
              for mc in range(4):
                  s2_p = pse.tile([128, EE], F32, tag="s2p", name="s2p", bufs=2)
                  for kc in range(2):
                      nc.tensor.matmul(s2_p[:],
                                       sew["fsw2T"][kc][:, mc * 128:(mc + 1) * 128],
                                       s1_sb[kc][:], start=(kc == 0), stop=(kc == 1))
                  sig = pwork.tile([128, EE], BF16, tag="sig", name="sig",
                                   bufs=2)
                  nc.scalar.activation(sig[:], s2_p[:], AF.Sigmoid,
                                       bias=cbb[mc][:], scale=sev["ses2"][mc][:])
                  for ee, lo, hi in ((nc.vector, 0, 11), (nc.gpsimd, 11, E)):
                      outv = fusedp[mc][:].rearrange(
                          "p (i j) -> p i j", j=26)[:, 2 + lo:2 + hi, 2:24]
                      ee.tensor_tensor(
                          out=outv,
                          in0=fmap[mc][:].rearrange("p (i j) -> p i j",
                                                    i=E)[:, lo:hi],
                          in1=sig[:].rearrange("p (i j) -> p i j",
                                               i=E)[:, lo:hi],
                          op=ALU.mult)

        if stages >= 4:
          # ================= stage 4: conv stack =================
          # Row-split pipeline: each conv computes its top (rows 0:11) and
          # bottom (rows 11:22) output halves separately; a half is relu'd
          # and AllGather'd while the next half / next conv keeps the PE
          # busy. Gathered halves land directly in zero-padded 26x26 tiles
          # in fixed rank order (weight chunks are packed in the same rank
          # order), so no masked combines are needed.
          # Row slices (0:8, 8:13, 13:22): the next conv's TOP outputs
          # (rows 0:11) only need input rows <= 12, i.e. the first two
          # slices, so they fully hide the third slice's exchange latency.
          SLICES = [(0, 8), (8, 13), (13, 17), (17, 22)]
          RH = 11 * 22

          def tap_rows(padt, tap, r0, nr):
              dy, dx = tap // 5, tap % 5
              return padt.rearrange("p (i j) -> p i j", j=26)[
                  :, dy + r0:dy + r0 + nr, dx:dx + 22]

          def rd_pair(gpc, r0, nr):
              # interior rows r0:r0+nr of both packed padded images
              return gpc[:].rearrange("p (c i j) -> p c i j", c=2, j=26)[
                  :, :, 2 + r0:2 + r0 + nr, 2:24]

          with tc.tile_pool(name="pcw", bufs=1) as pcw:
              psc = pse
              w2 = []
              for kc in range(2):
                  t = pcw.tile([128, 25 * 128], BF16, tag=f"w2_{kc}",
                               name=f"w2_{kc}")
                  for ch in range(2):
                      nc.gpsimd.dma_start(t[:, ch * 1600:(ch + 1) * 1600],
                                          w2sb_d[kc][:, ch * 1600:(ch + 1) * 1600])
                  w2.append(t)
              w3 = []
              for kc in range(2):
                  t = pcw.tile([128, 25 * 256], BF16, tag=f"w3_{kc}",
                               name=f"w3_{kc}")
                  for ch in range(4):
                      nc.gpsimd.dma_start(t[:, ch * 1600:(ch + 1) * 1600],
                                          w3sb_d[kc][:, ch * 1600:(ch + 1) * 1600])
                  w3.append(t)

              def exchange_slice(stage_sb, dram_pre, gpc, slices, sl_i):
                  """Relu'd slice -> DRAM -> AllGather over the pair -> both
                  packed padded tiles via one 4D-AP read, in fixed rank
                  order. Solo emulates the gather with two direct writes."""
                  r0, r1_ = slices[sl_i]
                  nr = r1_ - r0
                  seg = stage_sb[:, r0 * 22:r1_ * 22]
                  gseg = pdram.tile([256, nr * 22], BF16,
                                    tag=f"{dram_pre}g{sl_i}",
                                    name=f"{dram_pre}g{sl_i}")
                  if solo:
                      nc.sync.dma_start(gseg[0:128, :], seg)
                      nc.sync.dma_start(gseg[128:256, :], seg)
                  else:
                      bseg = pdram.tile([128, nr * 22], BF16,
                                        tag=f"{dram_pre}b{sl_i}",
                                        name=f"{dram_pre}b{sl_i}")
                      nc.sync.dma_start(bseg[:], seg)
                      nc.gpsimd.collective_compute(
                          "AllGather", ALU.bypass, replica_groups=groups,
                          ins=[bseg[:].opt()], outs=[gseg[:].opt()])
                  gv = gpc[:].rearrange("p (c i j) -> p c i j", c=2, j=26)
                  nc.scalar.dma_start(gv[:, 0, 2 + r0:2 + r0 + nr, 2:24],
                                      gseg[0:128, :])
                  nc.gpsimd.dma_start(gv[:, 1, 2 + r0:2 + r0 + nr, 2:24],
                                      gseg[128:256, :])

              def conv_sliced(wsel, srcs, nkc, stage_sb, bias, dram_pre,
                              gpc, slices):
                  """One conv layer: compute the row slices, relu each into
                  stage_sb and exchange it as soon as it's ready."""
                  for sl_i, (r0, r1_) in enumerate(slices):
                      nr = r1_ - r0
                      cp = psc.tile([128, RH], F32, tag="cp", name="cp",
                                    bufs=2)
                      cpv = cp[:, 0:nr * 22]
                      k = 0
                      for kc in range(nkc):
                          for tap in range(25):
                              nc.tensor.matmul(
                                  cpv, wsel(kc, tap),
                                  tap_rows(srcs[kc], tap, r0, nr),
                                  start=(k == 0), stop=(k == 25 * nkc - 1))
                              k += 1
                      nc.scalar.activation(stage_sb[:, r0 * 22:r1_ * 22], cpv,
                                           AF.Relu, bias=bias)
                      exchange_slice(stage_sb, dram_pre, gpc, slices, sl_i)

              # ---- conv1: fusedp -> 128 out-ch (my half) ----
              r1s = pcw.tile([128, EE], BF16, tag="r1s", name="r1s")
              conv_sliced(
                  lambda kc, tap: w1[kc][:, tap * 128:(tap + 1) * 128],
                  [t[:] for t in fusedp], 4, r1s, b1h[:], "r1", g1pc,
                  [(0, 8), (8, 13), (13, 17), (17, 22)])

              # ---- conv2: g1p -> 128 out-ch (my half) ----
              r2s = pcw.tile([128, EE], BF16, tag="r2s", name="r2s")
              conv_sliced(
                  lambda kc, tap: w2[kc][:, tap * 128:(tap + 1) * 128],
                  g1p, 2, r2s, b2h[:], "r2", g2pc,
                  [(0, 10), (10, 13), (13, 22)])

              # ---- conv3: g2p -> 256 out-ch (my half), two half-rows per
              # out chunk; both top chunks first (they only need conv2's
              # first two slices), hiding the last conv2 exchange ----
              for (oc, hh) in ((0, 0), (1, 0), (0, 1), (1, 1)):
                  # the final chunk runs as two independent PSUM chains so
                  # the first half's relu+output DMA overlaps the second's
                  last = (oc == 1 and hh == 1)
                  rows = [(0, 6), (6, 9), (9, 11)] if last else [(0, 11)]
                  for ri, (ra, rb) in enumerate(rows):
                      nr = rb - ra
                      cp = psc.tile([128, RH], F32, tag="cp", name="cp",
                                    bufs=2)
                      cpv = cp[:, 0:nr * 22]
                      order = ([t for t in range(25) if t // 5 <= 1] +
                               [t for t in range(25) if t // 5 > 1]) \
                          if hh == 0 else list(range(25))
                      k = 0
                      for tap in order:
                          for kc in range(2):
                              nc.tensor.matmul(
                                  cpv,
                                  w3[kc][:, tap * 256 + oc * 128:
                                         tap * 256 + (oc + 1) * 128],
                                  tap_rows(g2p[kc], tap, hh * 11 + ra, nr),
                                  start=(k == 0), stop=(k == 49))
                              k += 1
                      o_sb = pcw.tile([128, RH], F32, tag="osb",
                                      name="osb", bufs=3)
                      ov = o_sb[:, 0:nr * 22]
                      nc.scalar.activation(ov, cpv, AF.Relu, bias=b3h[oc][:])
                      eng = nc.sync if (oc + hh + ri) % 2 == 0 else nc.scalar
                      eng.dma_start(
                          out_d[oc * 128:(oc + 1) * 128,
                                hh * RH + ra * 22:hh * RH + rb * 22], ov)

        if stages >= 3:
            pse_cm.__exit__(None, None, None)

    nc.compile()
    return nc


_NC_CACHE = None


def _get_program():
    global _NC_CACHE
    if _NC_CACHE is None:
        _NC_CACHE = build_program()
    return _NC_CACHE


def _bf(a):
    return np.ascontiguousarray(a.astype(ml_dtypes.bfloat16))


def _prep_shared(w):
    """Packed weights/constants identical on every core."""
    ADJ = _build_adj()
    out = {}
    constb = np.zeros((128, _CB), np.float32)

    def put(nm, arr):
        c0, cols = _LAY_B[nm]
        r, cc = arr.shape
        constb[0:r, c0:c0 + cc] = arr
    wt = w['W_trans']
    for kc in range(6):
        put(f"wtr{kc}", wt[kc * 128:(kc + 1) * 128])
    put("brow", w['b_trans'].reshape(1, EMB))
    put("onesrow", np.ones((1, 128), np.float32))
    put("onescol", np.ones((128, 1), np.float32))
    g2T = np.zeros((EM, E), np.float32)
    for e in range(E):
        g2T[e * M:(e + 1) * M, e] = 1.0
    put("g2T", g2T)
    sumT = np.kron(np.eye(L, dtype=np.float32), np.ones((SPAN, 1), np.float32))
    for kc in range(4):
        put(f"sumT{kc}", sumT[kc * 128:(kc + 1) * 128])
    for nm, arr, nch in (("fsw1T", w['fs_w1'].T, 4), ("fcw1T", w['fc_w1'].T, 4),
                         ("fsw2T", w['fs_w2'].T, 2), ("fcw2T", w['fc_w2'].T, 2)):
        for kc in range(nch):
            put(f"{nm}{kc}", np.ascontiguousarray(arr[kc * 128:(kc + 1) * 128]))
    out['constb'] = _bf(constb)

    gT = np.zeros((EMH, E), np.float32)
    for e in range(E):
        gT[e * M * H:(e + 1) * M * H, e] = 1.0 / (M * H)
    gTb = np.zeros((128, 9 * E), np.float32)
    for kc in range(9):
        r = min(128, EMH - kc * 128)
        gTb[0:r, kc * E:(kc + 1) * E] = gT[kc * 128:kc * 128 + r]
    out['gTb'] = np.ascontiguousarray(gTb.astype(ml_dtypes.float8_e4m3))
    out['aallTb'] = _bf(np.concatenate(
        [ADJ[r].T for r in range(NREL)] + [np.eye(NN, dtype=np.float32)],
        axis=1))
    out['tfb'] = _bf(np.ascontiguousarray(w['type_embed'][_TYPES]))
    out['identb'] = _bf(np.eye(128, dtype=np.float32))

    constf = np.zeros((128, _CF), np.float32)

    def putf(nm, arr):
        c0, cols = _LAY_F[nm]
        constf[0:arr.shape[0], c0:c0 + 1] = arr.reshape(-1, 1)
    vecs = {"ses1": w['fs_g1'], "seb1": w['fs_b1'] * w['fs_g1'] + w['fs_be1'],
            "fcs1": w['fc_g1'], "fcb1": w['fc_b1'] * w['fc_g1'] + w['fc_be1'],
            "ses2": w['fs_g2'], "seb2": w['fs_b2'] * w['fs_g2'] + w['fs_be2'],
            "fcs2": w['fc_g2'],
            "fcb2": w['fc_b2'] * w['fc_g2'] + w['fc_be2'] +
                    w['fs_b2'] * w['fs_g2'] + w['fs_be2']}
    for nm, v in vecs.items():
        nch = 2 if v.shape[0] == INTER else 4
        for kc in range(nch):
            putf(f"{nm}{kc}", v[kc * 128:(kc + 1) * 128])
    out['constf_base'] = constf

    for layer in range(NLAYERS):
        din_l = D0 if layer == 0 else EMB
        kcs = _KC0 if layer == 0 else _KC1
        nk = len(kcs)
        Wst = w['rgcn_Wrel0'].reshape(NREL * D0, EMB) if layer == 0 else \
            w['rgcn_Wrel'][layer - 1].reshape(NREL * EMB, EMB)
        Wself = w['rgcn_Wself0'] if layer == 0 else w['rgcn_Wself'][layer - 1]
        p = np.zeros((128, (NREL + 1) * nk * EMB), np.float32)
        for si, (s0, sl) in enumerate(kcs):
            for r in range(NREL):
                b = si * (NREL + 1) + r
                p[0:sl, b * EMB:(b + 1) * EMB] = \
                    Wst[r * din_l + s0:r * din_l + s0 + sl]
            b = si * (NREL + 1) + NREL
            p[0:sl, b * EMB:(b + 1) * EMB] = Wself[s0:s0 + sl]
        out[f'wstp{layer}'] = _bf(p)
    return out


def _prep_conv_half(w, half, constf_base):
    out = {}
    w1 = w['cr_w1'][half * 128:(half + 1) * 128]
    out['w1sb'] = _bf(np.ascontiguousarray(
        w1.transpose(1, 2, 3, 0).reshape(4, 128, 25 * 128)))
    # conv2/conv3 weight chunks in natural (rank-ordered) input-half order
    w2 = w['cr_w2'][half * 128:(half + 1) * 128]
    out['w2sb'] = _bf(np.ascontiguousarray(
        w2.transpose(1, 2, 3, 0).reshape(2, 128, 25 * 128)))
    w3 = w['cr_w3'][half * 256:(half + 1) * 256]
    out['w3sb'] = _bf(np.ascontiguousarray(
        w3.transpose(1, 2, 3, 0).reshape(2, 128, 25 * 256)))
    constf = constf_base.copy()

    def putf(nm, arr):
        c0, cols = _LAY_F[nm]
        constf[0:arr.shape[0], c0:c0 + 1] = arr.reshape(-1, 1)
    putf("b1h", w['cr_b1'][half * 128:(half + 1) * 128])
    putf("b2h", w['cr_b2'][half * 128:(half + 1) * 128])
    putf("b3h0", w['cr_b3'][half * 256:half * 256 + 128])
    putf("b3h1", w['cr_b3'][half * 256 + 128:half * 256 + 256])
    putf("mtop", np.full(128, float(half), np.float32))
    putf("mbot", np.full(128, float(1 - half), np.float32))
    c0, cols = _LAY_F["identf"]
    constf[:, c0:c0 + 128] = np.eye(128, dtype=np.float32)
    out['constf'] = constf
    return out


def _prep_doc(x, att, mi, ls):
    out = {}
    mif = mi.reshape(EM)
    attm = np.ascontiguousarray(
        att[:, mif, :].transpose(1, 0, 2).reshape(EMH, C))
    amp = np.zeros((128, 9 * C), np.float32)
    for kc in range(9):
        r = min(128, EMH - kc * 128)
        amp[0:r, kc * C:kc * C + C] = attm[kc * 128:kc * 128 + r]
    out['amp'] = np.ascontiguousarray(amp.astype(ml_dtypes.float8_e4m3))
    idx = ls[:, None] + np.arange(SPAN)
    idxf = idx.reshape(LS)
    rows = att[:, idxf, :].reshape(H, L, SPAN, C)
    blocks = np.take_along_axis(rows, idx[None, :, None, :], axis=3)
    attl = blocks.transpose(0, 2, 1, 3).reshape(HS, LS)
    xmT = x[mif].T
    xspT = x[idxf].T
    actb = np.zeros((128, _CA), np.float32)

    def put(nm, arr):
        c0, cols = _LAY_A[nm]
        actb[0:arr.shape[0], c0:c0 + arr.shape[1]] = arr
    for kc in range(6):
        put(f"xmT{kc}", xmT[kc * 128:(kc + 1) * 128])
        put(f"xspT{kc}", xspT[kc * 128:(kc + 1) * 128])
    for kc in range(3):
        put(f"attl{kc}", attl[kc * 128:(kc + 1) * 128])
    out['actb'] = _bf(actb)
    xpk = np.zeros((128, 8 * HID), np.float32)
    for kc in range(8):
        xpk[:, kc * HID:(kc + 1) * HID] = x[kc * 128:(kc + 1) * 128]
    out['xp'] = _bf(xpk)
    return out


def build_in_maps(inputs):
    w = {}
    for k, v in inputs.items():
        a = np.asarray(v)
        w[k] = a if a.dtype in (np.int32, np.int64) else \
            np.asarray(a, np.float32)
    shared = _prep_shared(w)
    constf_base = shared.pop('constf_base')
    halves = [_prep_conv_half(w, h, constf_base) for h in range(2)]
    seq = np.asarray(inputs['sequence_output'], np.float32)
    att = np.asarray(inputs['attention'], np.float32)
    mi = np.asarray(inputs['mention_idx']).astype(np.int64)
    ls = np.asarray(inputs['link_start']).astype(np.int64)
    docs = [_prep_doc(seq[n], att[n], mi[n], ls[n]) for n in range(NB)]
    in_maps = []
    for core in range(N_CORES):
        n, half = core // 2, core % 2
        m = dict(shared)
        m.update(halves[half])
        m.update(docs[n])
        in_maps.append({k: (np.ascontiguousarray(v)
                            if v.dtype in (ml_dtypes.bfloat16,
                                           ml_dtypes.float8_e4m3)
                            else np.ascontiguousarray(v, np.float32))
                        for k, v in m.items()})
    return in_maps


def kernel(**inputs):
    nc = _get_program()
    in_maps = build_in_maps(inputs)
    res = run_bass_kernel_spmd(nc, in_maps, list(range(N_CORES)))
    out = np.zeros((NB, EMB, E, E), np.float32)
    for core in range(N_CORES):
        n, half = core // 2, core % 2
        out[n, half * 256:(half + 1) * 256] = \
            res.results[core]["out"].reshape(256, E, E)
    return out
